# revision 1
# baseline (speedup 1.0000x reference)
"""TGN-style GNN message passing + community detection on 8 TRN2 NeuronCores.

Node-sharded SPMD: nodes padded to 8*L and sharded contiguously; events
routed by host (index work only) to the owner core of their update target
and binned into 128-node windows (2x128 slots per window). Segment-mean via
inv-cnt-scaled one-hot matmuls on the PE; GRU/proj/sim as bf16 matmuls;
sparsemax taus via secant iterations on g(tau)=sum(relu(z-tau)) with an
AllGathered chunk-max warm start for the centroid direction; c_memory
partials AllReduced. All float arithmetic on device.
"""

import os
from contextlib import ExitStack

import numpy as np
import ml_dtypes

import concourse.bass as bass
import concourse.mybir as mybir
import concourse.tile as tile
from concourse.bass_utils import run_bass_kernel_spmd
from concourse.masks import make_identity

FP32 = mybir.dt.float32
BF16 = mybir.dt.bfloat16
AF = mybir.ActivationFunctionType
ALU = mybir.AluOpType
AX = mybir.AxisListType

NCORES = 8
D = 128
F = 128
T = 128
P = 128
C = 256
HALF_PI = float(np.pi / 2)

bfc = lambda x: np.ascontiguousarray(np.asarray(x).astype(ml_dtypes.bfloat16))
f32c = lambda x: np.ascontiguousarray(np.asarray(x).astype(np.float32))


def _bcast_row(dram_tensor, ncols, nparts=128, off=0):
    row = dram_tensor.ap()
    return bass.AP(tensor=row.tensor, offset=row.offset + off,
                   ap=[[0, nparts], [1, ncols]])


def split_waits(nc, sp_limit=1, default_limit=1):
    """This env's walrus rejects >1 sync-wait on SP CTRL instructions:
    move extra waits onto preceding NOPs."""
    limits = {mybir.EngineType.SP: sp_limit}
    for fn in nc.m.functions:
        for bb in fn.blocks:
            out = []
            for ins in bb.instructions:
                si = ins.sync_info
                w = list(si.on_wait) if (si is not None and si.on_wait) else []
                lim = limits.get(ins.engine, default_limit)
                if len(w) > lim:
                    extra, keep = w[:-lim], w[-lim:]
                    for j in range(0, len(extra), lim):
                        out.append(mybir.InstNoOp(
                            name=f"{ins.name}-ws{j}",
                            engine=ins.engine,
                            sync_info=mybir.SyncInfo(
                                on_wait=list(extra[j:j + lim]), on_update=[]),
                        ))
                    ins.sync_info = mybir.SyncInfo(
                        on_wait=list(keep),
                        on_update=list(si.on_update) if si.on_update else [])
                out.append(ins)
            bb.instructions = out
    return nc


def build_program(L, NIT_NC=9, NIT_MINI=16, NIT_GLB=13, debug=False):
    NW = L // 128
    SLOTS = 2 * L
    MGW = NW * NCORES
    # node batches of <=256 (SBUF headroom), multiples of 128
    batches = []
    off = 0
    while off < L:
        bs_ = min(256, L - off)
        batches.append((off, bs_))
        off += bs_

    nc = bass.Bass(num_devices=NCORES)

    memT = nc.dram_tensor("memT", [128, L], FP32, kind="ExternalInput")
    mem_node = nc.dram_tensor("mem_node", [L, D], FP32, kind="ExternalInput")
    nfT = nc.dram_tensor("nfT", [128, L], FP32, kind="ExternalInput")
    has_colT = nc.dram_tensor("has_colT", [128, NW], FP32, kind="ExternalInput")
    ev_mo = nc.dram_tensor("ev_mo", [SLOTS, D], BF16, kind="ExternalInput")
    ev_ef = nc.dram_tensor("ev_ef", [SLOTS, F], BF16, kind="ExternalInput")
    ev_dt = nc.dram_tensor("ev_dt", [SLOTS], FP32, kind="ExternalInput")
    ev_col = nc.dram_tensor("ev_col", [SLOTS], FP32, kind="ExternalInput")
    ev_icnt = nc.dram_tensor("ev_icnt", [SLOTS], FP32, kind="ExternalInput")
    W_ihT = nc.dram_tensor("W_ihT", [128, 4, 384], BF16, kind="ExternalInput")
    W_hhT = nc.dram_tensor("W_hhT", [128, 384], BF16, kind="ExternalInput")
    bsum = nc.dram_tensor("bsum", [128, 2], FP32, kind="ExternalInput")
    b_hh2 = nc.dram_tensor("b_hh2", [128, 1], FP32, kind="ExternalInput")
    b_ih2 = nc.dram_tensor("b_ih2", [128, 1], FP32, kind="ExternalInput")
    pWt = nc.dram_tensor("pWt", [128, P], BF16, kind="ExternalInput")
    pb = nc.dram_tensor("pb", [128, 1], FP32, kind="ExternalInput")
    cenT = nc.dram_tensor("cenT", [128, C], FP32, kind="ExternalInput")
    w_rep = nc.dram_tensor("w_rep", [128, T], FP32, kind="ExternalInput")
    bpi_rep = nc.dram_tensor("bpi_rep", [128, T], FP32, kind="ExternalInput")
    iota_t = nc.dram_tensor("iota_t", [128, 128], FP32, kind="ExternalInput")

    emb_out = nc.dram_tensor("emb", [L, D], FP32, kind="ExternalOutput")
    dbg = {}
    if debug:
        dbg['newmem'] = nc.dram_tensor("dbg_newmem", [L, D], FP32, kind="ExternalOutput")
        dbg['simT'] = nc.dram_tensor("dbg_simT", [128, 2, L], BF16, kind="ExternalOutput")
        dbg['taunc'] = nc.dram_tensor("dbg_taunc", [128, NW], FP32, kind="ExternalOutput")
        dbg['taucn'] = nc.dram_tensor("dbg_taucn", [128, 2], FP32, kind="ExternalOutput")
        dbg['cmem'] = nc.dram_tensor("dbg_cmem", [C, D], FP32, kind="ExternalOutput")
        dbg['aggT'] = nc.dram_tensor("dbg_aggT", [128, 3, L], BF16, kind="ExternalOutput")
        dbg['cg1'] = nc.dram_tensor("dbg_cg1", [128, 4], FP32, kind="ExternalOutput")
        dbg['stg1'] = nc.dram_tensor("dbg_stg1", [128, 4], FP32, kind="ExternalOutput")
        dbg['ct1'] = nc.dram_tensor("dbg_ct1", [128, 4], FP32, kind="ExternalOutput")

    newmem_dram = nc.dram_tensor("newmem_dram", [L, D], FP32)
    aggT_dram = nc.dram_tensor("aggT_dram", [128, 3, L], BF16)
    taunc_dram = nc.dram_tensor("taunc_dram", [NW, 128], BF16)
    rnorm_dram = nc.dram_tensor("rnorm_dram", [NW, 128], BF16)
    ssq_dram = nc.dram_tensor("ssq_dram", [1, L], FP32)
    crec_dram = nc.dram_tensor("crec_dram", [1, C], BF16)
    taucn_dram = nc.dram_tensor("taucn_dram", [2, 128], BF16)
    mg_local = nc.dram_tensor("mg_local", [NCORES, 2, 128, NW], FP32)
    mg_all = nc.dram_tensor("mg_all", [NCORES, 2, 128, NW], FP32, addr_space="Shared")
    st_l = [nc.dram_tensor(f"st_l{i}", [128, 4], FP32) for i in range(NIT_GLB)]
    st_a = [nc.dram_tensor(f"st_a{i}", [128, 4], FP32, addr_space="Shared")
            for i in range(NIT_GLB)]
    st_lm = nc.dram_tensor("st_lm", [128, 4], FP32)
    st_am = nc.dram_tensor("st_am", [128, 4], FP32, addr_space="Shared")
    cm_local = nc.dram_tensor("cm_local", [C, D], FP32)
    cm_all = nc.dram_tensor("cm_all", [C, D], FP32, addr_space="Shared")
    core_oh_in = nc.dram_tensor("core_oh_in", [128, NCORES], FP32, kind="ExternalInput")
    RG = [list(range(NCORES))]

    cc_sem = nc.alloc_semaphore("cc_done")
    ccv = [0]
    ctx = ExitStack()
    with tile.TileContext(nc) as tc, ctx:
        const = ctx.enter_context(tc.tile_pool(name="const", bufs=1))
        late = ctx.enter_context(tc.tile_pool(name="late", bufs=1))
        wk = ctx.enter_context(tc.tile_pool(name="wk", bufs=2))
        scr = ctx.enter_context(tc.tile_pool(name="scr", bufs=1))
        # PSUM: psA bufs=1 {acc3: 3 banks, g1: 1, cmacc: 1}; psB bufs=2 {b1: 2}
        psA = ctx.enter_context(tc.tile_pool(name="psA", bufs=1, space="PSUM"))
        psB = ctx.enter_context(tc.tile_pool(name="psB", bufs=2, space="PSUM"))

        # ----- constants -----
        ident = const.tile([128, 128], BF16)
        make_identity(nc, ident)
        identf = const.tile([128, 128], FP32)
        make_identity(nc, identf)
        iota = const.tile([128, 128], FP32)
        nc.sync.dma_start(out=iota, in_=iota_t[:, :])
        wih = const.tile([128, 4, 384], BF16)
        nc.sync.dma_start(out=wih, in_=W_ihT[:, :, :])
        whh = const.tile([128, 384], BF16)
        nc.sync.dma_start(out=whh, in_=W_hhT[:, :])
        bs = const.tile([128, 2], FP32)
        nc.sync.dma_start(out=bs, in_=bsum[:, :])
        bh2 = const.tile([128, 1], FP32)
        nc.sync.dma_start(out=bh2, in_=b_hh2[:, :])
        bi2 = const.tile([128, 1], FP32)
        nc.sync.dma_start(out=bi2, in_=b_ih2[:, :])
        pw = const.tile([128, P], BF16)
        nc.sync.dma_start(out=pw, in_=pWt[:, :])
        pbt = const.tile([128, 1], FP32)
        nc.sync.dma_start(out=pbt, in_=pb[:, :])
        wr = const.tile([128, T], FP32)
        nc.sync.dma_start(out=wr, in_=w_rep[:, :])
        br = const.tile([128, T], FP32)
        nc.sync.dma_start(out=br, in_=bpi_rep[:, :])
        hascol = const.tile([128, NW], FP32)
        nc.sync.dma_start(out=hascol, in_=has_colT[:, :])
        ones_col = const.tile([128, 1], BF16)
        nc.vector.memset(ones_col, 1.0)

        # centroid norms
        cen = const.tile([128, C], FP32)
        nc.sync.dma_start(out=cen, in_=cenT[:, :])
        censq = wk.tile([128, C], BF16, tag="censq")
        nc.vector.tensor_mul(censq, cen, cen)
        cnorm = wk.tile([1, C], FP32, tag="cnorm")
        ps_c = psB.tile([1, C], FP32, tag="b1")
        nc.tensor.matmul(ps_c, ones_col, censq, start=True, stop=True)
        nc.scalar.activation(cnorm, ps_c, AF.Sqrt)
        nc.vector.tensor_scalar_add(cnorm, cnorm, 1e-8)
        crec = wk.tile([1, C], FP32, tag="crec")
        nc.vector.reciprocal(crec, cnorm)
        crec_b = wk.tile([1, C], BF16, tag="crec_b")
        nc.vector.tensor_copy(crec_b, crec)
        nc.sync.dma_start(out=crec_dram[:, :], in_=crec_b)
        crec_rep = const.tile([128, C], BF16)
        nc.sync.dma_start(out=crec_rep, in_=_bcast_row(crec_dram, C))
        cennT = const.tile([128, C], BF16)
        nc.vector.tensor_mul(cennT, cen, crec_rep)

        # ----- long-lived tensors -----
        simT = late.tile([128, 2, L], BF16)
        nmb = late.tile([128, NW, 128], BF16)
        tau = late.tile([128, NW], FP32)
        tau_p = late.tile([128, NW], FP32)
        g_c = late.tile([128, NW], FP32)
        g_p = late.tile([128, NW], FP32)
        ctau = late.tile([128, 2], FP32)
        ctau_p = late.tile([128, 2], FP32)
        cg = late.tile([128, 2], FP32)
        cg_p = late.tile([128, 2], FP32)

        with tc.tile_pool(name="mid", bufs=1) as mid:
            featT = mid.tile([128, L], BF16)

            with tc.tile_pool(name="early", bufs=2) as early, \
                    tc.tile_pool(name="evp", bufs=1) as evp:

                # ===== phase 1: events -> aggT (staged to DRAM) =====
                GW = 7
                assert NW % GW == 0
                for grp in range(NW // GW):
                    w0 = grp * GW
                    xg = evp.tile([128, GW * 2, 384], BF16, tag="xg")
                    cols = evp.tile([128, GW * 2], FP32, tag="cols")
                    icnt = evp.tile([128, GW * 2], FP32, tag="icnt")
                    dts = evp.tile([128, GW * 2], FP32, tag="dts")
                    s0 = w0 * 256
                    n_ev = GW * 256
                    nc.sync.dma_start(
                        out=xg[:, :, 0:D],
                        in_=ev_mo[s0:s0 + n_ev, :].rearrange("(t p) d -> p t d", p=128))
                    nc.sync.dma_start(
                        out=xg[:, :, D:D + F],
                        in_=ev_ef[s0:s0 + n_ev, :].rearrange("(t p) d -> p t d", p=128))
                    nc.sync.dma_start(
                        out=cols, in_=ev_col[s0:s0 + n_ev].rearrange("(t p) -> p t", p=128))
                    nc.sync.dma_start(
                        out=icnt, in_=ev_icnt[s0:s0 + n_ev].rearrange("(t p) -> p t", p=128))
                    nc.sync.dma_start(
                        out=dts, in_=ev_dt[s0:s0 + n_ev].rearrange("(t p) -> p t", p=128))
                    for t_ in range(GW * 2):
                        ang = wk.tile([128, T], FP32, tag="ang")
                        nc.vector.scalar_tensor_tensor(
                            ang, wr, dts[:, t_:t_ + 1], br, op0=ALU.mult, op1=ALU.add)
                        # range-reduce to [-pi, pi]: ang -= 2pi*round(ang/2pi)
                        mm_ = wk.tile([128, T], FP32, tag="mm_")
                        nc.vector.tensor_scalar(
                            mm_, ang, 1.0 / (2 * np.pi), 12582912.0,
                            op0=ALU.mult, op1=ALU.add)
                        nc.vector.tensor_scalar_add(mm_, mm_, -12582912.0)
                        nc.vector.scalar_tensor_tensor(
                            ang, mm_, -2 * np.pi, ang, op0=ALU.mult, op1=ALU.add)
                        nc.scalar.activation(xg[:, t_, D + F:], ang, AF.Sin)
                    for wi in range(GW):
                        w = w0 + wi
                        psws = [psA.tile([128, 128], FP32, tag=f"aggfc{fc}",
                                          name=f"psw{fc}") for fc in range(3)]
                        for t_ in range(2):
                            ti = wi * 2 + t_
                            oh = wk.tile([128, 128], BF16, tag="oh")
                            nc.vector.tensor_scalar(
                                oh, iota, cols[:, ti:ti + 1], icnt[:, ti:ti + 1],
                                op0=ALU.is_equal, op1=ALU.mult)
                            for fc in range(3):
                                nc.tensor.matmul(
                                    psws[fc],
                                    xg[:, ti, fc * 128:(fc + 1) * 128],
                                    oh, start=(t_ == 0), stop=(t_ == 1))
                        awin = early.tile([128, 3, 128], BF16, tag="awin")
                        for fc in range(3):
                            nc.vector.tensor_copy(awin[:, fc, :], psws[fc])
                        nc.sync.dma_start(
                            out=aggT_dram[:, :, w * 128:(w + 1) * 128], in_=awin)


                # ===== phase 2: GRU + newmem + featT =====
                for (boff, bsz) in batches:
                    sl = bass.ds(boff, bsz)
                    mTf = wk.tile([128, bsz], FP32, tag="mTf")
                    nc.sync.dma_start(out=mTf, in_=memT[:, sl])
                    mTb = wk.tile([128, bsz], BF16, tag="mTb")
                    nc.vector.tensor_copy(mTb, mTf)
                    agg_b = wk.tile([128, 3, bsz], BF16, tag="agg_b")
                    nc.sync.dma_start(out=agg_b, in_=aggT_dram[:, :, sl])
                    gis = [psA.tile([128, bsz], FP32, tag=f"aggfc{m}",
                                    name=f"gi{m}") for m in range(3)]
                    gh2 = psA.tile([128, bsz], FP32, tag="g1")
                    for m in range(3):
                        nc.tensor.matmul(gis[m], wih[:, 0, m * 128:(m + 1) * 128],
                                         mTb, start=True, stop=False)
                        for k in range(1, 4):
                            last = (k == 3 and m >= 2)
                            nc.tensor.matmul(gis[m],
                                             wih[:, k, m * 128:(m + 1) * 128],
                                             agg_b[:, k - 1, :], start=False,
                                             stop=last)
                        if m < 2:
                            nc.tensor.matmul(gis[m], whh[:, m * 128:(m + 1) * 128],
                                             mTb, start=False, stop=True)
                    nc.tensor.matmul(gh2, whh[:, 256:384], mTb, start=True, stop=True)
                    r = wk.tile([128, bsz], FP32, tag="r")
                    nc.scalar.activation(r, gis[0], AF.Sigmoid, bias=bs[:, 0:1])
                    z = wk.tile([128, bsz], FP32, tag="z")
                    nc.scalar.activation(z, gis[1], AF.Sigmoid, bias=bs[:, 1:2])
                    gh2s = wk.tile([128, bsz], FP32, tag="gh2s")
                    nc.vector.tensor_scalar_add(gh2s, gh2, bh2[:, 0:1])
                    u = wk.tile([128, bsz], FP32, tag="u")
                    nc.vector.tensor_mul(u, r, gh2s)
                    v = wk.tile([128, bsz], FP32, tag="v")
                    nc.vector.tensor_add(v, u, gis[2])
                    n_g = wk.tile([128, bsz], FP32, tag="n_g")
                    nc.scalar.activation(n_g, v, AF.Tanh, bias=bi2[:, 0:1])
                    dmn = wk.tile([128, bsz], FP32, tag="dmn")
                    nc.vector.tensor_sub(dmn, mTf, n_g)
                    e_ = wk.tile([128, bsz], FP32, tag="e_")
                    nc.vector.tensor_mul(e_, z, dmn)
                    updT = wk.tile([128, bsz], FP32, tag="updT")
                    nc.vector.tensor_add(updT, n_g, e_)
                    for cc in range(bsz // 128):
                        ch = boff // 128 + cc
                        pst = psB.tile([128, 128], FP32, tag="b1")
                        nc.tensor.transpose(pst, updT[:, cc * 128:(cc + 1) * 128], identf)
                        mn = wk.tile([128, 128], FP32, tag="mn")
                        nc.sync.dma_start(out=mn, in_=mem_node[ch * 128:(ch + 1) * 128, :])
                        d2 = wk.tile([128, 128], FP32, tag="d2")
                        nc.vector.tensor_sub(d2, pst, mn)
                        e2 = wk.tile([128, 128], FP32, tag="e2")
                        nc.vector.tensor_scalar_mul(e2, d2, hascol[:, ch:ch + 1])
                        nm = wk.tile([128, 128], FP32, tag="nm")
                        nc.vector.tensor_add(nm, mn, e2)
                        nc.sync.dma_start(out=newmem_dram[ch * 128:(ch + 1) * 128, :], in_=nm)
                        if debug:
                            nc.sync.dma_start(out=dbg['newmem'][ch * 128:(ch + 1) * 128, :], in_=nm)
                        pst2 = psB.tile([128, 128], FP32, tag="b1")
                        nc.tensor.transpose(pst2, nm, identf)
                        nfc = wk.tile([128, 128], FP32, tag="nfc")
                        nc.sync.dma_start(out=nfc, in_=nfT[:, ch * 128:(ch + 1) * 128])
                        nc.vector.tensor_add(featT[:, ch * 128:(ch + 1) * 128], pst2, nfc)
            # early pool (aggT) freed here

            # ===== phase 3: pf + norms + sim =====
            with tc.tile_pool(name="pfp", bufs=1) as pfp:
                pfT = pfp.tile([128, L], BF16)
                for (boff, bsz) in batches:
                    sl = bass.ds(boff, bsz)
                    psp = psB.tile([128, bsz], FP32, tag="b1")
                    nc.tensor.matmul(psp, pw, featT[:, sl], start=True, stop=True)
                    pfc = wk.tile([128, bsz], FP32, tag="pfc")
                    nc.vector.tensor_scalar_add(pfc, psp, pbt[:, 0:1])
                    nc.vector.tensor_copy(pfT[:, sl], pfc)
                    sq = wk.tile([128, bsz], BF16, tag="sq")
                    nc.vector.tensor_mul(sq, pfc, pfc)
                    ps_s = psB.tile([1, bsz], FP32, tag="b1")
                    nc.tensor.matmul(ps_s, ones_col, sq, start=True, stop=True)
                    sqe = wk.tile([1, bsz], FP32, tag="sqe")
                    nc.vector.tensor_copy(sqe, ps_s)
                    nc.sync.dma_start(out=ssq_dram[0, sl], in_=sqe)
                ssq_t = wk.tile([128, NW], FP32, tag="ssq_t")
                nc.sync.dma_start(
                    out=ssq_t,
                    in_=ssq_dram.ap().rearrange("o (w p) -> (o p) w", p=128))
                sns = wk.tile([128, NW], FP32, tag="sns")
                nc.scalar.activation(sns, ssq_t, AF.Sqrt)
                nc.vector.tensor_scalar_add(sns, sns, 1e-8)
                rn_t = wk.tile([128, NW], FP32, tag="rn_t")
                nc.vector.reciprocal(rn_t, sns)
                rn_b = wk.tile([128, NW], BF16, tag="rn_b")
                nc.vector.tensor_copy(rn_b, rn_t)
                nc.sync.dma_start(
                    out=rnorm_dram.ap().rearrange("w p -> p w"), in_=rn_b)
                for (boff, bsz) in batches:
                    sl = bass.ds(boff, bsz)
                    rn_rep = wk.tile([128, bsz], BF16, tag="rn_rep")
                    nc.sync.dma_start(out=rn_rep,
                                      in_=_bcast_row(rnorm_dram, bsz, off=boff))
                    for m in range(2):
                        ps_m = psB.tile([128, bsz], FP32, tag="b1")
                        nc.tensor.matmul(ps_m, cennT[:, m * 128:(m + 1) * 128],
                                         pfT[:, sl], start=True, stop=True)
                        nc.vector.tensor_mul(simT[:, m, sl], ps_m, rn_rep)
                if debug:
                    nc.sync.dma_start(out=dbg['simT'][:, :, :], in_=simT)
        # mid pool (featT) freed

        with tc.tile_pool(name="nodep", bufs=1) as nodep:
            sim_node = nodep.tile([128, NW, C], BF16)
            for ch in range(NW):
                for m in range(2):
                    pstr = psB.tile([128, 128], BF16, tag="b1")
                    nc.tensor.transpose(pstr, simT[:, m, ch * 128:(ch + 1) * 128], ident)
                    nc.vector.tensor_copy(sim_node[:, ch, m * 128:(m + 1) * 128], pstr)

            # ===== phase 5: nc Newton (secant) =====
            junk_n = scr.tile([128, C], BF16, tag="junk_n")
            junk_n2 = scr.tile([128, C], BF16, tag="junk_n2")
            ngt = scr.tile([128, NW], FP32, tag="ngt")
            nc.vector.tensor_reduce(tau_p, sim_node, axis=AX.X, op=ALU.max)
            nc.vector.tensor_scalar_add(tau_p, tau_p, -1.0)

            def nc_eval(tau_tile, g_tile):
                nc.vector.tensor_scalar_mul(ngt, tau_tile, -1.0)
                for ch in range(NW):
                    jt = junk_n if ch % 2 == 0 else junk_n2
                    nc.scalar.activation(
                        jt, sim_node[:, ch, :], AF.Relu,
                        bias=ngt[:, ch:ch + 1],
                        accum_out=g_tile[:, ch:ch + 1])

            nc_eval(tau_p, g_p)
            st1 = wk.tile([128, NW], FP32, tag="st1")
            nc.vector.tensor_scalar(st1, g_p, -1.0, 1.0 / 256.0,
                                    op0=ALU.add, op1=ALU.mult)
            nc.vector.tensor_add(tau, tau_p, st1)

            def secant_update(tt, tp, gg, gp, wtag, shape):
                num = wk.tile(shape, FP32, tag=wtag + "n")
                nc.vector.tensor_sub(num, tt, tp)
                gm1 = wk.tile(shape, FP32, tag=wtag + "g")
                nc.vector.tensor_scalar_add(gm1, gg, -1.0)
                nc.vector.tensor_mul(num, num, gm1)
                den = wk.tile(shape, FP32, tag=wtag + "d")
                nc.vector.tensor_sub(den, gp, gg)
                nc.vector.tensor_scalar_max(den, den, 1e-12)
                rden = wk.tile(shape, FP32, tag=wtag + "r")
                nc.vector.reciprocal(rden, den)
                nc.vector.tensor_copy(tp, tt)
                nc.vector.tensor_copy(gp, gg)
                stp = wk.tile(shape, FP32, tag=wtag + "s")
                nc.vector.tensor_mul(stp, num, rden)
                # monotone safeguard: secant from below must step in [0, 1]
                nc.vector.tensor_scalar(stp, stp, 0.0, 1.0,
                                        op0=ALU.max, op1=ALU.min)
                nc.vector.tensor_add(tt, tt, stp)

            for it in range(NIT_NC):
                nc_eval(tau, g_c)
                secant_update(tau, tau_p, g_c, g_p, "ncs", [128, NW])
            if debug:
                nc.sync.dma_start(out=dbg['taunc'][:, :], in_=tau)
            tau_b = wk.tile([128, NW], BF16, tag="tau_b")
            nc.vector.tensor_copy(tau_b, tau)
            nc.sync.dma_start(
                out=taunc_dram.ap().rearrange("w p -> p w"), in_=tau_b)

            # ===== phase 6: cn Newton (global delta-probe secant) =====
            CNC = L // 7
            junk_c = scr.tile([128, CNC], BF16, tag="junk_c")
            gparts = scr.tile([128, 7], FP32, tag="gparts")
            CN_DELTA = 1e-3

            def cn_eval4(tt, out4):
                # out4[:, 0:2] = g(tau) per m; out4[:, 2:4] = g(tau + delta)
                td = wk.tile([128, 2], FP32, tag="td")
                nc.vector.tensor_scalar_add(td, tt, CN_DELTA)
                for m in range(2):
                    for pi, tvec in ((0, tt), (2, td)):
                        for j in range(7):
                            nc.vector.tensor_scalar(
                                junk_c, simT[:, m, bass.ds(j * CNC, CNC)],
                                tvec[:, m:m + 1], 0.0,
                                op0=ALU.subtract, op1=ALU.max)
                            nc.vector.tensor_reduce(
                                gparts[:, j:j + 1], junk_c, axis=AX.X, op=ALU.add)
                        nc.vector.tensor_reduce(
                            out4[:, pi + m:pi + m + 1], gparts, axis=AX.X, op=ALU.add)

            # global row max via AllReduce(max)
            rm4 = wk.tile([128, 4], FP32, tag="rm4")
            nc.vector.tensor_reduce(rm4[:, 0:2], simT, axis=AX.X, op=ALU.max)
            nc.vector.tensor_copy(rm4[:, 2:4], rm4[:, 0:2])
            rmg = wk.tile([128, 4], FP32, tag="rmg")
            with tc.tile_critical():
                nc.gpsimd.dma_start(out=st_lm[:, :], in_=rm4).then_inc(cc_sem, 16)
                ccv[0] += 16
                nc.gpsimd.wait_ge(cc_sem, ccv[0])
                nc.gpsimd.collective_compute(
                    "AllReduce", ALU.max, replica_groups=RG,
                    ins=[st_lm.ap().opt()], outs=[st_am.ap().opt()]).then_inc(cc_sem)
                ccv[0] += 1
                nc.gpsimd.wait_ge(cc_sem, ccv[0])
                nc.gpsimd.dma_start(out=rmg, in_=st_am[:, :]).then_inc(cc_sem, 16)
                ccv[0] += 16
                nc.gpsimd.wait_ge(cc_sem, ccv[0])
            nc.vector.tensor_scalar_add(ctau, rmg[:, 0:2], -1.0)

            for it in range(NIT_GLB):
                stt2 = wk.tile([128, 4], FP32, tag=f"stt{it}", name=f"stt{it}")
                cn_eval4(ctau, stt2)
                stg2 = wk.tile([128, 4], FP32, tag=f"stg{it}", name=f"stg{it}")
                with tc.tile_critical():
                    nc.gpsimd.dma_start(out=st_l[it][:, :], in_=stt2).then_inc(cc_sem, 16)
                    ccv[0] += 16
                    nc.gpsimd.wait_ge(cc_sem, ccv[0])
                    nc.gpsimd.collective_compute(
                        "AllReduce", ALU.add, replica_groups=RG,
                        ins=[st_l[it].ap().opt()], outs=[st_a[it].ap().opt()]).then_inc(cc_sem)
                    ccv[0] += 1
                    nc.gpsimd.wait_ge(cc_sem, ccv[0])
                    nc.gpsimd.dma_start(out=stg2, in_=st_a[it][:, :]).then_inc(cc_sem, 16)
                    ccv[0] += 16
                    nc.gpsimd.wait_ge(cc_sem, ccv[0])
                if debug and it == 1:
                    nc.sync.dma_start(out=dbg['cg1'][:, :], in_=stt2)
                    nc.sync.dma_start(out=dbg['stg1'][:, :], in_=stg2)
                    ctd = wk.tile([128, 4], FP32, tag="ctd")
                    nc.vector.tensor_copy(ctd[:, 0:2], ctau)
                    nc.vector.tensor_copy(ctd[:, 2:4], rmg[:, 0:2])
                    nc.sync.dma_start(out=dbg['ct1'][:, :], in_=ctd)
                dfc = wk.tile([128, 2], FP32, tag=f"dfc{it}", name=f"dfc{it}")
                nc.vector.tensor_sub(dfc, stg2[:, 0:2], stg2[:, 2:4])
                nc.vector.tensor_scalar_max(dfc, dfc, 1e-9)
                rdf = wk.tile([128, 2], FP32, tag=f"rdf{it}", name=f"rdf{it}")
                nc.vector.reciprocal(rdf, dfc)
                gm1 = wk.tile([128, 2], FP32, tag=f"gm1_{it}", name=f"gm1_{it}")
                nc.vector.tensor_scalar_add(gm1, stg2[:, 0:2], -1.0)
                stp = wk.tile([128, 2], FP32, tag=f"stp{it}", name=f"stp{it}")
                nc.vector.tensor_mul(stp, gm1, rdf)
                nc.vector.tensor_scalar(stp, stp, CN_DELTA, None, op0=ALU.mult)
                nc.vector.tensor_scalar(stp, stp, 0.0, 1.0, op0=ALU.max, op1=ALU.min)
                nc.vector.tensor_add(ctau, ctau, stp)
            if debug:
                nc.sync.dma_start(out=dbg['taucn'][:, :], in_=ctau)

            # ===== phase 7: c_memory =====
            taucn_b = wk.tile([128, 2], BF16, tag="taucn_b")
            nc.vector.tensor_copy(taucn_b, ctau)
            nc.sync.dma_start(
                out=taucn_dram.ap().rearrange("m p -> p m"), in_=taucn_b)
            taucn_rep = const.tile([128, C], BF16)
            nc.sync.dma_start(out=taucn_rep, in_=_bcast_row(taucn_dram, C))

            ps_cms = [psA.tile([128, 128], FP32, tag=f"cmacc{m}", name=f"pscm{m}")
                      for m in range(2)]
            for ch in range(NW):
                rp = wk.tile([128, C], BF16, tag="rp")
                nc.vector.scalar_tensor_tensor(
                    rp, sim_node[:, ch, :], 0.0, taucn_rep,
                    op0=ALU.bypass, op1=ALU.subtract)
                nc.vector.tensor_scalar_max(rp, rp, 0.0)
                nmcf = wk.tile([128, 128], FP32, tag="nmcf")
                nc.sync.dma_start(out=nmcf, in_=newmem_dram[ch * 128:(ch + 1) * 128, :])
                nmc = wk.tile([128, 128], BF16, tag="nmc")
                nc.vector.tensor_copy(nmc, nmcf)
                for m in range(2):
                    nc.tensor.matmul(
                        ps_cms[m], rp[:, m * 128:(m + 1) * 128],
                        nmc, start=(ch == 0), stop=(ch == NW - 1))
            cmf = wk.tile([128, 2, 128], FP32, tag="cmf")
            for m in range(2):
                nc.vector.tensor_copy(cmf[:, m, :], ps_cms[m])
            cmgf = wk.tile([128, 2, 128], FP32, tag="cmgf")
            with tc.tile_critical():
                nc.gpsimd.dma_start(
                    out=cm_local.ap().rearrange("(m p) d -> p m d", p=128),
                    in_=cmf).then_inc(cc_sem, 16)
                ccv[0] += 16
                nc.gpsimd.wait_ge(cc_sem, ccv[0])
                nc.gpsimd.collective_compute(
                    "AllReduce", ALU.add, replica_groups=RG,
                    ins=[cm_local.ap().opt()], outs=[cm_all.ap().opt()]).then_inc(cc_sem)
                ccv[0] += 1
                nc.gpsimd.wait_ge(cc_sem, ccv[0])
                nc.gpsimd.dma_start(
                    out=cmgf,
                    in_=cm_all.ap().rearrange("(m p) d -> p m d", p=128)
                ).then_inc(cc_sem, 16)
                ccv[0] += 16
                nc.gpsimd.wait_ge(cc_sem, ccv[0])
        # nodep (sim_node) freed

        cmg = const.tile([128, 2, 128], BF16)
        nc.vector.tensor_copy(cmg, cmgf)
        if debug:
            nc.sync.dma_start(
                out=dbg['cmem'].ap().rearrange("(m p) d -> p m d", p=128),
                in_=cmgf)

        # ===== phase 8: emb =====
        with tc.tile_pool(name="embp", bufs=2) as embp:
            for ch in range(NW):
                sl = bass.ds(ch * 128, 128)
                tnc = embp.tile([128, 128], BF16, tag="tnc")
                nc.sync.dma_start(out=tnc,
                                  in_=_bcast_row(taunc_dram, 128, off=ch * 128))
                ncm = wk.tile([128, 2, 128], BF16, tag="ncm")
                for m in range(2):
                    nc.vector.scalar_tensor_tensor(
                        ncm[:, m, :], simT[:, m, sl], 0.0, tnc,
                        op0=ALU.bypass, op1=ALU.subtract)
                nc.vector.tensor_scalar_max(ncm, ncm, 0.0)
                ps_z = psB.tile([128, 128], FP32, tag="b1")
                for m in range(2):
                    nc.tensor.matmul(ps_z, ncm[:, m, :], cmg[:, m, :],
                                     start=(m == 0), stop=(m == 1))
                nmf = wk.tile([128, 128], FP32, tag="nmf")
                nc.sync.dma_start(out=nmf, in_=newmem_dram[ch * 128:(ch + 1) * 128, :])
                emb_c = wk.tile([128, 128], FP32, tag="emb_c")
                nc.vector.tensor_add(emb_c, ps_z, nmf)
                nc.sync.dma_start(out=emb_out[ch * 128:(ch + 1) * 128, :], in_=emb_c)

    split_waits(nc)
    return nc


# ----------------------------------------------------------------------------
# host side
# ----------------------------------------------------------------------------

_CACHE = {}


def _route(L, src, dst, t):
    idx = np.concatenate([src, dst]).astype(np.int64)
    other = np.concatenate([dst, src]).astype(np.int64)
    tt = np.concatenate([t, t])
    eidx = np.concatenate([np.arange(len(src)), np.arange(len(src))])
    NW = L // 128
    order = np.argsort(idx, kind='stable')
    idx_s, other_s, tt_s, eidx_s = idx[order], other[order], tt[order], eidx[order]
    owner = idx_s // L
    cores = []
    for c in range(NCORES):
        msk = owner == c
        li = idx_s[msk] - c * L
        win = li // 128
        col = li % 128
        wcount = np.bincount(win, minlength=NW)
        assert wcount.max() <= 256, f"window overflow: {wcount.max()}"
        woff = np.zeros(NW + 1, np.int64)
        woff[1:] = np.cumsum(wcount)
        within = np.arange(len(li)) - woff[win]
        slot = win * 256 + within
        cores.append(dict(slot=slot, col=col, li=li, other=other_s[msk],
                          tt=tt_s[msk], eidx=eidx_s[msk]))
    return cores


def kernel(**inputs):
    node_memory = np.asarray(inputs['node_memory'])
    last_update = np.asarray(inputs['last_update'])
    node_features = np.asarray(inputs['node_features'])
    event_feat = np.asarray(inputs['event_feat'])
    t = np.asarray(inputs['t'])
    src = np.asarray(inputs['src']).astype(np.int64)
    dst = np.asarray(inputs['dst']).astype(np.int64)
    time_w = np.asarray(inputs['time_w'])
    time_b = np.asarray(inputs['time_b'])
    W_ih = np.asarray(inputs['W_ih'])
    b_ih = np.asarray(inputs['b_ih'])
    W_hh = np.asarray(inputs['W_hh'])
    b_hh = np.asarray(inputs['b_hh'])
    proj_W = np.asarray(inputs['proj_W'])
    proj_b = np.asarray(inputs['proj_b'])
    centroids = np.asarray(inputs['centroids'])

    Nn = node_memory.shape[0]
    GW = 7
    gran = 128 * GW * NCORES          # L must be multiple of 128*GW
    NP = -(-Nn // gran) * gran
    L = NP // NCORES
    SLOTS = 2 * L
    NW = L // 128

    nmp = np.zeros((NP, D), np.float32); nmp[:Nn] = node_memory
    nfp = np.zeros((NP, D), np.float32); nfp[:Nn] = node_features
    lup = np.zeros(NP, np.float32); lup[:Nn] = last_update

    idx_full = np.concatenate([src, dst])
    cnt_full = np.bincount(idx_full, minlength=NP).astype(np.float32)
    icnt_full = 1.0 / np.maximum(cnt_full, 1.0)
    has_full = (cnt_full > 0).astype(np.float32)

    cores = _route(L, src, dst, t)
    bsum_h = f32c(np.stack([(b_ih + b_hh)[0:128], (b_ih + b_hh)[128:256]], 1))
    wih_h = bfc(W_ih.T.reshape(4, 128, 384).transpose(1, 0, 2))

    in_maps = []
    for c in range(NCORES):
        r = cores[c]
        sl = r['slot']
        ev_mo = np.zeros((SLOTS, D), ml_dtypes.bfloat16)
        ev_ef = np.zeros((SLOTS, F), ml_dtypes.bfloat16)
        ev_dt = np.zeros(SLOTS, np.float32)
        ev_col = np.full(SLOTS, -1.0, np.float32)
        ev_icnt = np.zeros(SLOTS, np.float32)
        ev_mo[sl] = nmp[r['other']].astype(ml_dtypes.bfloat16)
        ev_ef[sl] = event_feat[r['eidx']].astype(ml_dtypes.bfloat16)
        ev_dt[sl] = r['tt'] - lup[r['li'] + c * L]
        ev_col[sl] = r['col'].astype(np.float32)
        ev_icnt[sl] = icnt_full[r['li'] + c * L]
        nsl = slice(c * L, (c + 1) * L)
        in_maps.append({
            'memT': f32c(nmp[nsl].T),
            'mem_node': f32c(nmp[nsl]),
            'nfT': f32c(nfp[nsl].T),
            'has_colT': f32c(has_full[nsl].reshape(NW, 128).T),
            'ev_mo': ev_mo, 'ev_ef': ev_ef, 'ev_dt': ev_dt,
            'ev_col': ev_col, 'ev_icnt': ev_icnt,
            'W_ihT': wih_h,
            'W_hhT': bfc(W_hh.T),
            'bsum': bsum_h,
            'b_hh2': f32c(b_hh[256:384].reshape(128, 1)),
            'b_ih2': f32c(b_ih[256:384].reshape(128, 1)),
            'pWt': bfc(proj_W),
            'pb': f32c(proj_b.reshape(128, 1)),
            'cenT': f32c(centroids.T),
            'w_rep': f32c(np.tile(time_w[None, :], (128, 1))),
            'bpi_rep': f32c(np.tile(time_b[None, :] + HALF_PI, (128, 1))),
            'iota_t': f32c(np.tile(np.arange(128, dtype=np.float32)[None, :],
                                   (128, 1))),
            'core_oh_in': f32c(np.tile(np.eye(NCORES, dtype=np.float32)[c][None, :],
                                       (128, 1))),
        })

    debug = bool(int(os.environ.get("KERNEL_DEBUG", "0")))
    key = (L, debug)
    if key not in _CACHE:
        _CACHE[key] = build_program(L, debug=debug)
    nc = _CACHE[key]
    trace = bool(int(os.environ.get("KERNEL_TRACE", "0")))
    res = run_bass_kernel_spmd(nc, in_maps, list(range(NCORES)), trace=trace)
    emb = np.concatenate([res.results[c]['emb'] for c in range(NCORES)], 0)
    kernel._last_exec_ns = getattr(res, 'exec_time_ns', None)
    kernel._last_profile = getattr(res, 'profile_json', None)
    if debug:
        kernel._last_results = res.results
    return emb[:Nn].astype(np.float32)



# revision 14
# speedup vs baseline: 1.3914x; 1.3914x over previous
"""TGN-style GNN message passing + community detection on 8 TRN2 NeuronCores.

Node-sharded SPMD: nodes padded to 8*L and sharded contiguously; events
routed by host (index work only) to the owner core of their update target
and binned into 128-node windows (2x128 slots per window). Segment-mean via
inv-cnt-scaled one-hot matmuls on the PE; GRU/proj/sim as bf16 matmuls;
sparsemax taus via secant iterations on g(tau)=sum(relu(z-tau)) with an
AllGathered chunk-max warm start for the centroid direction; c_memory
partials AllReduced. All float arithmetic on device.
"""

import os
from contextlib import ExitStack

import numpy as np
import ml_dtypes

import concourse.bass as bass
import concourse.mybir as mybir
import concourse.tile as tile
from concourse.bass_utils import run_bass_kernel_spmd
from concourse.masks import make_identity

FP32 = mybir.dt.float32
BF16 = mybir.dt.bfloat16
AF = mybir.ActivationFunctionType
ALU = mybir.AluOpType
AX = mybir.AxisListType

NCORES = 8
D = 128
F = 128
T = 128
P = 128
C = 256
HALF_PI = float(np.pi / 2)

bfc = lambda x: np.ascontiguousarray(np.asarray(x).astype(ml_dtypes.bfloat16))
f32c = lambda x: np.ascontiguousarray(np.asarray(x).astype(np.float32))


def _bcast_row(dram_tensor, ncols, nparts=128, off=0):
    row = dram_tensor.ap()
    return bass.AP(tensor=row.tensor, offset=row.offset + off,
                   ap=[[0, nparts], [1, ncols]])


def split_waits(nc, sp_limit=1, default_limit=1):
    """This env's walrus rejects >1 sync-wait on SP CTRL instructions:
    move extra waits onto preceding NOPs."""
    limits = {mybir.EngineType.SP: sp_limit}
    for fn in nc.m.functions:
        for bb in fn.blocks:
            out = []
            for ins in bb.instructions:
                si = ins.sync_info
                w = list(si.on_wait) if (si is not None and si.on_wait) else []
                lim = limits.get(ins.engine, default_limit)
                if len(w) > lim:
                    extra, keep = w[:-lim], w[-lim:]
                    for j in range(0, len(extra), lim):
                        out.append(mybir.InstNoOp(
                            name=f"{ins.name}-ws{j}",
                            engine=ins.engine,
                            sync_info=mybir.SyncInfo(
                                on_wait=list(extra[j:j + lim]), on_update=[]),
                        ))
                    ins.sync_info = mybir.SyncInfo(
                        on_wait=list(keep),
                        on_update=list(si.on_update) if si.on_update else [])
                out.append(ins)
            bb.instructions = out
    return nc


def build_program(L, NIT_NC=8, NIT_MINI=16, NIT_GLB=7, debug=False):
    NW = L // 128
    SLOTS = 2 * L
    MGW = NW * NCORES
    # node batches of <=256 (SBUF headroom), multiples of 128
    batches = []
    off = 0
    while off < L:
        bs_ = min(256, L - off)
        batches.append((off, bs_))
        off += bs_

    nc = bass.Bass(num_devices=NCORES)

    memT = nc.dram_tensor("memT", [128, L], FP32, kind="ExternalInput")
    mem_node = nc.dram_tensor("mem_node", [L, D], FP32, kind="ExternalInput")
    nfT = nc.dram_tensor("nfT", [128, L], FP32, kind="ExternalInput")
    has_colT = nc.dram_tensor("has_colT", [128, NW], FP32, kind="ExternalInput")
    ev_mo = nc.dram_tensor("ev_mo", [SLOTS, D], BF16, kind="ExternalInput")
    ev_ef = nc.dram_tensor("ev_ef", [SLOTS, F], BF16, kind="ExternalInput")
    ev_dt = nc.dram_tensor("ev_dt", [SLOTS], FP32, kind="ExternalInput")
    ev_col = nc.dram_tensor("ev_col", [SLOTS], FP32, kind="ExternalInput")
    ev_icnt = nc.dram_tensor("ev_icnt", [SLOTS], FP32, kind="ExternalInput")
    W_ihT = nc.dram_tensor("W_ihT", [128, 4, 384], BF16, kind="ExternalInput")
    W_hhT = nc.dram_tensor("W_hhT", [128, 384], BF16, kind="ExternalInput")
    bsum = nc.dram_tensor("bsum", [128, 2], FP32, kind="ExternalInput")
    b_hh2 = nc.dram_tensor("b_hh2", [128, 1], FP32, kind="ExternalInput")
    b_ih2 = nc.dram_tensor("b_ih2", [128, 1], FP32, kind="ExternalInput")
    pWt = nc.dram_tensor("pWt", [128, P], BF16, kind="ExternalInput")
    pb = nc.dram_tensor("pb", [128, 1], FP32, kind="ExternalInput")
    cenT = nc.dram_tensor("cenT", [128, C], FP32, kind="ExternalInput")
    w_rep = nc.dram_tensor("w_rep", [128, T], FP32, kind="ExternalInput")
    bpi_rep = nc.dram_tensor("bpi_rep", [128, T], FP32, kind="ExternalInput")
    iota_t = nc.dram_tensor("iota_t", [128, 128], FP32, kind="ExternalInput")

    emb_out = nc.dram_tensor("emb", [L, D], FP32, kind="ExternalOutput")
    dbg = {}
    if debug:
        dbg['newmem'] = nc.dram_tensor("dbg_newmem", [L, D], FP32, kind="ExternalOutput")
        dbg['simT'] = nc.dram_tensor("dbg_simT", [128, 2, L], BF16, kind="ExternalOutput")
        dbg['taunc'] = nc.dram_tensor("dbg_taunc", [128, NW], FP32, kind="ExternalOutput")
        dbg['taucn'] = nc.dram_tensor("dbg_taucn", [128, 2], FP32, kind="ExternalOutput")
        dbg['cmem'] = nc.dram_tensor("dbg_cmem", [C, D], FP32, kind="ExternalOutput")

    newmem_dram = nc.dram_tensor("newmem_dram", [L, D], FP32)
    aggT_dram = nc.dram_tensor("aggT_dram", [128, 3, L], BF16)
    taunc_dram = nc.dram_tensor("taunc_dram", [NW, 128], BF16)
    rnorm_dram = nc.dram_tensor("rnorm_dram", [NW, 128], BF16)
    ssq_dram = nc.dram_tensor("ssq_dram", [1, L], FP32)
    crec_dram = nc.dram_tensor("crec_dram", [1, C], BF16)
    taucn_dram = nc.dram_tensor("taucn_dram", [2, 128], BF16)
    mg_local = nc.dram_tensor("mg_local", [NCORES, 2, 128, NW], FP32)
    mg_all = nc.dram_tensor("mg_all", [NCORES, 2, 128, NW], FP32, addr_space="Shared")
    st_l = [nc.dram_tensor(f"st_l{i}", [128, 4], FP32) for i in range(NIT_GLB)]
    st_a = [nc.dram_tensor(f"st_a{i}", [128, 4], FP32, addr_space="Shared")
            for i in range(NIT_GLB)]
    st_lm = nc.dram_tensor("st_lm", [128, 4], FP32)
    st_am = nc.dram_tensor("st_am", [128, 4], FP32, addr_space="Shared")
    cm_local = nc.dram_tensor("cm_local", [C, D], FP32)
    cm_all = nc.dram_tensor("cm_all", [C, D], FP32, addr_space="Shared")
    core_oh_in = nc.dram_tensor("core_oh_in", [128, NCORES], FP32, kind="ExternalInput")
    RG = [list(range(NCORES))]

    cc_sem = nc.alloc_semaphore("cc_done")
    ccv = [0]
    ctx = ExitStack()
    with tile.TileContext(nc) as tc, ctx:
        const = ctx.enter_context(tc.tile_pool(name="const", bufs=1))
        late = ctx.enter_context(tc.tile_pool(name="late", bufs=1))
        wk = ctx.enter_context(tc.tile_pool(name="wk", bufs=2))
        scr = ctx.enter_context(tc.tile_pool(name="scr", bufs=1))
        # PSUM: psA bufs=1 {acc3: 3 banks, g1: 1, cmacc: 1}; psB bufs=2 {b1: 2}
        psA = ctx.enter_context(tc.tile_pool(name="psA", bufs=1, space="PSUM"))
        psB = ctx.enter_context(tc.tile_pool(name="psB", bufs=2, space="PSUM"))

        # ----- constants -----
        ident = const.tile([128, 128], BF16)
        make_identity(nc, ident)
        identf = const.tile([128, 128], FP32)
        make_identity(nc, identf)
        iota = const.tile([128, 128], FP32)
        nc.sync.dma_start(out=iota, in_=iota_t[:, :])
        wih = const.tile([128, 4, 384], BF16)
        nc.sync.dma_start(out=wih, in_=W_ihT[:, :, :])
        whh = const.tile([128, 384], BF16)
        nc.sync.dma_start(out=whh, in_=W_hhT[:, :])
        bs = const.tile([128, 2], FP32)
        nc.sync.dma_start(out=bs, in_=bsum[:, :])
        bh2 = const.tile([128, 1], FP32)
        nc.sync.dma_start(out=bh2, in_=b_hh2[:, :])
        bi2 = const.tile([128, 1], FP32)
        nc.sync.dma_start(out=bi2, in_=b_ih2[:, :])
        pw = const.tile([128, P], BF16)
        nc.sync.dma_start(out=pw, in_=pWt[:, :])
        pbt = const.tile([128, 1], FP32)
        nc.sync.dma_start(out=pbt, in_=pb[:, :])
        wr = const.tile([128, T], FP32)
        nc.sync.dma_start(out=wr, in_=w_rep[:, :])
        br = const.tile([128, T], FP32)
        nc.sync.dma_start(out=br, in_=bpi_rep[:, :])
        hascol = const.tile([128, NW], FP32)
        nc.sync.dma_start(out=hascol, in_=has_colT[:, :])
        ones_col = const.tile([128, 1], BF16)
        nc.vector.memset(ones_col, 1.0)

        # centroid norms
        cen = const.tile([128, C], FP32)
        nc.sync.dma_start(out=cen, in_=cenT[:, :])
        censq = wk.tile([128, C], BF16, tag="censq")
        nc.vector.tensor_mul(censq, cen, cen)
        cnorm = wk.tile([1, C], FP32, tag="cnorm")
        ps_c = psB.tile([1, C], FP32, tag="b1")
        nc.tensor.matmul(ps_c, ones_col, censq, start=True, stop=True)
        nc.scalar.activation(cnorm, ps_c, AF.Sqrt)
        nc.vector.tensor_scalar_add(cnorm, cnorm, 1e-8)
        crec = wk.tile([1, C], FP32, tag="crec")
        nc.vector.reciprocal(crec, cnorm)
        crec_b = wk.tile([1, C], BF16, tag="crec_b")
        nc.vector.tensor_copy(crec_b, crec)
        nc.sync.dma_start(out=crec_dram[:, :], in_=crec_b)
        crec_rep = const.tile([128, C], BF16)
        nc.sync.dma_start(out=crec_rep, in_=_bcast_row(crec_dram, C))
        cennT = const.tile([128, C], BF16)
        nc.vector.tensor_mul(cennT, cen, crec_rep)

        # ----- long-lived tensors -----
        simT = late.tile([128, 2, L], BF16)
        tau = late.tile([128, NW], FP32)
        tau_p = late.tile([128, NW], FP32)
        g_c = late.tile([128, NW], FP32)
        g_p = late.tile([128, NW], FP32)
        ctau = late.tile([128, 2], FP32)
        ctau_p = late.tile([128, 2], FP32)
        cg = late.tile([128, 2], FP32)
        cg_p = late.tile([128, 2], FP32)

        with tc.tile_pool(name="mid", bufs=1) as mid:
            featT = mid.tile([128, L], BF16)

            with tc.tile_pool(name="early", bufs=2) as early, \
                    tc.tile_pool(name="evp", bufs=1) as evp:

                # ===== phase 1: events -> aggT (staged to DRAM) =====
                GW = 7
                assert NW % GW == 0
                for grp in range(NW // GW):
                    w0 = grp * GW
                    xg = evp.tile([128, GW * 2, 384], BF16, tag="xg")
                    cols = evp.tile([128, GW * 2], FP32, tag="cols")
                    icnt = evp.tile([128, GW * 2], FP32, tag="icnt")
                    dts = evp.tile([128, GW * 2], FP32, tag="dts")
                    s0 = w0 * 256
                    n_ev = GW * 256
                    nc.sync.dma_start(
                        out=xg[:, :, 0:D],
                        in_=ev_mo[s0:s0 + n_ev, :].rearrange("(t p) d -> p t d", p=128))
                    nc.sync.dma_start(
                        out=xg[:, :, D:D + F],
                        in_=ev_ef[s0:s0 + n_ev, :].rearrange("(t p) d -> p t d", p=128))
                    nc.sync.dma_start(
                        out=cols, in_=ev_col[s0:s0 + n_ev].rearrange("(t p) -> p t", p=128))
                    nc.sync.dma_start(
                        out=icnt, in_=ev_icnt[s0:s0 + n_ev].rearrange("(t p) -> p t", p=128))
                    nc.sync.dma_start(
                        out=dts, in_=ev_dt[s0:s0 + n_ev].rearrange("(t p) -> p t", p=128))
                    for t_ in range(GW * 2):
                        ang = wk.tile([128, T], FP32, tag="ang")
                        nc.vector.scalar_tensor_tensor(
                            ang, wr, dts[:, t_:t_ + 1], br, op0=ALU.mult, op1=ALU.add)
                        # range-reduce to [-pi, pi]: ang -= 2pi*round(ang/2pi)
                        mm_ = wk.tile([128, T], FP32, tag="mm_")
                        nc.vector.tensor_scalar(
                            mm_, ang, 1.0 / (2 * np.pi), 12582912.0,
                            op0=ALU.mult, op1=ALU.add)
                        nc.vector.tensor_scalar_add(mm_, mm_, -12582912.0)
                        nc.vector.scalar_tensor_tensor(
                            ang, mm_, -2 * np.pi, ang, op0=ALU.mult, op1=ALU.add)
                        nc.scalar.activation(xg[:, t_, D + F:], ang, AF.Sin)
                    for wi in range(GW):
                        w = w0 + wi
                        psws = [psA.tile([128, 128], FP32, tag=f"aggfc{fc}",
                                          name=f"psw{fc}") for fc in range(3)]
                        for t_ in range(2):
                            ti = wi * 2 + t_
                            oh = wk.tile([128, 128], BF16, tag="oh")
                            nc.vector.tensor_scalar(
                                oh, iota, cols[:, ti:ti + 1], icnt[:, ti:ti + 1],
                                op0=ALU.is_equal, op1=ALU.mult)
                            for fc in range(3):
                                nc.tensor.matmul(
                                    psws[fc],
                                    xg[:, ti, fc * 128:(fc + 1) * 128],
                                    oh, start=(t_ == 0), stop=(t_ == 1))
                        awin = early.tile([128, 3, 128], BF16, tag="awin")
                        for fc in range(3):
                            nc.vector.tensor_copy(awin[:, fc, :], psws[fc])
                        nc.sync.dma_start(
                            out=aggT_dram[:, :, w * 128:(w + 1) * 128], in_=awin)


                # ===== phase 2: GRU + newmem + featT =====
                for (boff, bsz) in batches:
                    sl = bass.ds(boff, bsz)
                    mTf = wk.tile([128, bsz], FP32, tag="mTf")
                    nc.sync.dma_start(out=mTf, in_=memT[:, sl])
                    mTb = wk.tile([128, bsz], BF16, tag="mTb")
                    nc.vector.tensor_copy(mTb, mTf)
                    agg_b = wk.tile([128, 3, bsz], BF16, tag="agg_b")
                    nc.sync.dma_start(out=agg_b, in_=aggT_dram[:, :, sl])
                    gis = [psA.tile([128, bsz], FP32, tag=f"aggfc{m}",
                                    name=f"gi{m}") for m in range(3)]
                    gh2 = psA.tile([128, bsz], FP32, tag="g1")
                    for m in range(3):
                        nc.tensor.matmul(gis[m], wih[:, 0, m * 128:(m + 1) * 128],
                                         mTb, start=True, stop=False)
                        for k in range(1, 4):
                            last = (k == 3 and m >= 2)
                            nc.tensor.matmul(gis[m],
                                             wih[:, k, m * 128:(m + 1) * 128],
                                             agg_b[:, k - 1, :], start=False,
                                             stop=last)
                        if m < 2:
                            nc.tensor.matmul(gis[m], whh[:, m * 128:(m + 1) * 128],
                                             mTb, start=False, stop=True)
                    nc.tensor.matmul(gh2, whh[:, 256:384], mTb, start=True, stop=True)
                    r = wk.tile([128, bsz], FP32, tag="r")
                    nc.scalar.activation(r, gis[0], AF.Sigmoid, bias=bs[:, 0:1])
                    z = wk.tile([128, bsz], FP32, tag="z")
                    nc.scalar.activation(z, gis[1], AF.Sigmoid, bias=bs[:, 1:2])
                    gh2s = wk.tile([128, bsz], FP32, tag="gh2s")
                    nc.vector.tensor_scalar_add(gh2s, gh2, bh2[:, 0:1])
                    u = wk.tile([128, bsz], FP32, tag="u")
                    nc.vector.tensor_mul(u, r, gh2s)
                    v = wk.tile([128, bsz], FP32, tag="v")
                    nc.vector.tensor_add(v, u, gis[2])
                    n_g = wk.tile([128, bsz], FP32, tag="n_g")
                    nc.scalar.activation(n_g, v, AF.Tanh, bias=bi2[:, 0:1])
                    dmn = wk.tile([128, bsz], FP32, tag="dmn")
                    nc.vector.tensor_sub(dmn, mTf, n_g)
                    e_ = wk.tile([128, bsz], FP32, tag="e_")
                    nc.vector.tensor_mul(e_, z, dmn)
                    updT = wk.tile([128, bsz], FP32, tag="updT")
                    nc.vector.tensor_add(updT, n_g, e_)
                    for cc in range(bsz // 128):
                        ch = boff // 128 + cc
                        pst = psB.tile([128, 128], FP32, tag="b1")
                        nc.tensor.transpose(pst, updT[:, cc * 128:(cc + 1) * 128], identf)
                        mn = wk.tile([128, 128], FP32, tag="mn")
                        nc.sync.dma_start(out=mn, in_=mem_node[ch * 128:(ch + 1) * 128, :])
                        d2 = wk.tile([128, 128], FP32, tag="d2")
                        nc.vector.tensor_sub(d2, pst, mn)
                        e2 = wk.tile([128, 128], FP32, tag="e2")
                        nc.vector.tensor_scalar_mul(e2, d2, hascol[:, ch:ch + 1])
                        nm = wk.tile([128, 128], FP32, tag="nm")
                        nc.vector.tensor_add(nm, mn, e2)
                        nc.sync.dma_start(out=newmem_dram[ch * 128:(ch + 1) * 128, :], in_=nm)
                        if debug:
                            nc.sync.dma_start(out=dbg['newmem'][ch * 128:(ch + 1) * 128, :], in_=nm)
                        pst2 = psB.tile([128, 128], FP32, tag="b1")
                        nc.tensor.transpose(pst2, nm, identf)
                        nfc = wk.tile([128, 128], FP32, tag="nfc")
                        nc.sync.dma_start(out=nfc, in_=nfT[:, ch * 128:(ch + 1) * 128])
                        nc.vector.tensor_add(featT[:, ch * 128:(ch + 1) * 128], pst2, nfc)
            # early pool (aggT) freed here

            # ===== phase 3: pf + norms + sim =====
            with tc.tile_pool(name="pfp", bufs=1) as pfp:
                pfT = pfp.tile([128, L], BF16)
                for (boff, bsz) in batches:
                    sl = bass.ds(boff, bsz)
                    psp = psB.tile([128, bsz], FP32, tag="b1")
                    nc.tensor.matmul(psp, pw, featT[:, sl], start=True, stop=True)
                    pfc = wk.tile([128, bsz], FP32, tag="pfc")
                    nc.vector.tensor_scalar_add(pfc, psp, pbt[:, 0:1])
                    nc.vector.tensor_copy(pfT[:, sl], pfc)
                    sq = wk.tile([128, bsz], BF16, tag="sq")
                    nc.vector.tensor_mul(sq, pfc, pfc)
                    ps_s = psB.tile([1, bsz], FP32, tag="b1")
                    nc.tensor.matmul(ps_s, ones_col, sq, start=True, stop=True)
                    sqe = wk.tile([1, bsz], FP32, tag="sqe")
                    nc.vector.tensor_copy(sqe, ps_s)
                    nc.sync.dma_start(out=ssq_dram[0, sl], in_=sqe)
                ssq_t = wk.tile([128, NW], FP32, tag="ssq_t")
                nc.sync.dma_start(
                    out=ssq_t,
                    in_=ssq_dram.ap().rearrange("o (w p) -> (o p) w", p=128))
                sns = wk.tile([128, NW], FP32, tag="sns")
                nc.scalar.activation(sns, ssq_t, AF.Sqrt)
                nc.vector.tensor_scalar_add(sns, sns, 1e-8)
                rn_t = wk.tile([128, NW], FP32, tag="rn_t")
                nc.vector.reciprocal(rn_t, sns)
                rn_b = wk.tile([128, NW], BF16, tag="rn_b")
                nc.vector.tensor_copy(rn_b, rn_t)
                nc.sync.dma_start(
                    out=rnorm_dram.ap().rearrange("w p -> p w"), in_=rn_b)
                for (boff, bsz) in batches:
                    sl = bass.ds(boff, bsz)
                    rn_rep = wk.tile([128, bsz], BF16, tag="rn_rep")
                    nc.sync.dma_start(out=rn_rep,
                                      in_=_bcast_row(rnorm_dram, bsz, off=boff))
                    for m in range(2):
                        ps_m = psB.tile([128, bsz], FP32, tag="b1")
                        nc.tensor.matmul(ps_m, cennT[:, m * 128:(m + 1) * 128],
                                         pfT[:, sl], start=True, stop=True)
                        nc.vector.tensor_mul(simT[:, m, sl], ps_m, rn_rep)
                if debug:
                    nc.sync.dma_start(out=dbg['simT'][:, :, :], in_=simT)
        # mid pool (featT) freed

        with tc.tile_pool(name="nodep", bufs=1) as nodep:
            sim_node = nodep.tile([128, NW, C], BF16)
            for ch in range(NW):
                for m in range(2):
                    pstr = psB.tile([128, 128], BF16, tag="b1")
                    nc.tensor.transpose(pstr, simT[:, m, ch * 128:(ch + 1) * 128], ident)
                    nc.vector.tensor_copy(sim_node[:, ch, m * 128:(m + 1) * 128], pstr)

            # ===== phase 5+6: interleaved nc (per-node) & cn (global) sparsemax
            # Both evals are single fused relu+accumulate tensor_scalar ops.
            # nc windows split across DVE and Pool; cn runs on DVE with one
            # AllReduce per probe-Newton iteration, warm-started from the
            # global row max.
            junk_v = [scr.tile([128, C], BF16, tag=f"junk_v{i}", name=f"junk_v{i}")
                      for i in range(2)]
            junk_p = [scr.tile([128, C], BF16, tag=f"junk_p{i}", name=f"junk_p{i}")
                      for i in range(2)]
            ngt = scr.tile([128, NW], FP32, tag="ngt")
            zbig = scr.tile([128, max(C, L // 4)], BF16, tag="zbig")
            nc.gpsimd.memset(zbig, 0.0)
            nc.vector.tensor_reduce(tau_p, sim_node, axis=AX.X, op=ALU.max)
            nc.vector.tensor_scalar_add(tau_p, tau_p, -1.0)

            NWH = (NW * 4) // 5  # DVE share of windows (rest on Scalar)

            def nc_eval(tau_tile, g_tile):
                # fused relu+accumulate: DVE scalar_tensor_tensor for most
                # windows, Scalar activation(Relu, bias=-tau) for the tail
                nc.vector.tensor_scalar_mul(ngt[:, NWH:], tau_tile[:, NWH:], -1.0)
                for ch in range(NW):
                    if ch < NWH:
                        nc.vector.scalar_tensor_tensor(
                            junk_v[ch % 2], sim_node[:, ch, :],
                            tau_tile[:, ch:ch + 1], zbig[:, 0:C],
                            op0=ALU.subtract, op1=ALU.max,
                            accum_out=g_tile[:, ch:ch + 1])
                    else:
                        nc.scalar.activation(
                            junk_p[ch % 2], sim_node[:, ch, :], AF.Relu,
                            bias=ngt[:, ch:ch + 1],
                            accum_out=g_tile[:, ch:ch + 1])

            nc_eval(tau_p, g_p)
            st1 = wk.tile([128, NW], FP32, tag="st1")
            nc.vector.tensor_scalar(st1, g_p, -1.0, 1.0 / 256.0,
                                    op0=ALU.add, op1=ALU.mult)
            nc.vector.tensor_add(tau, tau_p, st1)

            def secant_update(tt, tp, gg, gp, wtag, shape):
                num = wk.tile(shape, FP32, tag=wtag + "n")
                nc.vector.tensor_sub(num, tt, tp)
                gm1 = wk.tile(shape, FP32, tag=wtag + "g")
                nc.vector.tensor_scalar_add(gm1, gg, -1.0)
                nc.vector.tensor_mul(num, num, gm1)
                den = wk.tile(shape, FP32, tag=wtag + "d")
                nc.vector.tensor_sub(den, gp, gg)
                nc.vector.tensor_scalar_max(den, den, 1e-12)
                rden = wk.tile(shape, FP32, tag=wtag + "r")
                nc.vector.reciprocal(rden, den)
                nc.vector.tensor_copy(tp, tt)
                nc.vector.tensor_copy(gp, gg)
                stp = wk.tile(shape, FP32, tag=wtag + "s")
                nc.vector.tensor_mul(stp, num, rden)
                # monotone safeguard: secant from below must step in [0, 1]
                nc.vector.tensor_scalar(stp, stp, 0.0, 1.0,
                                        op0=ALU.max, op1=ALU.min)
                nc.vector.tensor_add(tt, tt, stp)

            # cn eval: fused relu+accum over 4 chunks of simT
            CNC = L // 4
            cn_junk = [scr.tile([128, CNC], BF16, tag=f"cnj{i}", name=f"cnj{i}")
                       for i in range(2)]
            gp4 = scr.tile([128, 4, 4], FP32, tag="gp4")
            CN_DELTA = 1e-3
            CN_WARM = 0.15  # global rowmax - tau* is < 0.19 for this data;
            # the step clip allows downward correction so a high start recovers

            def cn_eval4(tt, out4):
                # out4[:, 0:2] = g(tau) per m; out4[:, 2:4] = g(tau + delta)
                td = wk.tile([128, 2], FP32, tag="td")
                nc.vector.tensor_scalar_add(td, tt, CN_DELTA)
                q = 0
                for m in range(2):
                    for pi, tvec in ((0, tt), (2, td)):
                        for j in range(4):
                            nc.vector.scalar_tensor_tensor(
                                cn_junk[q % 2], simT[:, m, bass.ds(j * CNC, CNC)],
                                tvec[:, m:m + 1], zbig[:, 0:CNC],
                                op0=ALU.subtract, op1=ALU.max,
                                accum_out=gp4[:, pi + m, j:j + 1])
                            q += 1
                        nc.vector.tensor_reduce(
                            out4[:, pi + m:pi + m + 1], gp4[:, pi + m, :],
                            axis=AX.X, op=ALU.add)

            # global row max via AllReduce(max)
            rm4 = wk.tile([128, 4], FP32, tag="rm4")
            nc.vector.tensor_reduce(rm4[:, 0:2], simT, axis=AX.X, op=ALU.max)
            nc.vector.tensor_copy(rm4[:, 2:4], rm4[:, 0:2])
            rmg = wk.tile([128, 4], FP32, tag="rmg")
            with tc.tile_critical():
                nc.gpsimd.dma_start(out=st_lm[:, :], in_=rm4).then_inc(cc_sem, 16)
                ccv[0] += 16
                nc.gpsimd.wait_ge(cc_sem, ccv[0])
                nc.gpsimd.collective_compute(
                    "AllReduce", ALU.max, replica_groups=RG,
                    ins=[st_lm.ap().opt()], outs=[st_am.ap().opt()]).then_inc(cc_sem)
                ccv[0] += 1
                nc.gpsimd.wait_ge(cc_sem, ccv[0])
                nc.gpsimd.dma_start(out=rmg, in_=st_am[:, :]).then_inc(cc_sem, 16)
                ccv[0] += 16
                nc.gpsimd.wait_ge(cc_sem, ccv[0])
            nc.vector.tensor_scalar_add(ctau, rmg[:, 0:2], -CN_WARM)

            # interleave: cn probe evals + AllReduce hide behind nc evals
            for it in range(max(NIT_NC, NIT_GLB)):
                if it < NIT_GLB:
                    stt2 = wk.tile([128, 4], FP32, tag=f"stt{it}", name=f"stt{it}")
                    cn_eval4(ctau, stt2)
                if it < NIT_NC:
                    nc_eval(tau, g_c)
                if it < NIT_GLB:
                    stg2 = wk.tile([128, 4], FP32, tag=f"stg{it}", name=f"stg{it}")
                    with tc.tile_critical():
                        nc.gpsimd.dma_start(out=st_l[it][:, :], in_=stt2).then_inc(cc_sem, 16)
                        ccv[0] += 16
                        nc.gpsimd.wait_ge(cc_sem, ccv[0])
                        nc.gpsimd.collective_compute(
                            "AllReduce", ALU.add, replica_groups=RG,
                            ins=[st_l[it].ap().opt()], outs=[st_a[it].ap().opt()]).then_inc(cc_sem)
                        ccv[0] += 1
                        nc.gpsimd.wait_ge(cc_sem, ccv[0])
                        nc.gpsimd.dma_start(out=stg2, in_=st_a[it][:, :]).then_inc(cc_sem, 16)
                        ccv[0] += 16
                        nc.gpsimd.wait_ge(cc_sem, ccv[0])
                if it < NIT_NC:
                    secant_update(tau, tau_p, g_c, g_p, "ncs", [128, NW])
                if it < NIT_GLB:
                    dfc = wk.tile([128, 2], FP32, tag=f"dfc{it}", name=f"dfc{it}")
                    nc.vector.tensor_sub(dfc, stg2[:, 0:2], stg2[:, 2:4])
                    nc.vector.tensor_scalar_max(dfc, dfc, 1e-9)
                    rdf = wk.tile([128, 2], FP32, tag=f"rdf{it}", name=f"rdf{it}")
                    nc.vector.reciprocal(rdf, dfc)
                    gm1 = wk.tile([128, 2], FP32, tag=f"gm1_{it}", name=f"gm1_{it}")
                    nc.vector.tensor_scalar_add(gm1, stg2[:, 0:2], -1.0)
                    stp = wk.tile([128, 2], FP32, tag=f"stp{it}", name=f"stp{it}")
                    nc.vector.tensor_mul(stp, gm1, rdf)
                    nc.vector.tensor_scalar(stp, stp, CN_DELTA, None, op0=ALU.mult)
                    nc.vector.tensor_scalar(stp, stp, -0.1, 1.0, op0=ALU.max, op1=ALU.min)
                    nc.vector.tensor_add(ctau, ctau, stp)
            if debug:
                nc.sync.dma_start(out=dbg['taunc'][:, :], in_=tau)
                nc.sync.dma_start(out=dbg['taucn'][:, :], in_=ctau)
            tau_b = wk.tile([128, NW], BF16, tag="tau_b")
            nc.vector.tensor_copy(tau_b, tau)
            nc.sync.dma_start(
                out=taunc_dram.ap().rearrange("w p -> p w"), in_=tau_b)

            # ===== phase 7: c_memory =====
            taucn_b = wk.tile([128, 2], BF16, tag="taucn_b")
            nc.vector.tensor_copy(taucn_b, ctau)
            nc.sync.dma_start(
                out=taucn_dram.ap().rearrange("m p -> p m"), in_=taucn_b)
            taucn_rep = const.tile([128, C], BF16)
            nc.sync.dma_start(out=taucn_rep, in_=_bcast_row(taucn_dram, C))

            ps_cms = [psA.tile([128, 128], FP32, tag=f"cmacc{m}", name=f"pscm{m}")
                      for m in range(2)]
            for ch in range(NW):
                rp = wk.tile([128, C], BF16, tag="rp")
                nc.vector.scalar_tensor_tensor(
                    rp, sim_node[:, ch, :], 0.0, taucn_rep,
                    op0=ALU.bypass, op1=ALU.subtract)
                nc.vector.tensor_scalar_max(rp, rp, 0.0)
                nmcf = wk.tile([128, 128], FP32, tag="nmcf")
                nc.sync.dma_start(out=nmcf, in_=newmem_dram[ch * 128:(ch + 1) * 128, :])
                nmc = wk.tile([128, 128], BF16, tag="nmc")
                nc.vector.tensor_copy(nmc, nmcf)
                for m in range(2):
                    nc.tensor.matmul(
                        ps_cms[m], rp[:, m * 128:(m + 1) * 128],
                        nmc, start=(ch == 0), stop=(ch == NW - 1))
            cmf = wk.tile([128, 2, 128], FP32, tag="cmf")
            for m in range(2):
                nc.vector.tensor_copy(cmf[:, m, :], ps_cms[m])
            cmgf = wk.tile([128, 2, 128], FP32, tag="cmgf")
            with tc.tile_critical():
                nc.gpsimd.dma_start(
                    out=cm_local.ap().rearrange("(m p) d -> p m d", p=128),
                    in_=cmf).then_inc(cc_sem, 16)
                ccv[0] += 16
                nc.gpsimd.wait_ge(cc_sem, ccv[0])
                nc.gpsimd.collective_compute(
                    "AllReduce", ALU.add, replica_groups=RG,
                    ins=[cm_local.ap().opt()], outs=[cm_all.ap().opt()]).then_inc(cc_sem)
                ccv[0] += 1
                nc.gpsimd.wait_ge(cc_sem, ccv[0])
                nc.gpsimd.dma_start(
                    out=cmgf,
                    in_=cm_all.ap().rearrange("(m p) d -> p m d", p=128)
                ).then_inc(cc_sem, 16)
                ccv[0] += 16
                nc.gpsimd.wait_ge(cc_sem, ccv[0])
        # nodep (sim_node) freed

        cmg = const.tile([128, 2, 128], BF16)
        nc.vector.tensor_copy(cmg, cmgf)
        if debug:
            nc.sync.dma_start(
                out=dbg['cmem'].ap().rearrange("(m p) d -> p m d", p=128),
                in_=cmgf)

        # ===== phase 8: emb =====
        with tc.tile_pool(name="embp", bufs=2) as embp:
            for ch in range(NW):
                sl = bass.ds(ch * 128, 128)
                tnc = embp.tile([128, 128], BF16, tag="tnc")
                nc.sync.dma_start(out=tnc,
                                  in_=_bcast_row(taunc_dram, 128, off=ch * 128))
                ncm = wk.tile([128, 2, 128], BF16, tag="ncm")
                for m in range(2):
                    nc.vector.scalar_tensor_tensor(
                        ncm[:, m, :], simT[:, m, sl], 0.0, tnc,
                        op0=ALU.bypass, op1=ALU.subtract)
                nc.vector.tensor_scalar_max(ncm, ncm, 0.0)
                ps_z = psB.tile([128, 128], FP32, tag="b1")
                for m in range(2):
                    nc.tensor.matmul(ps_z, ncm[:, m, :], cmg[:, m, :],
                                     start=(m == 0), stop=(m == 1))
                nmf = wk.tile([128, 128], FP32, tag="nmf")
                nc.sync.dma_start(out=nmf, in_=newmem_dram[ch * 128:(ch + 1) * 128, :])
                emb_c = wk.tile([128, 128], FP32, tag="emb_c")
                nc.vector.tensor_add(emb_c, ps_z, nmf)
                nc.sync.dma_start(out=emb_out[ch * 128:(ch + 1) * 128, :], in_=emb_c)

    split_waits(nc)
    return nc


# ----------------------------------------------------------------------------
# host side
# ----------------------------------------------------------------------------

_CACHE = {}


def _route(L, src, dst, t):
    idx = np.concatenate([src, dst]).astype(np.int64)
    other = np.concatenate([dst, src]).astype(np.int64)
    tt = np.concatenate([t, t])
    eidx = np.concatenate([np.arange(len(src)), np.arange(len(src))])
    NW = L // 128
    order = np.argsort(idx, kind='stable')
    idx_s, other_s, tt_s, eidx_s = idx[order], other[order], tt[order], eidx[order]
    owner = idx_s // L
    cores = []
    for c in range(NCORES):
        msk = owner == c
        li = idx_s[msk] - c * L
        win = li // 128
        col = li % 128
        wcount = np.bincount(win, minlength=NW)
        assert wcount.max() <= 256, f"window overflow: {wcount.max()}"
        woff = np.zeros(NW + 1, np.int64)
        woff[1:] = np.cumsum(wcount)
        within = np.arange(len(li)) - woff[win]
        slot = win * 256 + within
        cores.append(dict(slot=slot, col=col, li=li, other=other_s[msk],
                          tt=tt_s[msk], eidx=eidx_s[msk]))
    return cores


def kernel(**inputs):
    node_memory = np.asarray(inputs['node_memory'])
    last_update = np.asarray(inputs['last_update'])
    node_features = np.asarray(inputs['node_features'])
    event_feat = np.asarray(inputs['event_feat'])
    t = np.asarray(inputs['t'])
    src = np.asarray(inputs['src']).astype(np.int64)
    dst = np.asarray(inputs['dst']).astype(np.int64)
    time_w = np.asarray(inputs['time_w'])
    time_b = np.asarray(inputs['time_b'])
    W_ih = np.asarray(inputs['W_ih'])
    b_ih = np.asarray(inputs['b_ih'])
    W_hh = np.asarray(inputs['W_hh'])
    b_hh = np.asarray(inputs['b_hh'])
    proj_W = np.asarray(inputs['proj_W'])
    proj_b = np.asarray(inputs['proj_b'])
    centroids = np.asarray(inputs['centroids'])

    Nn = node_memory.shape[0]
    GW = 7
    gran = 128 * GW * NCORES          # L must be multiple of 128*GW
    NP = -(-Nn // gran) * gran
    L = NP // NCORES
    SLOTS = 2 * L
    NW = L // 128

    nmp = np.zeros((NP, D), np.float32); nmp[:Nn] = node_memory
    nfp = np.zeros((NP, D), np.float32); nfp[:Nn] = node_features
    lup = np.zeros(NP, np.float32); lup[:Nn] = last_update

    idx_full = np.concatenate([src, dst])
    cnt_full = np.bincount(idx_full, minlength=NP).astype(np.float32)
    icnt_full = 1.0 / np.maximum(cnt_full, 1.0)
    has_full = (cnt_full > 0).astype(np.float32)

    cores = _route(L, src, dst, t)
    bsum_h = f32c(np.stack([(b_ih + b_hh)[0:128], (b_ih + b_hh)[128:256]], 1))
    wih_h = bfc(W_ih.T.reshape(4, 128, 384).transpose(1, 0, 2))

    in_maps = []
    for c in range(NCORES):
        r = cores[c]
        sl = r['slot']
        ev_mo = np.zeros((SLOTS, D), ml_dtypes.bfloat16)
        ev_ef = np.zeros((SLOTS, F), ml_dtypes.bfloat16)
        ev_dt = np.zeros(SLOTS, np.float32)
        ev_col = np.full(SLOTS, -1.0, np.float32)
        ev_icnt = np.zeros(SLOTS, np.float32)
        ev_mo[sl] = nmp[r['other']].astype(ml_dtypes.bfloat16)
        ev_ef[sl] = event_feat[r['eidx']].astype(ml_dtypes.bfloat16)
        ev_dt[sl] = r['tt'] - lup[r['li'] + c * L]
        ev_col[sl] = r['col'].astype(np.float32)
        ev_icnt[sl] = icnt_full[r['li'] + c * L]
        nsl = slice(c * L, (c + 1) * L)
        in_maps.append({
            'memT': f32c(nmp[nsl].T),
            'mem_node': f32c(nmp[nsl]),
            'nfT': f32c(nfp[nsl].T),
            'has_colT': f32c(has_full[nsl].reshape(NW, 128).T),
            'ev_mo': ev_mo, 'ev_ef': ev_ef, 'ev_dt': ev_dt,
            'ev_col': ev_col, 'ev_icnt': ev_icnt,
            'W_ihT': wih_h,
            'W_hhT': bfc(W_hh.T),
            'bsum': bsum_h,
            'b_hh2': f32c(b_hh[256:384].reshape(128, 1)),
            'b_ih2': f32c(b_ih[256:384].reshape(128, 1)),
            'pWt': bfc(proj_W),
            'pb': f32c(proj_b.reshape(128, 1)),
            'cenT': f32c(centroids.T),
            'w_rep': f32c(np.tile(time_w[None, :], (128, 1))),
            'bpi_rep': f32c(np.tile(time_b[None, :] + HALF_PI, (128, 1))),
            'iota_t': f32c(np.tile(np.arange(128, dtype=np.float32)[None, :],
                                   (128, 1))),
            'core_oh_in': f32c(np.tile(np.eye(NCORES, dtype=np.float32)[c][None, :],
                                       (128, 1))),
        })

    debug = bool(int(os.environ.get("KERNEL_DEBUG", "0")))
    key = (L, debug)
    if key not in _CACHE:
        _CACHE[key] = build_program(L, debug=debug)
    nc = _CACHE[key]
    trace = bool(int(os.environ.get("KERNEL_TRACE", "0")))
    res = run_bass_kernel_spmd(nc, in_maps, list(range(NCORES)), trace=trace)
    emb = np.concatenate([res.results[c]['emb'] for c in range(NCORES)], 0)
    kernel._last_exec_ns = getattr(res, 'exec_time_ns', None)
    kernel._last_profile = getattr(res, 'profile_json', None)
    if debug:
        kernel._last_results = res.results
    return emb[:Nn].astype(np.float32)



# revision 41
# speedup vs baseline: 1.5504x; 1.1143x over previous
"""TGN-style GNN message passing + community detection on 8 TRN2 NeuronCores.

Node-sharded SPMD: nodes padded to 8*L and sharded contiguously; events
routed by host (index work only) to the owner core of their update target
and binned into 128-node windows (2x128 slots per window). Segment-mean via
inv-cnt-scaled one-hot matmuls on the PE; GRU/proj/sim as bf16 matmuls;
sparsemax taus via secant iterations on g(tau)=sum(relu(z-tau)) with an
AllGathered chunk-max warm start for the centroid direction; c_memory
partials AllReduced. All float arithmetic on device.
"""

import os
from contextlib import ExitStack

import numpy as np
import ml_dtypes

import concourse.bass as bass
import concourse.mybir as mybir
import concourse.tile as tile
from concourse.bass_utils import run_bass_kernel_spmd
from concourse.masks import make_identity

FP32 = mybir.dt.float32
BF16 = mybir.dt.bfloat16
AF = mybir.ActivationFunctionType
ALU = mybir.AluOpType
AX = mybir.AxisListType

NCORES = 8
D = 128
F = 128
T = 128
P = 128
C = 256
HALF_PI = float(np.pi / 2)

bfc = lambda x: np.ascontiguousarray(np.asarray(x).astype(ml_dtypes.bfloat16))
f32c = lambda x: np.ascontiguousarray(np.asarray(x).astype(np.float32))


def _bcast_row(dram_tensor, ncols, nparts=128, off=0):
    row = dram_tensor.ap()
    return bass.AP(tensor=row.tensor, offset=row.offset + off,
                   ap=[[0, nparts], [1, ncols]])


def split_waits(nc, sp_limit=1, default_limit=1):
    """This env's walrus rejects >1 sync-wait on SP CTRL instructions:
    move extra waits onto preceding NOPs."""
    limits = {mybir.EngineType.SP: sp_limit}
    for fn in nc.m.functions:
        for bb in fn.blocks:
            out = []
            for ins in bb.instructions:
                si = ins.sync_info
                w = list(si.on_wait) if (si is not None and si.on_wait) else []
                lim = limits.get(ins.engine, default_limit)
                if len(w) > lim:
                    extra, keep = w[:-lim], w[-lim:]
                    for j in range(0, len(extra), lim):
                        out.append(mybir.InstNoOp(
                            name=f"{ins.name}-ws{j}",
                            engine=ins.engine,
                            sync_info=mybir.SyncInfo(
                                on_wait=list(extra[j:j + lim]), on_update=[]),
                        ))
                    ins.sync_info = mybir.SyncInfo(
                        on_wait=list(keep),
                        on_update=list(si.on_update) if si.on_update else [])
                out.append(ins)
            bb.instructions = out
    return nc


def build_program(L, NIT_NC=8, NIT_MINI=16, NIT_GLB=6, debug=False):
    NW = L // 128
    SLOTS = 2 * L
    MGW = NW * NCORES
    # node batches of <=256 (SBUF headroom), multiples of 128
    batches = []
    off = 0
    while off < L:
        bs_ = min(256, L - off)
        batches.append((off, bs_))
        off += bs_

    nc = bass.Bass(num_devices=NCORES)

    memT = nc.dram_tensor("memT", [128, L], FP32, kind="ExternalInput")
    mem_node = nc.dram_tensor("mem_node", [L, D], FP32, kind="ExternalInput")
    nfT = nc.dram_tensor("nfT", [128, L], FP32, kind="ExternalInput")
    has_colT = nc.dram_tensor("has_colT", [128, NW], FP32, kind="ExternalInput")
    ev_mo = nc.dram_tensor("ev_mo", [SLOTS, D], BF16, kind="ExternalInput")
    ev_ef = nc.dram_tensor("ev_ef", [SLOTS, F], BF16, kind="ExternalInput")
    ev_dt = nc.dram_tensor("ev_dt", [SLOTS], FP32, kind="ExternalInput")
    ev_col = nc.dram_tensor("ev_col", [SLOTS], FP32, kind="ExternalInput")
    ev_icnt = nc.dram_tensor("ev_icnt", [SLOTS], FP32, kind="ExternalInput")
    W_ihT = nc.dram_tensor("W_ihT", [128, 4, 384], BF16, kind="ExternalInput")
    W_hhT = nc.dram_tensor("W_hhT", [128, 384], BF16, kind="ExternalInput")
    bsum = nc.dram_tensor("bsum", [128, 2], FP32, kind="ExternalInput")
    b_hh2 = nc.dram_tensor("b_hh2", [128, 1], FP32, kind="ExternalInput")
    b_ih2 = nc.dram_tensor("b_ih2", [128, 1], FP32, kind="ExternalInput")
    pWt = nc.dram_tensor("pWt", [128, P], BF16, kind="ExternalInput")
    pb = nc.dram_tensor("pb", [128, 1], FP32, kind="ExternalInput")
    cenT = nc.dram_tensor("cenT", [128, C], FP32, kind="ExternalInput")
    w_rep = nc.dram_tensor("w_rep", [128, T], FP32, kind="ExternalInput")
    bpi_rep = nc.dram_tensor("bpi_rep", [128, T], FP32, kind="ExternalInput")
    iota_t = nc.dram_tensor("iota_t", [128, 128], FP32, kind="ExternalInput")

    emb_out = nc.dram_tensor("emb", [L, D], FP32, kind="ExternalOutput")
    dbg = {}
    if debug:
        dbg['newmem'] = nc.dram_tensor("dbg_newmem", [L, D], FP32, kind="ExternalOutput")
        dbg['simT'] = nc.dram_tensor("dbg_simT", [128, 2, L], BF16, kind="ExternalOutput")
        dbg['taunc'] = nc.dram_tensor("dbg_taunc", [128, NW], FP32, kind="ExternalOutput")
        dbg['taucn'] = nc.dram_tensor("dbg_taucn", [128, 2], FP32, kind="ExternalOutput")
        dbg['cmem'] = nc.dram_tensor("dbg_cmem", [C, D], FP32, kind="ExternalOutput")
        dbg['simnode'] = nc.dram_tensor("dbg_simnode", [128, NW, C], BF16, kind="ExternalOutput")
        dbg['g0'] = nc.dram_tensor("dbg_g0", [128, 4, NW], FP32, kind="ExternalOutput")
        dbg['trace'] = nc.dram_tensor("dbg_trace", [128, 2 * NIT_NC, NW], FP32, kind="ExternalOutput")
        dbg['cntr'] = nc.dram_tensor("dbg_cntr", [128, NIT_GLB, 10], FP32, kind="ExternalOutput")

    aggT_dram = nc.dram_tensor("aggT_dram", [128, 3, L], BF16)
    taunc_dram = nc.dram_tensor("taunc_dram", [NW, 128], BF16)
    rnorm_dram = nc.dram_tensor("rnorm_dram", [NW, 128], BF16)
    ssq_dram = nc.dram_tensor("ssq_dram", [1, L], FP32)
    crec_dram = nc.dram_tensor("crec_dram", [1, C], BF16)
    taucn_dram = nc.dram_tensor("taucn_dram", [2, 128], BF16)
    mg_local = nc.dram_tensor("mg_local", [NCORES, 2, 128, NW], FP32)
    mg_all = nc.dram_tensor("mg_all", [NCORES, 2, 128, NW], FP32, addr_space="Shared")
    st_l = [nc.dram_tensor(f"st_l{i}", [128, 4], FP32) for i in range(NIT_GLB)]
    st_a = [nc.dram_tensor(f"st_a{i}", [128, 4], FP32, addr_space="Shared")
            for i in range(NIT_GLB)]
    st_lm = nc.dram_tensor("st_lm", [128, 4], FP32)
    st_am = nc.dram_tensor("st_am", [128, 4], FP32, addr_space="Shared")
    cm_local = nc.dram_tensor("cm_local", [C, D], FP32)
    cm_all = nc.dram_tensor("cm_all", [C, D], FP32, addr_space="Shared")
    core_oh_in = nc.dram_tensor("core_oh_in", [128, NCORES], FP32, kind="ExternalInput")
    RG = [list(range(NCORES))]

    cc_sem = nc.alloc_semaphore("cc_done")
    ccv = [0]
    ctx = ExitStack()
    with tile.TileContext(nc) as tc, ctx:
        const = ctx.enter_context(tc.tile_pool(name="const", bufs=1))
        late = ctx.enter_context(tc.tile_pool(name="late", bufs=1))
        wk = ctx.enter_context(tc.tile_pool(name="wk", bufs=2))
        scr = ctx.enter_context(tc.tile_pool(name="scr", bufs=1))
        # PSUM: psA bufs=1 {acc3: 3 banks, g1: 1, cmacc: 1}; psB bufs=2 {b1: 2}
        psA = ctx.enter_context(tc.tile_pool(name="psA", bufs=1, space="PSUM"))
        psB = ctx.enter_context(tc.tile_pool(name="psB", bufs=2, space="PSUM"))

        # ----- constants -----
        ident = const.tile([128, 128], BF16)
        make_identity(nc, ident)
        identf = const.tile([128, 128], FP32)
        make_identity(nc, identf)
        iota = const.tile([128, 128], FP32)
        nc.sync.dma_start(out=iota, in_=iota_t[:, :])
        wih = const.tile([128, 4, 384], BF16)
        nc.sync.dma_start(out=wih, in_=W_ihT[:, :, :])
        whh = const.tile([128, 384], BF16)
        nc.sync.dma_start(out=whh, in_=W_hhT[:, :])
        bs = const.tile([128, 2], FP32)
        nc.sync.dma_start(out=bs, in_=bsum[:, :])
        bh2 = const.tile([128, 1], FP32)
        nc.sync.dma_start(out=bh2, in_=b_hh2[:, :])
        bi2 = const.tile([128, 1], FP32)
        nc.sync.dma_start(out=bi2, in_=b_ih2[:, :])
        pw = const.tile([128, P], BF16)
        nc.sync.dma_start(out=pw, in_=pWt[:, :])
        pbt = const.tile([128, 1], FP32)
        nc.sync.dma_start(out=pbt, in_=pb[:, :])
        wr = const.tile([128, T], FP32)
        nc.sync.dma_start(out=wr, in_=w_rep[:, :])
        br = const.tile([128, T], FP32)
        nc.sync.dma_start(out=br, in_=bpi_rep[:, :])
        hascol = const.tile([128, NW], FP32)
        nc.sync.dma_start(out=hascol, in_=has_colT[:, :])
        ones_col = const.tile([128, 1], BF16)
        nc.vector.memset(ones_col, 1.0)

        # centroid norms
        cen = const.tile([128, C], FP32)
        nc.sync.dma_start(out=cen, in_=cenT[:, :])
        censq = wk.tile([128, C], BF16, tag="censq")
        nc.vector.tensor_mul(censq, cen, cen)
        cnorm = wk.tile([1, C], FP32, tag="cnorm")
        ps_c = psB.tile([1, C], FP32, tag="b1")
        nc.tensor.matmul(ps_c, ones_col, censq, start=True, stop=True)
        nc.scalar.activation(cnorm, ps_c, AF.Sqrt)
        nc.vector.tensor_scalar_add(cnorm, cnorm, 1e-8)
        crec = wk.tile([1, C], FP32, tag="crec")
        nc.vector.reciprocal(crec, cnorm)
        crec_b = wk.tile([1, C], BF16, tag="crec_b")
        nc.vector.tensor_copy(crec_b, crec)
        nc.sync.dma_start(out=crec_dram[:, :], in_=crec_b)
        crec_rep = const.tile([128, C], BF16)
        nc.sync.dma_start(out=crec_rep, in_=_bcast_row(crec_dram, C))
        cennT = const.tile([128, C], BF16)
        nc.vector.tensor_mul(cennT, cen, crec_rep)

        # ----- long-lived tensors -----
        simT = late.tile([128, 2, L], BF16)
        nmem = late.tile([128, NW, 128], BF16)  # node-major new memory
        tau = late.tile([128, NW], FP32)
        tau_p = late.tile([128, NW], FP32)
        g_c = late.tile([128, NW], FP32)
        g_p = late.tile([128, NW], FP32)
        ctau = late.tile([128, 2], FP32)
        ctau_p = late.tile([128, 2], FP32)
        cg = late.tile([128, 2], FP32)
        cg_p = late.tile([128, 2], FP32)

        with tc.tile_pool(name="mid", bufs=1) as mid:
            featT = mid.tile([128, L], BF16)

            with tc.tile_pool(name="early", bufs=2) as early, \
                    tc.tile_pool(name="evp", bufs=1) as evp, \
                    tc.tile_pool(name="gruw", bufs=2) as gruw:

                # ===== phase 1: events -> aggT (staged to DRAM) =====
                GW = 7
                assert NW % GW == 0
                for grp in range(NW // GW):
                    w0 = grp * GW
                    xg = evp.tile([128, GW * 2, 384], BF16, tag="xg")
                    cols = evp.tile([128, GW * 2], FP32, tag="cols")
                    icnt = evp.tile([128, GW * 2], FP32, tag="icnt")
                    dts = evp.tile([128, GW * 2], FP32, tag="dts")
                    s0 = w0 * 256
                    n_ev = GW * 256
                    nc.sync.dma_start(
                        out=xg[:, :, 0:D],
                        in_=ev_mo[s0:s0 + n_ev, :].rearrange("(t p) d -> p t d", p=128))
                    nc.sync.dma_start(
                        out=xg[:, :, D:D + F],
                        in_=ev_ef[s0:s0 + n_ev, :].rearrange("(t p) d -> p t d", p=128))
                    nc.sync.dma_start(
                        out=cols, in_=ev_col[s0:s0 + n_ev].rearrange("(t p) -> p t", p=128))
                    nc.sync.dma_start(
                        out=icnt, in_=ev_icnt[s0:s0 + n_ev].rearrange("(t p) -> p t", p=128))
                    nc.sync.dma_start(
                        out=dts, in_=ev_dt[s0:s0 + n_ev].rearrange("(t p) -> p t", p=128))
                    for t_ in range(GW * 2):
                        ang = gruw.tile([128, T], FP32, tag="ang")
                        nc.vector.scalar_tensor_tensor(
                            ang, wr, dts[:, t_:t_ + 1], br, op0=ALU.mult, op1=ALU.add)
                        # range-reduce to [-pi, pi]: ang -= 2pi*round(ang/2pi)
                        mm_ = gruw.tile([128, T], FP32, tag="mm_")
                        nc.vector.tensor_scalar(
                            mm_, ang, 1.0 / (2 * np.pi), 12582912.0,
                            op0=ALU.mult, op1=ALU.add)
                        nc.vector.tensor_scalar_add(mm_, mm_, -12582912.0)
                        nc.vector.scalar_tensor_tensor(
                            ang, mm_, -2 * np.pi, ang, op0=ALU.mult, op1=ALU.add)
                        nc.scalar.activation(xg[:, t_, D + F:], ang, AF.Sin)
                    for wi in range(GW):
                        w = w0 + wi
                        psws = [psA.tile([128, 128], FP32, tag=f"aggfc{fc}",
                                          name=f"psw{fc}") for fc in range(3)]
                        for t_ in range(2):
                            ti = wi * 2 + t_
                            oh = gruw.tile([128, 128], BF16, tag="oh")
                            nc.vector.tensor_scalar(
                                oh, iota, cols[:, ti:ti + 1], icnt[:, ti:ti + 1],
                                op0=ALU.is_equal, op1=ALU.mult)
                            for fc in range(3):
                                nc.tensor.matmul(
                                    psws[fc],
                                    xg[:, ti, fc * 128:(fc + 1) * 128],
                                    oh, start=(t_ == 0), stop=(t_ == 1))
                        awin = early.tile([128, 3, 128], BF16, tag="awin")
                        for fc in range(3):
                            nc.vector.tensor_copy(awin[:, fc, :], psws[fc])
                        nc.sync.dma_start(
                            out=aggT_dram[:, :, w * 128:(w + 1) * 128], in_=awin)


                # ===== phase 2: GRU + newmem + featT =====
                for (boff, bsz) in batches:
                    sl = bass.ds(boff, bsz)
                    mTf = gruw.tile([128, bsz], FP32, tag="mTf")
                    nc.sync.dma_start(out=mTf, in_=memT[:, sl])
                    mTb = gruw.tile([128, bsz], BF16, tag="mTb")
                    nc.vector.tensor_copy(mTb, mTf)
                    agg_b = gruw.tile([128, 3, bsz], BF16, tag="agg_b")
                    nc.sync.dma_start(out=agg_b, in_=aggT_dram[:, :, sl])
                    gis = [psA.tile([128, bsz], FP32, tag=f"aggfc{m}",
                                    name=f"gi{m}") for m in range(3)]
                    gh2 = psA.tile([128, bsz], FP32, tag="g1")
                    for m in range(3):
                        nc.tensor.matmul(gis[m], wih[:, 0, m * 128:(m + 1) * 128],
                                         mTb, start=True, stop=False)
                        for k in range(1, 4):
                            last = (k == 3 and m >= 2)
                            nc.tensor.matmul(gis[m],
                                             wih[:, k, m * 128:(m + 1) * 128],
                                             agg_b[:, k - 1, :], start=False,
                                             stop=last)
                        if m < 2:
                            nc.tensor.matmul(gis[m], whh[:, m * 128:(m + 1) * 128],
                                             mTb, start=False, stop=True)
                    nc.tensor.matmul(gh2, whh[:, 256:384], mTb, start=True, stop=True)
                    r = gruw.tile([128, bsz], FP32, tag="r")
                    nc.scalar.activation(r, gis[0], AF.Sigmoid, bias=bs[:, 0:1])
                    z = gruw.tile([128, bsz], FP32, tag="z")
                    nc.scalar.activation(z, gis[1], AF.Sigmoid, bias=bs[:, 1:2])
                    gh2s = gruw.tile([128, bsz], FP32, tag="gh2s")
                    nc.vector.tensor_scalar_add(gh2s, gh2, bh2[:, 0:1])
                    u = gruw.tile([128, bsz], FP32, tag="u")
                    nc.vector.tensor_mul(u, r, gh2s)
                    v = gruw.tile([128, bsz], FP32, tag="v")
                    nc.vector.tensor_add(v, u, gis[2])
                    n_g = gruw.tile([128, bsz], FP32, tag="n_g")
                    nc.scalar.activation(n_g, v, AF.Tanh, bias=bi2[:, 0:1])
                    dmn = gruw.tile([128, bsz], FP32, tag="dmn")
                    nc.vector.tensor_sub(dmn, mTf, n_g)
                    e_ = gruw.tile([128, bsz], FP32, tag="e_")
                    nc.vector.tensor_mul(e_, z, dmn)
                    updT = gruw.tile([128, bsz], FP32, tag="updT")
                    nc.vector.tensor_add(updT, n_g, e_)
                    for cc in range(bsz // 128):
                        ch = boff // 128 + cc
                        pst = psB.tile([128, 128], FP32, tag="b1")
                        nc.tensor.transpose(pst, updT[:, cc * 128:(cc + 1) * 128], identf)
                        mn = gruw.tile([128, 128], FP32, tag="mn")
                        nc.sync.dma_start(out=mn, in_=mem_node[ch * 128:(ch + 1) * 128, :])
                        d2 = gruw.tile([128, 128], FP32, tag="d2")
                        nc.vector.tensor_sub(d2, pst, mn)
                        # nmem = mn + has*(upd - mn), fused
                        nc.vector.scalar_tensor_tensor(
                            nmem[:, ch, :], d2, hascol[:, ch:ch + 1], mn,
                            op0=ALU.mult, op1=ALU.add)
                        if debug:
                            nc.gpsimd.dma_start(out=dbg['newmem'][ch * 128:(ch + 1) * 128, :],
                                                in_=nmem[:, ch, :])
                        pst2 = psA.tile([128, 128], BF16, tag="cmacc0", name="pst2")
                        nc.tensor.transpose(pst2, nmem[:, ch, :], ident)
                        nfc = gruw.tile([128, 128], FP32, tag="nfc")
                        nc.sync.dma_start(out=nfc, in_=nfT[:, ch * 128:(ch + 1) * 128])
                        nc.vector.tensor_add(featT[:, ch * 128:(ch + 1) * 128], pst2, nfc)
            # early pool (aggT) freed here

            # ===== phase 3: pf + norms + sim =====
            with tc.tile_pool(name="pfp", bufs=1) as pfp:
                pfT = pfp.tile([128, L], BF16)
                for (boff, bsz) in batches:
                    sl = bass.ds(boff, bsz)
                    psp = psB.tile([128, bsz], FP32, tag="b1")
                    nc.tensor.matmul(psp, pw, featT[:, sl], start=True, stop=True)
                    pfc = wk.tile([128, bsz], FP32, tag="pfc")
                    nc.vector.tensor_scalar_add(pfc, psp, pbt[:, 0:1])
                    nc.vector.tensor_copy(pfT[:, sl], pfc)
                    sq = wk.tile([128, bsz], BF16, tag="sq")
                    nc.vector.tensor_mul(sq, pfc, pfc)
                    ps_s = psB.tile([1, bsz], FP32, tag="b1")
                    nc.tensor.matmul(ps_s, ones_col, sq, start=True, stop=True)
                    sqe = wk.tile([1, bsz], FP32, tag="sqe")
                    nc.vector.tensor_copy(sqe, ps_s)
                    nc.sync.dma_start(out=ssq_dram[0, sl], in_=sqe)
                ssq_t = wk.tile([128, NW], FP32, tag="ssq_t")
                nc.sync.dma_start(
                    out=ssq_t,
                    in_=ssq_dram.ap().rearrange("o (w p) -> (o p) w", p=128))
                sns = wk.tile([128, NW], FP32, tag="sns")
                nc.scalar.activation(sns, ssq_t, AF.Sqrt)
                nc.vector.tensor_scalar_add(sns, sns, 1e-8)
                rn_t = wk.tile([128, NW], FP32, tag="rn_t")
                nc.vector.reciprocal(rn_t, sns)
                rn_b = wk.tile([128, NW], BF16, tag="rn_b")
                nc.vector.tensor_copy(rn_b, rn_t)
                nc.sync.dma_start(
                    out=rnorm_dram.ap().rearrange("w p -> p w"), in_=rn_b)
                for (boff, bsz) in batches:
                    sl = bass.ds(boff, bsz)
                    rn_rep = wk.tile([128, bsz], BF16, tag="rn_rep")
                    nc.sync.dma_start(out=rn_rep,
                                      in_=_bcast_row(rnorm_dram, bsz, off=boff))
                    for m in range(2):
                        ps_m = psB.tile([128, bsz], FP32, tag="b1")
                        nc.tensor.matmul(ps_m, cennT[:, m * 128:(m + 1) * 128],
                                         pfT[:, sl], start=True, stop=True)
                        nc.vector.tensor_mul(simT[:, m, sl], ps_m, rn_rep)
                if debug:
                    nc.sync.dma_start(out=dbg['simT'][:, :, :], in_=simT)
        # mid pool (featT) freed

        with tc.tile_pool(name="nodep", bufs=1) as nodep:
            # sim_node built by xbar DMA transposes (frees PE + Vector)
            sim_node = nodep.tile([128, NW, C], BF16)
            for ch in range(NW):
                for m in range(2):
                    eng = nc.sync if (2 * ch + m) % 2 == 0 else nc.scalar
                    eng.dma_start_transpose(
                        out=sim_node[:, ch, m * 128:(m + 1) * 128],
                        in_=simT[:, m, ch * 128:(ch + 1) * 128])
            if debug:
                nc.sync.dma_start(out=dbg['simnode'][:, :, :], in_=sim_node)

            # ===== phase 5+6: interleaved nc (per-node) & cn (global) sparsemax
            # Both evals are single fused relu+accumulate tensor_scalar ops.
            # nc windows split across DVE and Pool; cn runs on DVE with one
            # AllReduce per probe-Newton iteration, warm-started from the
            # global row max.
            junk_v = scr.tile([128, C], BF16, tag="junk_v")
            junk_p = scr.tile([128, C], BF16, tag="junk_p")
            ngt = scr.tile([128, NW], FP32, tag="ngt")
            nc.vector.tensor_reduce(tau_p, sim_node, axis=AX.X, op=ALU.max)
            nc.vector.tensor_scalar_add(tau_p, tau_p, -1.0)

            # all nc windows on DVE max-trick (C=256-term sums: offset bias
            # ~1e-3, fine); cn chunks all on Scalar exact relu-accum (3136-term
            # max-trick sums carry ~0.1-0.5 fp32 truncation bias - too noisy)
            NWH = NW

            def nc_eval(tau_tile, g_tile):
                # DVE: acc = sum(max(sim, tau)) = g + C*tau in ONE fused op;
                # Scalar: activation(Relu, bias=-tau) accumulates exact g.
                if NWH < NW:
                    nc.vector.tensor_scalar_mul(ngt[:, NWH:], tau_tile[:, NWH:], -1.0)
                for ch in range(NW):
                    if ch < NWH:
                        nc.vector.tensor_scalar(
                            junk_v, sim_node[:, ch, :],
                            tau_tile[:, ch:ch + 1], None,
                            op0=ALU.max, op1=ALU.add,
                            accum_out=g_tile[:, ch:ch + 1])
                    else:
                        nc.scalar.activation(
                            junk_p, sim_node[:, ch, :], AF.Relu,
                            bias=ngt[:, ch:ch + 1],
                            accum_out=g_tile[:, ch:ch + 1])
                # strip the C*tau offset from the DVE half: g -= C*tau
                nc.vector.scalar_tensor_tensor(
                    g_tile[:, 0:NWH], tau_tile[:, 0:NWH], -float(C),
                    g_tile[:, 0:NWH], op0=ALU.mult, op1=ALU.add)

            nc_eval(tau_p, g_p)
            if debug:
                nc.sync.dma_start(out=dbg['g0'][:, 0, :], in_=g_p)
            st1 = wk.tile([128, NW], FP32, tag="st1")
            nc.vector.tensor_scalar(st1, g_p, -1.0, 1.0 / 256.0,
                                    op0=ALU.add, op1=ALU.mult)
            nc.vector.tensor_add(tau, tau_p, st1)
            if debug:
                nc.sync.dma_start(out=dbg['g0'][:, 1, :], in_=tau)

            def secant_update(tt, tp, gg, gp, wtag, shape):
                num = wk.tile(shape, FP32, tag=wtag + "n")
                nc.vector.tensor_sub(num, tt, tp)
                gm1 = wk.tile(shape, FP32, tag=wtag + "g")
                nc.vector.tensor_scalar_add(gm1, gg, -1.0)
                nc.vector.tensor_mul(num, num, gm1)
                den = wk.tile(shape, FP32, tag=wtag + "d")
                nc.vector.tensor_sub(den, gp, gg)
                # floor guards against den collapse at convergence: tiny den
                # with positive g-noise would clip the step to +1 (overshoot)
                nc.vector.tensor_scalar_max(den, den, 1e-3)
                rden = wk.tile(shape, FP32, tag=wtag + "r")
                nc.vector.reciprocal(rden, den)
                nc.vector.tensor_copy(tp, tt)
                nc.vector.tensor_copy(gp, gg)
                stp = wk.tile(shape, FP32, tag=wtag + "s")
                nc.vector.tensor_mul(stp, num, rden)
                # monotone safeguard: secant from below must step in [0, 1]
                nc.vector.tensor_scalar(stp, stp, 0.0, 1.0,
                                        op0=ALU.max, op1=ALU.min)
                nc.vector.tensor_add(tt, tt, stp)

            # cn eval: fused relu+accum over 4 chunks of simT, split DVE/Scalar
            CNC = L // 4
            cn_junk = scr.tile([128, CNC], BF16, tag="cn_junk")
            cn_junk_s = scr.tile([128, CNC], BF16, tag="cn_junk_s")
            gp4 = scr.tile([128, 4, 4], FP32, tag="gp4")
            CN_DELTA = 1e-3
            CN_WARM = 0.15  # global rowmax - tau* is < 0.19 for this data;
            # the step clip allows downward correction so a high start recovers

            def cn_eval4(tt, out4):
                # out4 columns: [g(t)_m0, g(t)_m1, g(t+d)_m0, g(t+d)_m1]
                ngc = wk.tile([128, 4], FP32, tag="ngc")
                nc.vector.tensor_scalar_mul(ngc[:, 0:2], tt, -1.0)
                nc.vector.tensor_scalar(ngc[:, 2:4], tt, -1.0, -CN_DELTA,
                                        op0=ALU.mult, op1=ALU.add)
                for m in range(2):
                    for pi in (0, 2):
                        col = pi + m
                        for j in range(4):
                            jt = cn_junk_s if j % 2 else cn_junk
                            nc.scalar.activation(
                                jt, simT[:, m, bass.ds(j * CNC, CNC)],
                                AF.Relu, bias=ngc[:, col:col + 1],
                                accum_out=gp4[:, col, j:j + 1])
                        nc.vector.tensor_reduce(
                            out4[:, col:col + 1], gp4[:, col, :],
                            axis=AX.X, op=ALU.add)

            # global row max via AllReduce(max)
            rm4 = wk.tile([128, 4], FP32, tag="rm4")
            nc.vector.tensor_reduce(rm4[:, 0:2], simT, axis=AX.X, op=ALU.max)
            nc.vector.tensor_copy(rm4[:, 2:4], rm4[:, 0:2])
            rmg = wk.tile([128, 4], FP32, tag="rmg")
            with tc.tile_critical():
                nc.gpsimd.dma_start(out=st_lm[:, :], in_=rm4).then_inc(cc_sem, 16)
                ccv[0] += 16
                nc.gpsimd.wait_ge(cc_sem, ccv[0])
                nc.gpsimd.collective_compute(
                    "AllReduce", ALU.max, replica_groups=RG,
                    ins=[st_lm.ap().opt()], outs=[st_am.ap().opt()]).then_inc(cc_sem)
                ccv[0] += 1
                nc.gpsimd.wait_ge(cc_sem, ccv[0])
                nc.gpsimd.dma_start(out=rmg, in_=st_am[:, :]).then_inc(cc_sem, 16)
                ccv[0] += 16
                nc.gpsimd.wait_ge(cc_sem, ccv[0])
            nc.vector.tensor_scalar_add(ctau, rmg[:, 0:2], -CN_WARM)

            # interleave: cn probe evals + AllReduce hide behind nc evals
            for it in range(max(NIT_NC, NIT_GLB)):
                if it < NIT_GLB:
                    stt2 = wk.tile([128, 4], FP32, tag=f"stt{it}", name=f"stt{it}")
                    cn_eval4(ctau, stt2)
                if it < NIT_NC:
                    nc_eval(tau, g_c)
                if it < NIT_GLB:
                    stg2 = wk.tile([128, 4], FP32, tag=f"stg{it}", name=f"stg{it}")
                    with tc.tile_critical():
                        nc.gpsimd.dma_start(out=st_l[it][:, :], in_=stt2).then_inc(cc_sem, 16)
                        ccv[0] += 16
                        nc.gpsimd.wait_ge(cc_sem, ccv[0])
                        nc.gpsimd.collective_compute(
                            "AllReduce", ALU.add, replica_groups=RG,
                            ins=[st_l[it].ap().opt()], outs=[st_a[it].ap().opt()]).then_inc(cc_sem)
                        ccv[0] += 1
                        nc.gpsimd.wait_ge(cc_sem, ccv[0])
                        nc.gpsimd.dma_start(out=stg2, in_=st_a[it][:, :]).then_inc(cc_sem, 16)
                        ccv[0] += 16
                        nc.gpsimd.wait_ge(cc_sem, ccv[0])
                if it < NIT_NC:
                    if debug:
                        nc.sync.dma_start(out=dbg['trace'][:, 2 * it, :], in_=g_c)
                    secant_update(tau, tau_p, g_c, g_p, "ncs", [128, NW])
                    if debug:
                        nc.sync.dma_start(out=dbg['trace'][:, 2 * it + 1, :], in_=tau)
                if it < NIT_GLB and debug:
                    ctr = wk.tile([128, 10], FP32, tag=f"ctr{it}", name=f"ctr{it}")
                    nc.vector.tensor_copy(ctr[:, 0:2], ctau)
                    nc.vector.tensor_copy(ctr[:, 2:6], stt2)
                    nc.vector.tensor_copy(ctr[:, 6:10], stg2)
                    nc.sync.dma_start(out=dbg['cntr'][:, it, :], in_=ctr)
                if it < NIT_GLB:
                    dfc = wk.tile([128, 2], FP32, tag=f"dfc{it}", name=f"dfc{it}")
                    nc.vector.tensor_sub(dfc, stg2[:, 0:2], stg2[:, 2:4])
                    nc.vector.tensor_scalar_max(dfc, dfc, 5e-4)
                    rdf = wk.tile([128, 2], FP32, tag=f"rdf{it}", name=f"rdf{it}")
                    nc.vector.reciprocal(rdf, dfc)
                    gm1 = wk.tile([128, 2], FP32, tag=f"gm1_{it}", name=f"gm1_{it}")
                    nc.vector.tensor_scalar_add(gm1, stg2[:, 0:2], -1.0)
                    stp = wk.tile([128, 2], FP32, tag=f"stp{it}", name=f"stp{it}")
                    nc.vector.tensor_mul(stp, gm1, rdf)
                    nc.vector.tensor_scalar(stp, stp, CN_DELTA, None, op0=ALU.mult)
                    nc.vector.tensor_scalar(stp, stp, -0.1, 1.0, op0=ALU.max, op1=ALU.min)
                    nc.vector.tensor_add(ctau, ctau, stp)
            if debug:
                nc.sync.dma_start(out=dbg['taunc'][:, :], in_=tau)
                nc.sync.dma_start(out=dbg['taucn'][:, :], in_=ctau)
            tau_b = wk.tile([128, NW], BF16, tag="tau_b")
            nc.vector.tensor_copy(tau_b, tau)
            nc.sync.dma_start(
                out=taunc_dram.ap().rearrange("w p -> p w"), in_=tau_b)

            # ===== phase 7: c_memory =====
            taucn_b = wk.tile([128, 2], BF16, tag="taucn_b")
            nc.vector.tensor_copy(taucn_b, ctau)
            nc.sync.dma_start(
                out=taucn_dram.ap().rearrange("m p -> p m"), in_=taucn_b)
            taucn_rep = const.tile([128, C], BF16)
            nc.sync.dma_start(out=taucn_rep, in_=_bcast_row(taucn_dram, C))

            ps_cms = [psA.tile([128, 128], FP32, tag=f"cmacc{m}", name=f"pscm{m}")
                      for m in range(2)]
            for ch in range(NW):
                # rp = relu(sim_node - taucn) computed in place in sim_node
                nc.vector.tensor_sub(sim_node[:, ch, :], sim_node[:, ch, :],
                                     taucn_rep)
                nc.vector.tensor_scalar_max(sim_node[:, ch, :],
                                            sim_node[:, ch, :], 0.0)
                for m in range(2):
                    nc.tensor.matmul(
                        ps_cms[m], sim_node[:, ch, m * 128:(m + 1) * 128],
                        nmem[:, ch, :], start=(ch == 0), stop=(ch == NW - 1))
            cmf = wk.tile([128, 2, 128], FP32, tag="cmf")
            for m in range(2):
                nc.vector.tensor_copy(cmf[:, m, :], ps_cms[m])
            cmgf = wk.tile([128, 2, 128], FP32, tag="cmgf")
            with tc.tile_critical():
                nc.gpsimd.dma_start(
                    out=cm_local.ap().rearrange("(m p) d -> p m d", p=128),
                    in_=cmf).then_inc(cc_sem, 16)
                ccv[0] += 16
                nc.gpsimd.wait_ge(cc_sem, ccv[0])
                nc.gpsimd.collective_compute(
                    "AllReduce", ALU.add, replica_groups=RG,
                    ins=[cm_local.ap().opt()], outs=[cm_all.ap().opt()]).then_inc(cc_sem)
                ccv[0] += 1
                nc.gpsimd.wait_ge(cc_sem, ccv[0])
                nc.gpsimd.dma_start(
                    out=cmgf,
                    in_=cm_all.ap().rearrange("(m p) d -> p m d", p=128)
                ).then_inc(cc_sem, 16)
                ccv[0] += 16
                nc.gpsimd.wait_ge(cc_sem, ccv[0])
        # nodep (sim_node) freed

        cmg = const.tile([128, 2, 128], BF16)
        nc.vector.tensor_copy(cmg, cmgf)
        if debug:
            nc.sync.dma_start(
                out=dbg['cmem'].ap().rearrange("(m p) d -> p m d", p=128),
                in_=cmgf)

        # ===== phase 8: emb =====
        with tc.tile_pool(name="tncp", bufs=1) as tncp:
            # ncm = relu(simT - taunc) computed in place in simT via two big
            # ops per m; taunc broadcast with a single stride-0 DMA
            tnc_all = tncp.tile([128, L], BF16)
            nc.sync.dma_start(out=tnc_all, in_=_bcast_row(taunc_dram, L))
            for m in range(2):
                nc.vector.tensor_sub(simT[:, m, :], simT[:, m, :], tnc_all)
                nc.vector.tensor_scalar_max(simT[:, m, :], simT[:, m, :], 0.0)
            for ch in range(NW):
                sl = bass.ds(ch * 128, 128)
                ps_z = psB.tile([128, 128], FP32, tag="b1")
                for m in range(2):
                    nc.tensor.matmul(ps_z, simT[:, m, sl], cmg[:, m, :],
                                     start=(m == 0), stop=(m == 1))
                emb_c = wk.tile([128, 128], FP32, tag="emb_c")
                nc.vector.tensor_add(emb_c, ps_z, nmem[:, ch, :])
                nc.sync.dma_start(out=emb_out[ch * 128:(ch + 1) * 128, :], in_=emb_c)

    split_waits(nc)
    return nc


# ----------------------------------------------------------------------------
# host side
# ----------------------------------------------------------------------------

_CACHE = {}


def _route(L, src, dst, t):
    idx = np.concatenate([src, dst]).astype(np.int64)
    other = np.concatenate([dst, src]).astype(np.int64)
    tt = np.concatenate([t, t])
    eidx = np.concatenate([np.arange(len(src)), np.arange(len(src))])
    NW = L // 128
    order = np.argsort(idx, kind='stable')
    idx_s, other_s, tt_s, eidx_s = idx[order], other[order], tt[order], eidx[order]
    owner = idx_s // L
    cores = []
    for c in range(NCORES):
        msk = owner == c
        li = idx_s[msk] - c * L
        win = li // 128
        col = li % 128
        wcount = np.bincount(win, minlength=NW)
        assert wcount.max() <= 256, f"window overflow: {wcount.max()}"
        woff = np.zeros(NW + 1, np.int64)
        woff[1:] = np.cumsum(wcount)
        within = np.arange(len(li)) - woff[win]
        slot = win * 256 + within
        cores.append(dict(slot=slot, col=col, li=li, other=other_s[msk],
                          tt=tt_s[msk], eidx=eidx_s[msk]))
    return cores


def kernel(**inputs):
    node_memory = np.asarray(inputs['node_memory'])
    last_update = np.asarray(inputs['last_update'])
    node_features = np.asarray(inputs['node_features'])
    event_feat = np.asarray(inputs['event_feat'])
    t = np.asarray(inputs['t'])
    src = np.asarray(inputs['src']).astype(np.int64)
    dst = np.asarray(inputs['dst']).astype(np.int64)
    time_w = np.asarray(inputs['time_w'])
    time_b = np.asarray(inputs['time_b'])
    W_ih = np.asarray(inputs['W_ih'])
    b_ih = np.asarray(inputs['b_ih'])
    W_hh = np.asarray(inputs['W_hh'])
    b_hh = np.asarray(inputs['b_hh'])
    proj_W = np.asarray(inputs['proj_W'])
    proj_b = np.asarray(inputs['proj_b'])
    centroids = np.asarray(inputs['centroids'])

    Nn = node_memory.shape[0]
    GW = 7
    gran = 128 * GW * NCORES          # L must be multiple of 128*GW
    NP = -(-Nn // gran) * gran
    L = NP // NCORES
    SLOTS = 2 * L
    NW = L // 128

    nmp = np.zeros((NP, D), np.float32); nmp[:Nn] = node_memory
    nfp = np.zeros((NP, D), np.float32); nfp[:Nn] = node_features
    lup = np.zeros(NP, np.float32); lup[:Nn] = last_update

    idx_full = np.concatenate([src, dst])
    cnt_full = np.bincount(idx_full, minlength=NP).astype(np.float32)
    icnt_full = 1.0 / np.maximum(cnt_full, 1.0)
    has_full = (cnt_full > 0).astype(np.float32)

    cores = _route(L, src, dst, t)
    bsum_h = f32c(np.stack([(b_ih + b_hh)[0:128], (b_ih + b_hh)[128:256]], 1))
    wih_h = bfc(W_ih.T.reshape(4, 128, 384).transpose(1, 0, 2))

    in_maps = []
    for c in range(NCORES):
        r = cores[c]
        sl = r['slot']
        ev_mo = np.zeros((SLOTS, D), ml_dtypes.bfloat16)
        ev_ef = np.zeros((SLOTS, F), ml_dtypes.bfloat16)
        ev_dt = np.zeros(SLOTS, np.float32)
        ev_col = np.full(SLOTS, -1.0, np.float32)
        ev_icnt = np.zeros(SLOTS, np.float32)
        ev_mo[sl] = nmp[r['other']].astype(ml_dtypes.bfloat16)
        ev_ef[sl] = event_feat[r['eidx']].astype(ml_dtypes.bfloat16)
        ev_dt[sl] = r['tt'] - lup[r['li'] + c * L]
        ev_col[sl] = r['col'].astype(np.float32)
        ev_icnt[sl] = icnt_full[r['li'] + c * L]
        nsl = slice(c * L, (c + 1) * L)
        in_maps.append({
            'memT': f32c(nmp[nsl].T),
            'mem_node': f32c(nmp[nsl]),
            'nfT': f32c(nfp[nsl].T),
            'has_colT': f32c(has_full[nsl].reshape(NW, 128).T),
            'ev_mo': ev_mo, 'ev_ef': ev_ef, 'ev_dt': ev_dt,
            'ev_col': ev_col, 'ev_icnt': ev_icnt,
            'W_ihT': wih_h,
            'W_hhT': bfc(W_hh.T),
            'bsum': bsum_h,
            'b_hh2': f32c(b_hh[256:384].reshape(128, 1)),
            'b_ih2': f32c(b_ih[256:384].reshape(128, 1)),
            'pWt': bfc(proj_W),
            'pb': f32c(proj_b.reshape(128, 1)),
            'cenT': f32c(centroids.T),
            'w_rep': f32c(np.tile(time_w[None, :], (128, 1))),
            'bpi_rep': f32c(np.tile(time_b[None, :] + HALF_PI, (128, 1))),
            'iota_t': f32c(np.tile(np.arange(128, dtype=np.float32)[None, :],
                                   (128, 1))),
            'core_oh_in': f32c(np.tile(np.eye(NCORES, dtype=np.float32)[c][None, :],
                                       (128, 1))),
        })

    debug = bool(int(os.environ.get("KERNEL_DEBUG", "0")))
    key = (L, debug)
    if key not in _CACHE:
        _CACHE[key] = build_program(L, debug=debug)
    nc = _CACHE[key]
    trace = bool(int(os.environ.get("KERNEL_TRACE", "0")))
    res = run_bass_kernel_spmd(nc, in_maps, list(range(NCORES)), trace=trace)
    emb = np.concatenate([res.results[c]['emb'] for c in range(NCORES)], 0)
    kernel._last_exec_ns = getattr(res, 'exec_time_ns', None)
    kernel._last_profile = getattr(res, 'profile_json', None)
    if debug:
        kernel._last_results = res.results
    return emb[:Nn].astype(np.float32)



# revision 50
# speedup vs baseline: 1.7186x; 1.1085x over previous
"""TGN-style GNN message passing + community detection on 8 TRN2 NeuronCores.

Node-sharded SPMD: nodes padded to 8*L and sharded contiguously; events
routed by host (index work only) to the owner core of their update target
and binned into 128-node windows (2x128 slots per window). Segment-mean via
inv-cnt-scaled one-hot matmuls on the PE; GRU/proj/sim as bf16 matmuls;
sparsemax taus via secant iterations on g(tau)=sum(relu(z-tau)) with an
AllGathered chunk-max warm start for the centroid direction; c_memory
partials AllReduced. All float arithmetic on device.
"""

import os
from contextlib import ExitStack

import numpy as np
import ml_dtypes

import concourse.bass as bass
import concourse.mybir as mybir
import concourse.tile as tile
from concourse.bass_utils import run_bass_kernel_spmd
from concourse.masks import make_identity

FP32 = mybir.dt.float32
BF16 = mybir.dt.bfloat16
AF = mybir.ActivationFunctionType
ALU = mybir.AluOpType
AX = mybir.AxisListType

NCORES = 8
D = 128
F = 128
T = 128
P = 128
C = 256
HALF_PI = float(np.pi / 2)

bfc = lambda x: np.ascontiguousarray(np.asarray(x).astype(ml_dtypes.bfloat16))
f32c = lambda x: np.ascontiguousarray(np.asarray(x).astype(np.float32))


def _bcast_row(dram_tensor, ncols, nparts=128, off=0):
    row = dram_tensor.ap()
    return bass.AP(tensor=row.tensor, offset=row.offset + off,
                   ap=[[0, nparts], [1, ncols]])


def split_waits(nc, sp_limit=1, default_limit=1):
    """This env's walrus rejects >1 sync-wait on SP CTRL instructions:
    move extra waits onto preceding NOPs."""
    limits = {mybir.EngineType.SP: sp_limit}
    for fn in nc.m.functions:
        for bb in fn.blocks:
            out = []
            for ins in bb.instructions:
                si = ins.sync_info
                w = list(si.on_wait) if (si is not None and si.on_wait) else []
                lim = limits.get(ins.engine, default_limit)
                if len(w) > lim:
                    extra, keep = w[:-lim], w[-lim:]
                    for j in range(0, len(extra), lim):
                        out.append(mybir.InstNoOp(
                            name=f"{ins.name}-ws{j}",
                            engine=ins.engine,
                            sync_info=mybir.SyncInfo(
                                on_wait=list(extra[j:j + lim]), on_update=[]),
                        ))
                    ins.sync_info = mybir.SyncInfo(
                        on_wait=list(keep),
                        on_update=list(si.on_update) if si.on_update else [])
                out.append(ins)
            bb.instructions = out
    return nc


def build_program(L, NIT_NC=7, NIT_MINI=16, NIT_GLB=6, debug=False):
    NW = L // 128
    SLOTS = 2 * L
    MGW = NW * NCORES
    # node batches of <=512 (PSUM bank limit), multiples of 128
    batches = []
    off = 0
    while off < L:
        bs_ = min(512, L - off)
        batches.append((off, bs_))
        off += bs_

    nc = bass.Bass(num_devices=NCORES)

    memT = nc.dram_tensor("memT", [128, L], FP32, kind="ExternalInput")
    mem_node = nc.dram_tensor("mem_node", [L, D], FP32, kind="ExternalInput")
    nfT = nc.dram_tensor("nfT", [128, L], FP32, kind="ExternalInput")
    has_colT = nc.dram_tensor("has_colT", [128, NW], FP32, kind="ExternalInput")
    ev_mo = nc.dram_tensor("ev_mo", [SLOTS, D], BF16, kind="ExternalInput")
    ev_ef = nc.dram_tensor("ev_ef", [SLOTS, F], BF16, kind="ExternalInput")
    ev_dt = nc.dram_tensor("ev_dt", [SLOTS], FP32, kind="ExternalInput")
    ev_col = nc.dram_tensor("ev_col", [SLOTS], FP32, kind="ExternalInput")
    ev_icnt = nc.dram_tensor("ev_icnt", [SLOTS], FP32, kind="ExternalInput")
    W_ihT = nc.dram_tensor("W_ihT", [128, 4, 384], BF16, kind="ExternalInput")
    W_hhT = nc.dram_tensor("W_hhT", [128, 384], BF16, kind="ExternalInput")
    bsum = nc.dram_tensor("bsum", [128, 2], FP32, kind="ExternalInput")
    b_hh2 = nc.dram_tensor("b_hh2", [128, 1], FP32, kind="ExternalInput")
    b_ih2 = nc.dram_tensor("b_ih2", [128, 1], FP32, kind="ExternalInput")
    pWt = nc.dram_tensor("pWt", [128, P], BF16, kind="ExternalInput")
    pb = nc.dram_tensor("pb", [128, 1], FP32, kind="ExternalInput")
    cenT = nc.dram_tensor("cenT", [128, C], FP32, kind="ExternalInput")
    w_rep = nc.dram_tensor("w_rep", [128, T], FP32, kind="ExternalInput")
    bpi_rep = nc.dram_tensor("bpi_rep", [128, T], FP32, kind="ExternalInput")
    iota_t = nc.dram_tensor("iota_t", [128, 128], FP32, kind="ExternalInput")

    emb_out = nc.dram_tensor("emb", [L, D], FP32, kind="ExternalOutput")
    dbg = {}
    if debug:
        dbg['newmem'] = nc.dram_tensor("dbg_newmem", [L, D], FP32, kind="ExternalOutput")
        dbg['simT'] = nc.dram_tensor("dbg_simT", [128, 2, L], BF16, kind="ExternalOutput")
        dbg['taunc'] = nc.dram_tensor("dbg_taunc", [128, NW], FP32, kind="ExternalOutput")
        dbg['taucn'] = nc.dram_tensor("dbg_taucn", [128, 2], FP32, kind="ExternalOutput")
        dbg['cmem'] = nc.dram_tensor("dbg_cmem", [C, D], FP32, kind="ExternalOutput")
        dbg['simnode'] = nc.dram_tensor("dbg_simnode", [128, NW, C], BF16, kind="ExternalOutput")
        dbg['g0'] = nc.dram_tensor("dbg_g0", [128, 4, NW], FP32, kind="ExternalOutput")
        dbg['trace'] = nc.dram_tensor("dbg_trace", [128, 2 * NIT_NC, NW], FP32, kind="ExternalOutput")
        dbg['cntr'] = nc.dram_tensor("dbg_cntr", [128, NIT_GLB, 10], FP32, kind="ExternalOutput")

    aggT_dram = nc.dram_tensor("aggT_dram", [128, 3, L], BF16)
    taunc_dram = nc.dram_tensor("taunc_dram", [NW, 128], BF16)
    rnorm_dram = nc.dram_tensor("rnorm_dram", [NW, 128], BF16)
    ssq_dram = nc.dram_tensor("ssq_dram", [1, L], FP32)
    crec_dram = nc.dram_tensor("crec_dram", [1, C], BF16)
    taucn_dram = nc.dram_tensor("taucn_dram", [2, 128], BF16)
    mg_local = nc.dram_tensor("mg_local", [NCORES, 2, 128, NW], FP32)
    mg_all = nc.dram_tensor("mg_all", [NCORES, 2, 128, NW], FP32, addr_space="Shared")
    st_l = [nc.dram_tensor(f"st_l{i}", [128, 4], FP32) for i in range(NIT_GLB)]
    st_a = [nc.dram_tensor(f"st_a{i}", [128, 4], FP32, addr_space="Shared")
            for i in range(NIT_GLB)]
    st_lm = nc.dram_tensor("st_lm", [128, 4], FP32)
    st_am = nc.dram_tensor("st_am", [128, 4], FP32, addr_space="Shared")
    cm_local = nc.dram_tensor("cm_local", [C, D], FP32)
    cm_all = nc.dram_tensor("cm_all", [C, D], FP32, addr_space="Shared")
    core_oh_in = nc.dram_tensor("core_oh_in", [128, NCORES], FP32, kind="ExternalInput")
    RG = [list(range(NCORES))]

    cc_sem = nc.alloc_semaphore("cc_done")
    ccv = [0]
    ctx = ExitStack()
    with tile.TileContext(nc) as tc, ctx:
        const = ctx.enter_context(tc.tile_pool(name="const", bufs=1))
        late = ctx.enter_context(tc.tile_pool(name="late", bufs=1))
        wk = ctx.enter_context(tc.tile_pool(name="wk", bufs=2))
        scr = ctx.enter_context(tc.tile_pool(name="scr", bufs=1))
        # PSUM: psA bufs=1 {acc3: 3 banks, g1: 1, cmacc: 1}; psB bufs=2 {b1: 2}
        psA = ctx.enter_context(tc.tile_pool(name="psA", bufs=1, space="PSUM"))
        psB = ctx.enter_context(tc.tile_pool(name="psB", bufs=2, space="PSUM"))

        # ----- constants -----
        ident = const.tile([128, 128], BF16)
        make_identity(nc, ident)
        identf = const.tile([128, 128], FP32)
        make_identity(nc, identf)
        iota = const.tile([128, 128], FP32)
        nc.sync.dma_start(out=iota, in_=iota_t[:, :])
        iotab = const.tile([128, 128], BF16)
        nc.vector.tensor_copy(iotab, iota)
        wih = const.tile([128, 4, 384], BF16)
        nc.sync.dma_start(out=wih, in_=W_ihT[:, :, :])
        whh = const.tile([128, 384], BF16)
        nc.sync.dma_start(out=whh, in_=W_hhT[:, :])
        bs = const.tile([128, 2], FP32)
        nc.sync.dma_start(out=bs, in_=bsum[:, :])
        bh2 = const.tile([128, 1], FP32)
        nc.sync.dma_start(out=bh2, in_=b_hh2[:, :])
        bi2 = const.tile([128, 1], FP32)
        nc.sync.dma_start(out=bi2, in_=b_ih2[:, :])
        pw = const.tile([128, P], BF16)
        nc.sync.dma_start(out=pw, in_=pWt[:, :])
        pbt = const.tile([128, 1], FP32)
        nc.sync.dma_start(out=pbt, in_=pb[:, :])
        wr = const.tile([128, T], FP32)
        nc.sync.dma_start(out=wr, in_=w_rep[:, :])
        br = const.tile([128, T], FP32)
        nc.sync.dma_start(out=br, in_=bpi_rep[:, :])
        hascol = const.tile([128, NW], FP32)
        nc.sync.dma_start(out=hascol, in_=has_colT[:, :])
        ones_col = const.tile([128, 1], BF16)
        nc.vector.memset(ones_col, 1.0)

        # centroid norms
        cen = const.tile([128, C], FP32)
        nc.sync.dma_start(out=cen, in_=cenT[:, :])
        censq = wk.tile([128, C], BF16, tag="censq")
        nc.vector.tensor_mul(censq, cen, cen)
        cnorm = wk.tile([1, C], FP32, tag="cnorm")
        ps_c = psB.tile([1, C], FP32, tag="b1")
        nc.tensor.matmul(ps_c, ones_col, censq, start=True, stop=True)
        nc.scalar.activation(cnorm, ps_c, AF.Sqrt)
        nc.vector.tensor_scalar_add(cnorm, cnorm, 1e-8)
        crec = wk.tile([1, C], FP32, tag="crec")
        nc.vector.reciprocal(crec, cnorm)
        crec_b = wk.tile([1, C], BF16, tag="crec_b")
        nc.vector.tensor_copy(crec_b, crec)
        nc.sync.dma_start(out=crec_dram[:, :], in_=crec_b)
        crec_rep = const.tile([128, C], BF16)
        nc.sync.dma_start(out=crec_rep, in_=_bcast_row(crec_dram, C))
        cennT = const.tile([128, C], BF16)
        nc.vector.tensor_mul(cennT, cen, crec_rep)

        # ----- long-lived tensors -----
        simT = late.tile([128, 2, L], BF16)
        nmem = late.tile([128, NW, 128], BF16)  # node-major new memory
        tau = late.tile([128, NW], FP32)
        tau_p = late.tile([128, NW], FP32)
        g_c = late.tile([128, NW], FP32)
        g_p = late.tile([128, NW], FP32)
        ctau = late.tile([128, 2], FP32)
        ctau_p = late.tile([128, 2], FP32)
        cg = late.tile([128, 2], FP32)
        cg_p = late.tile([128, 2], FP32)

        with tc.tile_pool(name="mid", bufs=1) as mid:
            featT = mid.tile([128, L], BF16)

            with tc.tile_pool(name="early", bufs=2) as early, \
                    tc.tile_pool(name="evp", bufs=1) as evp, \
                    tc.tile_pool(name="gruw", bufs=2) as gruw:

                # ===== phase 1: events -> aggT (staged to DRAM) =====
                GW = 7
                assert NW % GW == 0
                for grp in range(NW // GW):
                    w0 = grp * GW
                    xg = evp.tile([128, GW * 2, 384], BF16, tag="xg")
                    cols = evp.tile([128, GW * 2], FP32, tag="cols")
                    icnt = evp.tile([128, GW * 2], FP32, tag="icnt")
                    dts = evp.tile([128, GW * 2], FP32, tag="dts")
                    s0 = w0 * 256
                    n_ev = GW * 256
                    nc.sync.dma_start(
                        out=xg[:, :, 0:D],
                        in_=ev_mo[s0:s0 + n_ev, :].rearrange("(t p) d -> p t d", p=128))
                    nc.sync.dma_start(
                        out=xg[:, :, D:D + F],
                        in_=ev_ef[s0:s0 + n_ev, :].rearrange("(t p) d -> p t d", p=128))
                    nc.sync.dma_start(
                        out=cols, in_=ev_col[s0:s0 + n_ev].rearrange("(t p) -> p t", p=128))
                    nc.sync.dma_start(
                        out=icnt, in_=ev_icnt[s0:s0 + n_ev].rearrange("(t p) -> p t", p=128))
                    nc.sync.dma_start(
                        out=dts, in_=ev_dt[s0:s0 + n_ev].rearrange("(t p) -> p t", p=128))
                    for t_ in range(GW * 2):
                        ang = gruw.tile([128, T], FP32, tag="ang")
                        nc.vector.scalar_tensor_tensor(
                            ang, wr, dts[:, t_:t_ + 1], br, op0=ALU.mult, op1=ALU.add)
                        # range-reduce to [-pi, pi]: ang -= 2pi*round(ang/2pi)
                        # alternate the two magic-round ops between DVE/Scalar
                        mm_ = gruw.tile([128, T], FP32, tag="mm_")
                        if t_ % 2 == 0:
                            nc.vector.tensor_scalar(
                                mm_, ang, 1.0 / (2 * np.pi), 12582912.0,
                                op0=ALU.mult, op1=ALU.add)
                            nc.vector.tensor_scalar_add(mm_, mm_, -12582912.0)
                        else:
                            nc.scalar.activation(mm_, ang, AF.Copy,
                                                 bias=12582912.0,
                                                 scale=1.0 / (2 * np.pi))
                            nc.scalar.activation(mm_, mm_, AF.Copy,
                                                 bias=-12582912.0)
                        nc.vector.scalar_tensor_tensor(
                            ang, mm_, -2 * np.pi, ang, op0=ALU.mult, op1=ALU.add)
                        nc.scalar.activation(xg[:, t_, D + F:], ang, AF.Sin)
                    for wi in range(GW):
                        w = w0 + wi
                        psws = [psA.tile([128, 128], FP32, tag=f"aggfc{fc}",
                                          name=f"psw{fc}") for fc in range(3)]
                        for t_ in range(2):
                            ti = wi * 2 + t_
                            oh = gruw.tile([128, 128], BF16, tag="oh")
                            nc.vector.tensor_scalar(
                                oh, iotab, cols[:, ti:ti + 1], icnt[:, ti:ti + 1],
                                op0=ALU.is_equal, op1=ALU.mult)
                            for fc in range(3):
                                nc.tensor.matmul(
                                    psws[fc],
                                    xg[:, ti, fc * 128:(fc + 1) * 128],
                                    oh, start=(t_ == 0), stop=(t_ == 1))
                        awin = early.tile([128, 3, 128], BF16, tag="awin")
                        for fc in range(3):
                            if (wi + fc) % 2 == 0:
                                nc.vector.tensor_copy(awin[:, fc, :], psws[fc])
                            else:
                                nc.scalar.activation(awin[:, fc, :], psws[fc],
                                                     AF.Copy)
                        nc.sync.dma_start(
                            out=aggT_dram[:, :, w * 128:(w + 1) * 128], in_=awin)


                # ===== phase 2: GRU + newmem + featT =====
                for (boff, bsz) in batches:
                    sl = bass.ds(boff, bsz)
                    mTf = gruw.tile([128, bsz], FP32, tag="mTf")
                    nc.sync.dma_start(out=mTf, in_=memT[:, sl])
                    mTb = gruw.tile([128, bsz], BF16, tag="mTb")
                    nc.vector.tensor_copy(mTb, mTf)
                    agg_b = gruw.tile([128, 3, bsz], BF16, tag="agg_b")
                    nc.sync.dma_start(out=agg_b, in_=aggT_dram[:, :, sl])
                    gis = [psA.tile([128, bsz], FP32, tag=f"aggfc{m}",
                                    name=f"gi{m}") for m in range(3)]
                    gh2 = psA.tile([128, bsz], FP32, tag="g1")
                    for m in range(3):
                        nc.tensor.matmul(gis[m], wih[:, 0, m * 128:(m + 1) * 128],
                                         mTb, start=True, stop=False)
                        for k in range(1, 4):
                            last = (k == 3 and m >= 2)
                            nc.tensor.matmul(gis[m],
                                             wih[:, k, m * 128:(m + 1) * 128],
                                             agg_b[:, k - 1, :], start=False,
                                             stop=last)
                        if m < 2:
                            nc.tensor.matmul(gis[m], whh[:, m * 128:(m + 1) * 128],
                                             mTb, start=False, stop=True)
                    nc.tensor.matmul(gh2, whh[:, 256:384], mTb, start=True, stop=True)
                    # GRU elementwise with 5 rotating buffers (SBUF pressure)
                    r = gruw.tile([128, bsz], FP32, tag="ga", name="r")
                    nc.scalar.activation(r, gis[0], AF.Sigmoid, bias=bs[:, 0:1])
                    z = gruw.tile([128, bsz], FP32, tag="gb", name="z")
                    nc.scalar.activation(z, gis[1], AF.Sigmoid, bias=bs[:, 1:2])
                    gh2s = gruw.tile([128, bsz], FP32, tag="gc", name="gh2s")
                    nc.vector.tensor_scalar_add(gh2s, gh2, bh2[:, 0:1])
                    u = gruw.tile([128, bsz], FP32, tag="gd", name="u")
                    nc.vector.tensor_mul(u, r, gh2s)
                    v = gruw.tile([128, bsz], FP32, tag="ge", name="v")
                    nc.vector.tensor_add(v, u, gis[2])
                    n_g = gruw.tile([128, bsz], FP32, tag="gc", name="n_g")
                    nc.scalar.activation(n_g, v, AF.Tanh, bias=bi2[:, 0:1])
                    dmn = gruw.tile([128, bsz], FP32, tag="gd", name="dmn")
                    nc.vector.tensor_sub(dmn, mTf, n_g)
                    e_ = gruw.tile([128, bsz], FP32, tag="ga", name="e_")
                    nc.vector.tensor_mul(e_, z, dmn)
                    updT = gruw.tile([128, bsz], FP32, tag="ge", name="updT")
                    nc.vector.tensor_add(updT, n_g, e_)
                    for cc in range(bsz // 128):
                        ch = boff // 128 + cc
                        pst = psB.tile([128, 128], FP32, tag="b1")
                        nc.tensor.transpose(pst, updT[:, cc * 128:(cc + 1) * 128], identf)
                        mn = gruw.tile([128, 128], FP32, tag="mn")
                        nc.sync.dma_start(out=mn, in_=mem_node[ch * 128:(ch + 1) * 128, :])
                        d2 = gruw.tile([128, 128], FP32, tag="d2")
                        nc.vector.tensor_sub(d2, pst, mn)
                        # nmem = mn + has*(upd - mn), fused
                        nc.vector.scalar_tensor_tensor(
                            nmem[:, ch, :], d2, hascol[:, ch:ch + 1], mn,
                            op0=ALU.mult, op1=ALU.add)
                        if debug:
                            nc.gpsimd.dma_start(out=dbg['newmem'][ch * 128:(ch + 1) * 128, :],
                                                in_=nmem[:, ch, :])
                        pst2 = psA.tile([128, 128], BF16, tag="cmacc0", name="pst2")
                        nc.tensor.transpose(pst2, nmem[:, ch, :], ident)
                        nfc = gruw.tile([128, 128], FP32, tag="nfc")
                        nc.sync.dma_start(out=nfc, in_=nfT[:, ch * 128:(ch + 1) * 128])
                        nc.vector.tensor_add(featT[:, ch * 128:(ch + 1) * 128], pst2, nfc)
            # early pool (aggT) freed here

            # ===== phase 3: pf + norms + sim =====
            with tc.tile_pool(name="pfp", bufs=1) as pfp:
                pfT = pfp.tile([128, L], BF16)
                for (boff, bsz) in batches:
                    sl = bass.ds(boff, bsz)
                    psp = psB.tile([128, bsz], FP32, tag="b1")
                    nc.tensor.matmul(psp, pw, featT[:, sl], start=True, stop=True)
                    pfc = wk.tile([128, bsz], FP32, tag="pfc")
                    nc.vector.tensor_scalar_add(pfc, psp, pbt[:, 0:1])
                    nc.vector.tensor_copy(pfT[:, sl], pfc)
                    sq = wk.tile([128, bsz], BF16, tag="sq")
                    nc.vector.tensor_mul(sq, pfc, pfc)
                    ps_s = psB.tile([1, bsz], FP32, tag="b1")
                    nc.tensor.matmul(ps_s, ones_col, sq, start=True, stop=True)
                    sqe = wk.tile([1, bsz], FP32, tag="sqe")
                    nc.vector.tensor_copy(sqe, ps_s)
                    nc.sync.dma_start(out=ssq_dram[0, sl], in_=sqe)
                ssq_t = wk.tile([128, NW], FP32, tag="ssq_t")
                nc.sync.dma_start(
                    out=ssq_t,
                    in_=ssq_dram.ap().rearrange("o (w p) -> (o p) w", p=128))
                sns = wk.tile([128, NW], FP32, tag="sns")
                nc.scalar.activation(sns, ssq_t, AF.Sqrt)
                nc.vector.tensor_scalar_add(sns, sns, 1e-8)
                rn_t = wk.tile([128, NW], FP32, tag="rn_t")
                nc.vector.reciprocal(rn_t, sns)
                rn_b = wk.tile([128, NW], BF16, tag="rn_b")
                nc.vector.tensor_copy(rn_b, rn_t)
                nc.sync.dma_start(
                    out=rnorm_dram.ap().rearrange("w p -> p w"), in_=rn_b)
                for (boff, bsz) in batches:
                    sl = bass.ds(boff, bsz)
                    rn_rep = wk.tile([128, bsz], BF16, tag="rn_rep")
                    nc.sync.dma_start(out=rn_rep,
                                      in_=_bcast_row(rnorm_dram, bsz, off=boff))
                    for m in range(2):
                        ps_m = psB.tile([128, bsz], FP32, tag="b1")
                        nc.tensor.matmul(ps_m, cennT[:, m * 128:(m + 1) * 128],
                                         pfT[:, sl], start=True, stop=True)
                        nc.vector.tensor_mul(simT[:, m, sl], ps_m, rn_rep)
                if debug:
                    nc.sync.dma_start(out=dbg['simT'][:, :, :], in_=simT)
        # mid pool (featT) freed

        with tc.tile_pool(name="nodep", bufs=1) as nodep:
            # sim_node via PE transposes; copyback split Vector/Scalar
            sim_node = nodep.tile([128, NW, C], BF16)
            for ch in range(NW):
                for m in range(2):
                    pstr = psB.tile([128, 128], BF16, tag="b1")
                    nc.tensor.transpose(pstr, simT[:, m, ch * 128:(ch + 1) * 128], ident)
                    if (2 * ch + m) % 2 == 0:
                        nc.vector.tensor_copy(sim_node[:, ch, m * 128:(m + 1) * 128], pstr)
                    else:
                        nc.scalar.activation(sim_node[:, ch, m * 128:(m + 1) * 128],
                                             pstr, AF.Copy)
            if debug:
                nc.sync.dma_start(out=dbg['simnode'][:, :, :], in_=sim_node)

            # ===== phase 5+6: interleaved nc (per-node) & cn (global) sparsemax
            # Both evals are single fused relu+accumulate tensor_scalar ops.
            # nc windows split across DVE and Pool; cn runs on DVE with one
            # AllReduce per probe-Newton iteration, warm-started from the
            # global row max.
            junk_v = scr.tile([128, C], BF16, tag="junk_v")
            junk_p = scr.tile([128, C], BF16, tag="junk_p")
            ngt = scr.tile([128, NW], FP32, tag="ngt")
            nc.vector.tensor_reduce(tau_p, sim_node, axis=AX.X, op=ALU.max)
            nc.vector.tensor_scalar_add(tau_p, tau_p, -1.0)

            # nc windows mostly on DVE max-trick (C=256-term sums: offset bias
            # ~1e-3, fine) with a Scalar tail for balance; cn chunks all on
            # Scalar exact relu-accum (3136-term max-trick sums carry ~0.1-0.5
            # fp32 truncation bias - too noisy)
            NWH = NW - NW // 6

            def nc_eval(tau_tile, g_tile):
                # DVE: acc = sum(max(sim, tau)) = g + C*tau in ONE fused op;
                # Scalar: activation(Relu, bias=-tau) accumulates exact g.
                if NWH < NW:
                    nc.vector.tensor_scalar_mul(ngt[:, NWH:], tau_tile[:, NWH:], -1.0)
                for ch in range(NW):
                    if ch < NWH:
                        nc.vector.tensor_scalar(
                            junk_v, sim_node[:, ch, :],
                            tau_tile[:, ch:ch + 1], None,
                            op0=ALU.max, op1=ALU.add,
                            accum_out=g_tile[:, ch:ch + 1])
                    else:
                        nc.scalar.activation(
                            junk_p, sim_node[:, ch, :], AF.Relu,
                            bias=ngt[:, ch:ch + 1],
                            accum_out=g_tile[:, ch:ch + 1])
                # strip the C*tau offset from the DVE half: g -= C*tau
                nc.vector.scalar_tensor_tensor(
                    g_tile[:, 0:NWH], tau_tile[:, 0:NWH], -float(C),
                    g_tile[:, 0:NWH], op0=ALU.mult, op1=ALU.add)

            nc_eval(tau_p, g_p)
            if debug:
                nc.sync.dma_start(out=dbg['g0'][:, 0, :], in_=g_p)
            st1 = wk.tile([128, NW], FP32, tag="st1")
            nc.vector.tensor_scalar(st1, g_p, -1.0, 1.0 / 256.0,
                                    op0=ALU.add, op1=ALU.mult)
            nc.vector.tensor_add(tau, tau_p, st1)
            if debug:
                nc.sync.dma_start(out=dbg['g0'][:, 1, :], in_=tau)

            def secant_update(tt, tp, gg, gp, wtag, shape):
                num = wk.tile(shape, FP32, tag=wtag + "n")
                nc.vector.tensor_sub(num, tt, tp)
                gm1 = wk.tile(shape, FP32, tag=wtag + "g")
                nc.vector.tensor_scalar_add(gm1, gg, -1.0)
                nc.vector.tensor_mul(num, num, gm1)
                den = wk.tile(shape, FP32, tag=wtag + "d")
                nc.vector.tensor_sub(den, gp, gg)
                # floor guards against den collapse at convergence: tiny den
                # with positive g-noise would clip the step to +1 (overshoot)
                nc.vector.tensor_scalar_max(den, den, 1e-3)
                rden = wk.tile(shape, FP32, tag=wtag + "r")
                nc.vector.reciprocal(rden, den)
                nc.vector.tensor_copy(tp, tt)
                nc.vector.tensor_copy(gp, gg)
                stp = wk.tile(shape, FP32, tag=wtag + "s")
                nc.vector.tensor_mul(stp, num, rden)
                # monotone safeguard: secant from below must step in [0, 1]
                nc.vector.tensor_scalar(stp, stp, 0.0, 1.0,
                                        op0=ALU.max, op1=ALU.min)
                nc.vector.tensor_add(tt, tt, stp)

            # cn eval: fused relu+accum over 4 chunks of simT, split DVE/Scalar
            CNC = L // 4
            cn_junk = scr.tile([128, CNC], BF16, tag="cn_junk")
            cn_junk_s = scr.tile([128, CNC], BF16, tag="cn_junk_s")
            gp4 = scr.tile([128, 4, 4], FP32, tag="gp4")
            CN_DELTA = 1e-3
            CN_WARM = 0.15  # global rowmax - tau* is < 0.19 for this data;
            # the step clip allows downward correction so a high start recovers

            def cn_eval4(tt, out4):
                # out4 columns: [g(t)_m0, g(t)_m1, g(t+d)_m0, g(t+d)_m1]
                ngc = wk.tile([128, 4], FP32, tag="ngc")
                nc.vector.tensor_scalar_mul(ngc[:, 0:2], tt, -1.0)
                nc.vector.tensor_scalar(ngc[:, 2:4], tt, -1.0, -CN_DELTA,
                                        op0=ALU.mult, op1=ALU.add)
                for m in range(2):
                    for pi in (0, 2):
                        col = pi + m
                        for j in range(4):
                            jt = cn_junk_s if j % 2 else cn_junk
                            nc.scalar.activation(
                                jt, simT[:, m, bass.ds(j * CNC, CNC)],
                                AF.Relu, bias=ngc[:, col:col + 1],
                                accum_out=gp4[:, col, j:j + 1])
                        nc.vector.tensor_reduce(
                            out4[:, col:col + 1], gp4[:, col, :],
                            axis=AX.X, op=ALU.add)

            # global row max via AllReduce(max)
            rm4 = wk.tile([128, 4], FP32, tag="rm4")
            nc.vector.tensor_reduce(rm4[:, 0:2], simT, axis=AX.X, op=ALU.max)
            nc.vector.tensor_copy(rm4[:, 2:4], rm4[:, 0:2])
            rmg = wk.tile([128, 4], FP32, tag="rmg")
            with tc.tile_critical():
                nc.gpsimd.dma_start(out=st_lm[:, :], in_=rm4).then_inc(cc_sem, 16)
                ccv[0] += 16
                nc.gpsimd.wait_ge(cc_sem, ccv[0])
                nc.gpsimd.collective_compute(
                    "AllReduce", ALU.max, replica_groups=RG,
                    ins=[st_lm.ap().opt()], outs=[st_am.ap().opt()]).then_inc(cc_sem)
                ccv[0] += 1
                nc.gpsimd.wait_ge(cc_sem, ccv[0])
                nc.gpsimd.dma_start(out=rmg, in_=st_am[:, :]).then_inc(cc_sem, 16)
                ccv[0] += 16
                nc.gpsimd.wait_ge(cc_sem, ccv[0])
            nc.vector.tensor_scalar_add(ctau, rmg[:, 0:2], -CN_WARM)

            # interleave: cn probe evals + AllReduce hide behind nc evals
            for it in range(max(NIT_NC, NIT_GLB)):
                if it < NIT_GLB:
                    stt2 = wk.tile([128, 4], FP32, tag=f"stt{it}", name=f"stt{it}")
                    cn_eval4(ctau, stt2)
                if it < NIT_NC:
                    nc_eval(tau, g_c)
                if it < NIT_GLB:
                    stg2 = wk.tile([128, 4], FP32, tag=f"stg{it}", name=f"stg{it}")
                    with tc.tile_critical():
                        nc.gpsimd.dma_start(out=st_l[it][:, :], in_=stt2).then_inc(cc_sem, 16)
                        ccv[0] += 16
                        nc.gpsimd.wait_ge(cc_sem, ccv[0])
                        nc.gpsimd.collective_compute(
                            "AllReduce", ALU.add, replica_groups=RG,
                            ins=[st_l[it].ap().opt()], outs=[st_a[it].ap().opt()]).then_inc(cc_sem)
                        ccv[0] += 1
                        nc.gpsimd.wait_ge(cc_sem, ccv[0])
                        nc.gpsimd.dma_start(out=stg2, in_=st_a[it][:, :]).then_inc(cc_sem, 16)
                        ccv[0] += 16
                        nc.gpsimd.wait_ge(cc_sem, ccv[0])
                if it < NIT_NC:
                    if debug:
                        nc.sync.dma_start(out=dbg['trace'][:, 2 * it, :], in_=g_c)
                    secant_update(tau, tau_p, g_c, g_p, "ncs", [128, NW])
                    if debug:
                        nc.sync.dma_start(out=dbg['trace'][:, 2 * it + 1, :], in_=tau)
                if it < NIT_GLB and debug:
                    ctr = wk.tile([128, 10], FP32, tag=f"ctr{it}", name=f"ctr{it}")
                    nc.vector.tensor_copy(ctr[:, 0:2], ctau)
                    nc.vector.tensor_copy(ctr[:, 2:6], stt2)
                    nc.vector.tensor_copy(ctr[:, 6:10], stg2)
                    nc.sync.dma_start(out=dbg['cntr'][:, it, :], in_=ctr)
                if it < NIT_GLB:
                    dfc = wk.tile([128, 2], FP32, tag=f"dfc{it}", name=f"dfc{it}")
                    nc.vector.tensor_sub(dfc, stg2[:, 0:2], stg2[:, 2:4])
                    nc.vector.tensor_scalar_max(dfc, dfc, 5e-4)
                    rdf = wk.tile([128, 2], FP32, tag=f"rdf{it}", name=f"rdf{it}")
                    nc.vector.reciprocal(rdf, dfc)
                    gm1 = wk.tile([128, 2], FP32, tag=f"gm1_{it}", name=f"gm1_{it}")
                    nc.vector.tensor_scalar_add(gm1, stg2[:, 0:2], -1.0)
                    stp = wk.tile([128, 2], FP32, tag=f"stp{it}", name=f"stp{it}")
                    nc.vector.tensor_mul(stp, gm1, rdf)
                    nc.vector.tensor_scalar(stp, stp, CN_DELTA, None, op0=ALU.mult)
                    nc.vector.tensor_scalar(stp, stp, -0.1, 1.0, op0=ALU.max, op1=ALU.min)
                    nc.vector.tensor_add(ctau, ctau, stp)
            if debug:
                nc.sync.dma_start(out=dbg['taunc'][:, :], in_=tau)
                nc.sync.dma_start(out=dbg['taucn'][:, :], in_=ctau)
            tau_b = wk.tile([128, NW], BF16, tag="tau_b")
            nc.vector.tensor_copy(tau_b, tau)
            nc.sync.dma_start(
                out=taunc_dram.ap().rearrange("w p -> p w"), in_=tau_b)

            # ===== phase 7: c_memory =====
            taucn_b = wk.tile([128, 2], BF16, tag="taucn_b")
            nc.vector.tensor_copy(taucn_b, ctau)
            nc.sync.dma_start(
                out=taucn_dram.ap().rearrange("m p -> p m"), in_=taucn_b)
            taucn_rep = const.tile([128, C], BF16)
            nc.sync.dma_start(out=taucn_rep, in_=_bcast_row(taucn_dram, C))

            ps_cms = [psA.tile([128, 128], FP32, tag=f"cmacc{m}", name=f"pscm{m}")
                      for m in range(2)]
            for ch in range(NW):
                # rp = relu(sim_node - taucn) computed in place in sim_node
                nc.vector.tensor_sub(sim_node[:, ch, :], sim_node[:, ch, :],
                                     taucn_rep)
                nc.vector.tensor_scalar_max(sim_node[:, ch, :],
                                            sim_node[:, ch, :], 0.0)
                for m in range(2):
                    nc.tensor.matmul(
                        ps_cms[m], sim_node[:, ch, m * 128:(m + 1) * 128],
                        nmem[:, ch, :], start=(ch == 0), stop=(ch == NW - 1))
            cmf = wk.tile([128, 2, 128], FP32, tag="cmf")
            for m in range(2):
                nc.vector.tensor_copy(cmf[:, m, :], ps_cms[m])
            cmgf = wk.tile([128, 2, 128], FP32, tag="cmgf")
            with tc.tile_critical():
                nc.gpsimd.dma_start(
                    out=cm_local.ap().rearrange("(m p) d -> p m d", p=128),
                    in_=cmf).then_inc(cc_sem, 16)
                ccv[0] += 16
                nc.gpsimd.wait_ge(cc_sem, ccv[0])
                nc.gpsimd.collective_compute(
                    "AllReduce", ALU.add, replica_groups=RG,
                    ins=[cm_local.ap().opt()], outs=[cm_all.ap().opt()]).then_inc(cc_sem)
                ccv[0] += 1
                nc.gpsimd.wait_ge(cc_sem, ccv[0])
                nc.gpsimd.dma_start(
                    out=cmgf,
                    in_=cm_all.ap().rearrange("(m p) d -> p m d", p=128)
                ).then_inc(cc_sem, 16)
                ccv[0] += 16
                nc.gpsimd.wait_ge(cc_sem, ccv[0])
        # nodep (sim_node) freed

        cmg = const.tile([128, 2, 128], BF16)
        nc.vector.tensor_copy(cmg, cmgf)
        if debug:
            nc.sync.dma_start(
                out=dbg['cmem'].ap().rearrange("(m p) d -> p m d", p=128),
                in_=cmgf)

        # ===== phase 8: emb =====
        with tc.tile_pool(name="tncp", bufs=1) as tncp:
            # ncm = relu(simT - taunc) computed in place in simT via two big
            # ops per m; taunc broadcast with a single stride-0 DMA
            tnc_all = tncp.tile([128, L], BF16)
            nc.sync.dma_start(out=tnc_all, in_=_bcast_row(taunc_dram, L))
            for m in range(2):
                nc.vector.tensor_sub(simT[:, m, :], simT[:, m, :], tnc_all)
                nc.vector.tensor_scalar_max(simT[:, m, :], simT[:, m, :], 0.0)
            for ch in range(NW):
                sl = bass.ds(ch * 128, 128)
                ps_z = psB.tile([128, 128], FP32, tag="b1")
                for m in range(2):
                    nc.tensor.matmul(ps_z, simT[:, m, sl], cmg[:, m, :],
                                     start=(m == 0), stop=(m == 1))
                emb_c = wk.tile([128, 128], FP32, tag="emb_c")
                nc.vector.tensor_add(emb_c, ps_z, nmem[:, ch, :])
                nc.sync.dma_start(out=emb_out[ch * 128:(ch + 1) * 128, :], in_=emb_c)

    split_waits(nc)
    return nc


# ----------------------------------------------------------------------------
# host side
# ----------------------------------------------------------------------------

_CACHE = {}


def _route(L, src, dst, t):
    idx = np.concatenate([src, dst]).astype(np.int64)
    other = np.concatenate([dst, src]).astype(np.int64)
    tt = np.concatenate([t, t])
    eidx = np.concatenate([np.arange(len(src)), np.arange(len(src))])
    NW = L // 128
    order = np.argsort(idx, kind='stable')
    idx_s, other_s, tt_s, eidx_s = idx[order], other[order], tt[order], eidx[order]
    owner = idx_s // L
    cores = []
    for c in range(NCORES):
        msk = owner == c
        li = idx_s[msk] - c * L
        win = li // 128
        col = li % 128
        wcount = np.bincount(win, minlength=NW)
        assert wcount.max() <= 256, f"window overflow: {wcount.max()}"
        woff = np.zeros(NW + 1, np.int64)
        woff[1:] = np.cumsum(wcount)
        within = np.arange(len(li)) - woff[win]
        slot = win * 256 + within
        cores.append(dict(slot=slot, col=col, li=li, other=other_s[msk],
                          tt=tt_s[msk], eidx=eidx_s[msk]))
    return cores


def kernel(**inputs):
    node_memory = np.asarray(inputs['node_memory'])
    last_update = np.asarray(inputs['last_update'])
    node_features = np.asarray(inputs['node_features'])
    event_feat = np.asarray(inputs['event_feat'])
    t = np.asarray(inputs['t'])
    src = np.asarray(inputs['src']).astype(np.int64)
    dst = np.asarray(inputs['dst']).astype(np.int64)
    time_w = np.asarray(inputs['time_w'])
    time_b = np.asarray(inputs['time_b'])
    W_ih = np.asarray(inputs['W_ih'])
    b_ih = np.asarray(inputs['b_ih'])
    W_hh = np.asarray(inputs['W_hh'])
    b_hh = np.asarray(inputs['b_hh'])
    proj_W = np.asarray(inputs['proj_W'])
    proj_b = np.asarray(inputs['proj_b'])
    centroids = np.asarray(inputs['centroids'])

    Nn = node_memory.shape[0]
    GW = 7
    gran = 128 * GW * NCORES          # L must be multiple of 128*GW
    NP = -(-Nn // gran) * gran
    L = NP // NCORES
    SLOTS = 2 * L
    NW = L // 128

    nmp = np.zeros((NP, D), np.float32); nmp[:Nn] = node_memory
    nfp = np.zeros((NP, D), np.float32); nfp[:Nn] = node_features
    lup = np.zeros(NP, np.float32); lup[:Nn] = last_update

    idx_full = np.concatenate([src, dst])
    cnt_full = np.bincount(idx_full, minlength=NP).astype(np.float32)
    icnt_full = 1.0 / np.maximum(cnt_full, 1.0)
    has_full = (cnt_full > 0).astype(np.float32)

    cores = _route(L, src, dst, t)
    bsum_h = f32c(np.stack([(b_ih + b_hh)[0:128], (b_ih + b_hh)[128:256]], 1))
    wih_h = bfc(W_ih.T.reshape(4, 128, 384).transpose(1, 0, 2))

    in_maps = []
    for c in range(NCORES):
        r = cores[c]
        sl = r['slot']
        ev_mo = np.zeros((SLOTS, D), ml_dtypes.bfloat16)
        ev_ef = np.zeros((SLOTS, F), ml_dtypes.bfloat16)
        ev_dt = np.zeros(SLOTS, np.float32)
        ev_col = np.full(SLOTS, -1.0, np.float32)
        ev_icnt = np.zeros(SLOTS, np.float32)
        ev_mo[sl] = nmp[r['other']].astype(ml_dtypes.bfloat16)
        ev_ef[sl] = event_feat[r['eidx']].astype(ml_dtypes.bfloat16)
        ev_dt[sl] = r['tt'] - lup[r['li'] + c * L]
        ev_col[sl] = r['col'].astype(np.float32)
        ev_icnt[sl] = icnt_full[r['li'] + c * L]
        nsl = slice(c * L, (c + 1) * L)
        in_maps.append({
            'memT': f32c(nmp[nsl].T),
            'mem_node': f32c(nmp[nsl]),
            'nfT': f32c(nfp[nsl].T),
            'has_colT': f32c(has_full[nsl].reshape(NW, 128).T),
            'ev_mo': ev_mo, 'ev_ef': ev_ef, 'ev_dt': ev_dt,
            'ev_col': ev_col, 'ev_icnt': ev_icnt,
            'W_ihT': wih_h,
            'W_hhT': bfc(W_hh.T),
            'bsum': bsum_h,
            'b_hh2': f32c(b_hh[256:384].reshape(128, 1)),
            'b_ih2': f32c(b_ih[256:384].reshape(128, 1)),
            'pWt': bfc(proj_W),
            'pb': f32c(proj_b.reshape(128, 1)),
            'cenT': f32c(centroids.T),
            'w_rep': f32c(np.tile(time_w[None, :], (128, 1))),
            'bpi_rep': f32c(np.tile(time_b[None, :] + HALF_PI, (128, 1))),
            'iota_t': f32c(np.tile(np.arange(128, dtype=np.float32)[None, :],
                                   (128, 1))),
            'core_oh_in': f32c(np.tile(np.eye(NCORES, dtype=np.float32)[c][None, :],
                                       (128, 1))),
        })

    debug = bool(int(os.environ.get("KERNEL_DEBUG", "0")))
    key = (L, debug)
    if key not in _CACHE:
        _CACHE[key] = build_program(L, debug=debug)
    nc = _CACHE[key]
    trace = bool(int(os.environ.get("KERNEL_TRACE", "0")))
    res = run_bass_kernel_spmd(nc, in_maps, list(range(NCORES)), trace=trace)
    emb = np.concatenate([res.results[c]['emb'] for c in range(NCORES)], 0)
    kernel._last_exec_ns = getattr(res, 'exec_time_ns', None)
    kernel._last_profile = getattr(res, 'profile_json', None)
    if debug:
        kernel._last_results = res.results
    return emb[:Nn].astype(np.float32)



# revision 53
# speedup vs baseline: 1.7721x; 1.0312x over previous
"""TGN-style GNN message passing + community detection on 8 TRN2 NeuronCores.

Node-sharded SPMD: nodes padded to 8*L and sharded contiguously; events
routed by host (index work only) to the owner core of their update target
and binned into 128-node windows (2x128 slots per window). Segment-mean via
inv-cnt-scaled one-hot matmuls on the PE; GRU/proj/sim as bf16 matmuls;
sparsemax taus via secant iterations on g(tau)=sum(relu(z-tau)) with an
AllGathered chunk-max warm start for the centroid direction; c_memory
partials AllReduced. All float arithmetic on device.
"""

import os
from contextlib import ExitStack

import numpy as np
import ml_dtypes

import concourse.bass as bass
import concourse.mybir as mybir
import concourse.tile as tile
from concourse.bass_utils import run_bass_kernel_spmd
from concourse.masks import make_identity

FP32 = mybir.dt.float32
BF16 = mybir.dt.bfloat16
AF = mybir.ActivationFunctionType
ALU = mybir.AluOpType
AX = mybir.AxisListType

NCORES = 8
D = 128
F = 128
T = 128
P = 128
C = 256
HALF_PI = float(np.pi / 2)

bfc = lambda x: np.ascontiguousarray(np.asarray(x).astype(ml_dtypes.bfloat16))
f32c = lambda x: np.ascontiguousarray(np.asarray(x).astype(np.float32))


def _bcast_row(dram_tensor, ncols, nparts=128, off=0):
    row = dram_tensor.ap()
    return bass.AP(tensor=row.tensor, offset=row.offset + off,
                   ap=[[0, nparts], [1, ncols]])


def split_waits(nc, sp_limit=1, default_limit=1):
    """This env's walrus rejects >1 sync-wait on SP CTRL instructions:
    move extra waits onto preceding NOPs."""
    limits = {mybir.EngineType.SP: sp_limit}
    for fn in nc.m.functions:
        for bb in fn.blocks:
            out = []
            for ins in bb.instructions:
                si = ins.sync_info
                w = list(si.on_wait) if (si is not None and si.on_wait) else []
                lim = limits.get(ins.engine, default_limit)
                if len(w) > lim:
                    extra, keep = w[:-lim], w[-lim:]
                    for j in range(0, len(extra), lim):
                        out.append(mybir.InstNoOp(
                            name=f"{ins.name}-ws{j}",
                            engine=ins.engine,
                            sync_info=mybir.SyncInfo(
                                on_wait=list(extra[j:j + lim]), on_update=[]),
                        ))
                    ins.sync_info = mybir.SyncInfo(
                        on_wait=list(keep),
                        on_update=list(si.on_update) if si.on_update else [])
                out.append(ins)
            bb.instructions = out
    return nc


def build_program(L, NIT_NC=6, NIT_MINI=16, NIT_GLB=5, debug=False):
    NW = L // 128
    SLOTS = 2 * L
    MGW = NW * NCORES
    # node batches of <=512 (PSUM bank limit), multiples of 128
    batches = []
    off = 0
    while off < L:
        bs_ = min(512, L - off)
        batches.append((off, bs_))
        off += bs_

    nc = bass.Bass(num_devices=NCORES)

    memT = nc.dram_tensor("memT", [128, L], FP32, kind="ExternalInput")
    mem_node = nc.dram_tensor("mem_node", [L, D], FP32, kind="ExternalInput")
    nfT = nc.dram_tensor("nfT", [128, L], FP32, kind="ExternalInput")
    has_colT = nc.dram_tensor("has_colT", [128, NW], FP32, kind="ExternalInput")
    ev_mo = nc.dram_tensor("ev_mo", [SLOTS, D], BF16, kind="ExternalInput")
    ev_ef = nc.dram_tensor("ev_ef", [SLOTS, F], BF16, kind="ExternalInput")
    ev_dt = nc.dram_tensor("ev_dt", [SLOTS], FP32, kind="ExternalInput")
    ev_col = nc.dram_tensor("ev_col", [SLOTS], FP32, kind="ExternalInput")
    ev_icnt = nc.dram_tensor("ev_icnt", [SLOTS], FP32, kind="ExternalInput")
    W_ihT = nc.dram_tensor("W_ihT", [128, 4, 384], BF16, kind="ExternalInput")
    W_hhT = nc.dram_tensor("W_hhT", [128, 384], BF16, kind="ExternalInput")
    bsum = nc.dram_tensor("bsum", [128, 2], FP32, kind="ExternalInput")
    b_hh2 = nc.dram_tensor("b_hh2", [128, 1], FP32, kind="ExternalInput")
    b_ih2 = nc.dram_tensor("b_ih2", [128, 1], FP32, kind="ExternalInput")
    pWt = nc.dram_tensor("pWt", [128, P], BF16, kind="ExternalInput")
    pb = nc.dram_tensor("pb", [128, 1], FP32, kind="ExternalInput")
    cenT = nc.dram_tensor("cenT", [128, C], FP32, kind="ExternalInput")
    w_rep = nc.dram_tensor("w_rep", [128, T], FP32, kind="ExternalInput")
    bpi_rep = nc.dram_tensor("bpi_rep", [128, T], FP32, kind="ExternalInput")
    iota_t = nc.dram_tensor("iota_t", [128, 128], FP32, kind="ExternalInput")

    emb_out = nc.dram_tensor("emb", [L, D], FP32, kind="ExternalOutput")
    dbg = {}
    if debug:
        dbg['newmem'] = nc.dram_tensor("dbg_newmem", [L, D], FP32, kind="ExternalOutput")
        dbg['simT'] = nc.dram_tensor("dbg_simT", [128, 2, L], BF16, kind="ExternalOutput")
        dbg['taunc'] = nc.dram_tensor("dbg_taunc", [128, NW], FP32, kind="ExternalOutput")
        dbg['taucn'] = nc.dram_tensor("dbg_taucn", [128, 2], FP32, kind="ExternalOutput")
        dbg['cmem'] = nc.dram_tensor("dbg_cmem", [C, D], FP32, kind="ExternalOutput")
        dbg['simnode'] = nc.dram_tensor("dbg_simnode", [128, NW, C], BF16, kind="ExternalOutput")
        dbg['g0'] = nc.dram_tensor("dbg_g0", [128, 4, NW], FP32, kind="ExternalOutput")
        dbg['trace'] = nc.dram_tensor("dbg_trace", [128, 2 * NIT_NC, NW], FP32, kind="ExternalOutput")
        dbg['cntr'] = nc.dram_tensor("dbg_cntr", [128, NIT_GLB, 10], FP32, kind="ExternalOutput")

    aggT_dram = nc.dram_tensor("aggT_dram", [128, 3, L], BF16)
    taunc_dram = nc.dram_tensor("taunc_dram", [NW, 128], BF16)
    rnorm_dram = nc.dram_tensor("rnorm_dram", [NW, 128], BF16)
    ssq_dram = nc.dram_tensor("ssq_dram", [1, L], FP32)
    crec_dram = nc.dram_tensor("crec_dram", [1, C], BF16)
    taucn_dram = nc.dram_tensor("taucn_dram", [2, 128], BF16)
    mg_local = nc.dram_tensor("mg_local", [NCORES, 2, 128, NW], FP32)
    mg_all = nc.dram_tensor("mg_all", [NCORES, 2, 128, NW], FP32, addr_space="Shared")
    st_l = [nc.dram_tensor(f"st_l{i}", [128, 4], FP32) for i in range(NIT_GLB)]
    st_a = [nc.dram_tensor(f"st_a{i}", [128, 4], FP32, addr_space="Shared")
            for i in range(NIT_GLB)]
    st_lm = nc.dram_tensor("st_lm", [128, 4], FP32)
    st_am = nc.dram_tensor("st_am", [128, 4], FP32, addr_space="Shared")
    cm_local = nc.dram_tensor("cm_local", [C, D], FP32)
    cm_all = nc.dram_tensor("cm_all", [C, D], FP32, addr_space="Shared")
    core_oh_in = nc.dram_tensor("core_oh_in", [128, NCORES], FP32, kind="ExternalInput")
    RG = [list(range(NCORES))]

    cc_sem = nc.alloc_semaphore("cc_done")
    ccv = [0]
    ctx = ExitStack()
    with tile.TileContext(nc) as tc, ctx:
        const = ctx.enter_context(tc.tile_pool(name="const", bufs=1))
        late = ctx.enter_context(tc.tile_pool(name="late", bufs=1))
        wk = ctx.enter_context(tc.tile_pool(name="wk", bufs=2))
        scr = ctx.enter_context(tc.tile_pool(name="scr", bufs=1))
        # PSUM: psA bufs=1 {acc3: 3 banks, g1: 1, cmacc: 1}; psB bufs=2 {b1: 2}
        psA = ctx.enter_context(tc.tile_pool(name="psA", bufs=1, space="PSUM"))
        psB = ctx.enter_context(tc.tile_pool(name="psB", bufs=2, space="PSUM"))

        # ----- constants -----
        ident = const.tile([128, 128], BF16)
        make_identity(nc, ident)
        identf = const.tile([128, 128], FP32)
        make_identity(nc, identf)
        iota = const.tile([128, 128], FP32)
        nc.sync.dma_start(out=iota, in_=iota_t[:, :])
        iotab = const.tile([128, 128], BF16)
        nc.vector.tensor_copy(iotab, iota)
        wih = const.tile([128, 4, 384], BF16)
        nc.sync.dma_start(out=wih, in_=W_ihT[:, :, :])
        whh = const.tile([128, 384], BF16)
        nc.sync.dma_start(out=whh, in_=W_hhT[:, :])
        bs = const.tile([128, 2], FP32)
        nc.sync.dma_start(out=bs, in_=bsum[:, :])
        bh2 = const.tile([128, 1], FP32)
        nc.sync.dma_start(out=bh2, in_=b_hh2[:, :])
        bi2 = const.tile([128, 1], FP32)
        nc.sync.dma_start(out=bi2, in_=b_ih2[:, :])
        pw = const.tile([128, P], BF16)
        nc.sync.dma_start(out=pw, in_=pWt[:, :])
        pbt = const.tile([128, 1], FP32)
        nc.sync.dma_start(out=pbt, in_=pb[:, :])
        wr = const.tile([128, T], FP32)
        nc.sync.dma_start(out=wr, in_=w_rep[:, :])
        br = const.tile([128, T], FP32)
        nc.sync.dma_start(out=br, in_=bpi_rep[:, :])
        hascol = const.tile([128, NW], FP32)
        nc.sync.dma_start(out=hascol, in_=has_colT[:, :])
        ones_col = const.tile([128, 1], BF16)
        nc.vector.memset(ones_col, 1.0)

        # centroid norms
        cen = const.tile([128, C], FP32)
        nc.sync.dma_start(out=cen, in_=cenT[:, :])
        censq = wk.tile([128, C], BF16, tag="censq")
        nc.vector.tensor_mul(censq, cen, cen)
        cnorm = wk.tile([1, C], FP32, tag="cnorm")
        ps_c = psB.tile([1, C], FP32, tag="b1")
        nc.tensor.matmul(ps_c, ones_col, censq, start=True, stop=True)
        nc.scalar.activation(cnorm, ps_c, AF.Sqrt)
        nc.vector.tensor_scalar_add(cnorm, cnorm, 1e-8)
        crec = wk.tile([1, C], FP32, tag="crec")
        nc.vector.reciprocal(crec, cnorm)
        crec_b = wk.tile([1, C], BF16, tag="crec_b")
        nc.vector.tensor_copy(crec_b, crec)
        nc.sync.dma_start(out=crec_dram[:, :], in_=crec_b)
        crec_rep = const.tile([128, C], BF16)
        nc.sync.dma_start(out=crec_rep, in_=_bcast_row(crec_dram, C))
        cennT = const.tile([128, C], BF16)
        nc.vector.tensor_mul(cennT, cen, crec_rep)

        # ----- long-lived tensors -----
        simT = late.tile([128, 2, L], BF16)
        nmem = late.tile([128, NW, 128], BF16)  # node-major new memory
        tau = late.tile([128, NW], FP32)
        tau_p = late.tile([128, NW], FP32)
        g_c = late.tile([128, NW], FP32)
        g_p = late.tile([128, NW], FP32)
        ctau = late.tile([128, 2], FP32)
        ctau_p = late.tile([128, 2], FP32)
        cg = late.tile([128, 2], FP32)
        cg_p = late.tile([128, 2], FP32)

        with tc.tile_pool(name="mid", bufs=1) as mid:
            featT = mid.tile([128, L], BF16)

            with tc.tile_pool(name="early", bufs=2) as early, \
                    tc.tile_pool(name="evp", bufs=1) as evp, \
                    tc.tile_pool(name="gruw", bufs=2) as gruw:

                # ===== phase 1: events -> aggT (staged to DRAM) =====
                GW = 7
                assert NW % GW == 0
                for grp in range(NW // GW):
                    w0 = grp * GW
                    xg = evp.tile([128, GW * 2, 384], BF16, tag="xg")
                    cols = evp.tile([128, GW * 2], FP32, tag="cols")
                    icnt = evp.tile([128, GW * 2], FP32, tag="icnt")
                    dts = evp.tile([128, GW * 2], FP32, tag="dts")
                    s0 = w0 * 256
                    n_ev = GW * 256
                    nc.sync.dma_start(
                        out=xg[:, :, 0:D],
                        in_=ev_mo[s0:s0 + n_ev, :].rearrange("(t p) d -> p t d", p=128))
                    nc.sync.dma_start(
                        out=xg[:, :, D:D + F],
                        in_=ev_ef[s0:s0 + n_ev, :].rearrange("(t p) d -> p t d", p=128))
                    nc.sync.dma_start(
                        out=cols, in_=ev_col[s0:s0 + n_ev].rearrange("(t p) -> p t", p=128))
                    nc.sync.dma_start(
                        out=icnt, in_=ev_icnt[s0:s0 + n_ev].rearrange("(t p) -> p t", p=128))
                    nc.sync.dma_start(
                        out=dts, in_=ev_dt[s0:s0 + n_ev].rearrange("(t p) -> p t", p=128))
                    for t_ in range(GW * 2):
                        ang = gruw.tile([128, T], FP32, tag="ang")
                        nc.vector.scalar_tensor_tensor(
                            ang, wr, dts[:, t_:t_ + 1], br, op0=ALU.mult, op1=ALU.add)
                        # range-reduce to [-pi, pi]: ang -= 2pi*round(ang/2pi)
                        # alternate the two magic-round ops between DVE/Scalar
                        mm_ = gruw.tile([128, T], FP32, tag="mm_")
                        if t_ % 2 == 0:
                            nc.vector.tensor_scalar(
                                mm_, ang, 1.0 / (2 * np.pi), 12582912.0,
                                op0=ALU.mult, op1=ALU.add)
                            nc.vector.tensor_scalar_add(mm_, mm_, -12582912.0)
                        else:
                            nc.scalar.activation(mm_, ang, AF.Copy,
                                                 bias=12582912.0,
                                                 scale=1.0 / (2 * np.pi))
                            nc.scalar.activation(mm_, mm_, AF.Copy,
                                                 bias=-12582912.0)
                        nc.vector.scalar_tensor_tensor(
                            ang, mm_, -2 * np.pi, ang, op0=ALU.mult, op1=ALU.add)
                        nc.scalar.activation(xg[:, t_, D + F:], ang, AF.Sin)
                    for wi in range(GW):
                        w = w0 + wi
                        psws = [psA.tile([128, 128], FP32, tag=f"aggfc{fc}",
                                          name=f"psw{fc}") for fc in range(3)]
                        for t_ in range(2):
                            ti = wi * 2 + t_
                            oh = gruw.tile([128, 128], BF16, tag="oh")
                            nc.vector.tensor_scalar(
                                oh, iotab, cols[:, ti:ti + 1], icnt[:, ti:ti + 1],
                                op0=ALU.is_equal, op1=ALU.mult)
                            for fc in range(3):
                                nc.tensor.matmul(
                                    psws[fc],
                                    xg[:, ti, fc * 128:(fc + 1) * 128],
                                    oh, start=(t_ == 0), stop=(t_ == 1))
                        awin = early.tile([128, 3, 128], BF16, tag="awin")
                        for fc in range(3):
                            if (wi + fc) % 2 == 0:
                                nc.vector.tensor_copy(awin[:, fc, :], psws[fc])
                            else:
                                nc.scalar.activation(awin[:, fc, :], psws[fc],
                                                     AF.Copy)
                        nc.sync.dma_start(
                            out=aggT_dram[:, :, w * 128:(w + 1) * 128], in_=awin)


                # ===== phase 2: GRU + newmem + featT =====
                for (boff, bsz) in batches:
                    sl = bass.ds(boff, bsz)
                    mTf = gruw.tile([128, bsz], FP32, tag="mTf")
                    nc.sync.dma_start(out=mTf, in_=memT[:, sl])
                    mTb = gruw.tile([128, bsz], BF16, tag="mTb")
                    nc.vector.tensor_copy(mTb, mTf)
                    agg_b = gruw.tile([128, 3, bsz], BF16, tag="agg_b")
                    nc.sync.dma_start(out=agg_b, in_=aggT_dram[:, :, sl])
                    gis = [psA.tile([128, bsz], FP32, tag=f"aggfc{m}",
                                    name=f"gi{m}") for m in range(3)]
                    gh2 = psA.tile([128, bsz], FP32, tag="g1")
                    for m in range(3):
                        nc.tensor.matmul(gis[m], wih[:, 0, m * 128:(m + 1) * 128],
                                         mTb, start=True, stop=False)
                        for k in range(1, 4):
                            last = (k == 3 and m >= 2)
                            nc.tensor.matmul(gis[m],
                                             wih[:, k, m * 128:(m + 1) * 128],
                                             agg_b[:, k - 1, :], start=False,
                                             stop=last)
                        if m < 2:
                            nc.tensor.matmul(gis[m], whh[:, m * 128:(m + 1) * 128],
                                             mTb, start=False, stop=True)
                    nc.tensor.matmul(gh2, whh[:, 256:384], mTb, start=True, stop=True)
                    # GRU elementwise with 5 rotating buffers (SBUF pressure)
                    r = gruw.tile([128, bsz], FP32, tag="ga", name="r")
                    nc.scalar.activation(r, gis[0], AF.Sigmoid, bias=bs[:, 0:1])
                    z = gruw.tile([128, bsz], FP32, tag="gb", name="z")
                    nc.scalar.activation(z, gis[1], AF.Sigmoid, bias=bs[:, 1:2])
                    gh2s = gruw.tile([128, bsz], FP32, tag="gc", name="gh2s")
                    nc.vector.tensor_scalar_add(gh2s, gh2, bh2[:, 0:1])
                    u = gruw.tile([128, bsz], FP32, tag="gd", name="u")
                    nc.vector.tensor_mul(u, r, gh2s)
                    v = gruw.tile([128, bsz], FP32, tag="ge", name="v")
                    nc.vector.tensor_add(v, u, gis[2])
                    n_g = gruw.tile([128, bsz], FP32, tag="gc", name="n_g")
                    nc.scalar.activation(n_g, v, AF.Tanh, bias=bi2[:, 0:1])
                    dmn = gruw.tile([128, bsz], FP32, tag="gd", name="dmn")
                    nc.vector.tensor_sub(dmn, mTf, n_g)
                    e_ = gruw.tile([128, bsz], FP32, tag="ga", name="e_")
                    nc.vector.tensor_mul(e_, z, dmn)
                    updT = gruw.tile([128, bsz], FP32, tag="ge", name="updT")
                    nc.vector.tensor_add(updT, n_g, e_)
                    for cc in range(bsz // 128):
                        ch = boff // 128 + cc
                        pst = psB.tile([128, 128], FP32, tag="b1")
                        nc.tensor.transpose(pst, updT[:, cc * 128:(cc + 1) * 128], identf)
                        mn = gruw.tile([128, 128], FP32, tag="mn")
                        nc.sync.dma_start(out=mn, in_=mem_node[ch * 128:(ch + 1) * 128, :])
                        d2 = gruw.tile([128, 128], FP32, tag="d2")
                        nc.vector.tensor_sub(d2, pst, mn)
                        # nmem = mn + has*(upd - mn), fused
                        nc.vector.scalar_tensor_tensor(
                            nmem[:, ch, :], d2, hascol[:, ch:ch + 1], mn,
                            op0=ALU.mult, op1=ALU.add)
                        if debug:
                            nc.gpsimd.dma_start(out=dbg['newmem'][ch * 128:(ch + 1) * 128, :],
                                                in_=nmem[:, ch, :])
                        pst2 = psA.tile([128, 128], BF16, tag="cmacc0", name="pst2")
                        nc.tensor.transpose(pst2, nmem[:, ch, :], ident)
                        nfc = gruw.tile([128, 128], FP32, tag="nfc")
                        nc.sync.dma_start(out=nfc, in_=nfT[:, ch * 128:(ch + 1) * 128])
                        nc.vector.tensor_add(featT[:, ch * 128:(ch + 1) * 128], pst2, nfc)
            # early pool (aggT) freed here

            # ===== phase 3: pf + norms + sim =====
            with tc.tile_pool(name="pfp", bufs=1) as pfp:
                pfT = pfp.tile([128, L], BF16)
                for (boff, bsz) in batches:
                    sl = bass.ds(boff, bsz)
                    psp = psB.tile([128, bsz], FP32, tag="b1")
                    nc.tensor.matmul(psp, pw, featT[:, sl], start=True, stop=True)
                    pfc = wk.tile([128, bsz], FP32, tag="pfc")
                    nc.vector.tensor_scalar_add(pfc, psp, pbt[:, 0:1])
                    nc.vector.tensor_copy(pfT[:, sl], pfc)
                    sq = wk.tile([128, bsz], BF16, tag="sq")
                    nc.vector.tensor_mul(sq, pfc, pfc)
                    ps_s = psB.tile([1, bsz], FP32, tag="b1")
                    nc.tensor.matmul(ps_s, ones_col, sq, start=True, stop=True)
                    sqe = wk.tile([1, bsz], FP32, tag="sqe")
                    nc.vector.tensor_copy(sqe, ps_s)
                    nc.sync.dma_start(out=ssq_dram[0, sl], in_=sqe)
                ssq_t = wk.tile([128, NW], FP32, tag="ssq_t")
                nc.sync.dma_start(
                    out=ssq_t,
                    in_=ssq_dram.ap().rearrange("o (w p) -> (o p) w", p=128))
                sns = wk.tile([128, NW], FP32, tag="sns")
                nc.scalar.activation(sns, ssq_t, AF.Sqrt)
                nc.vector.tensor_scalar_add(sns, sns, 1e-8)
                rn_t = wk.tile([128, NW], FP32, tag="rn_t")
                nc.vector.reciprocal(rn_t, sns)
                rn_b = wk.tile([128, NW], BF16, tag="rn_b")
                nc.vector.tensor_copy(rn_b, rn_t)
                nc.sync.dma_start(
                    out=rnorm_dram.ap().rearrange("w p -> p w"), in_=rn_b)
                for (boff, bsz) in batches:
                    sl = bass.ds(boff, bsz)
                    rn_rep = wk.tile([128, bsz], BF16, tag="rn_rep")
                    nc.sync.dma_start(out=rn_rep,
                                      in_=_bcast_row(rnorm_dram, bsz, off=boff))
                    for m in range(2):
                        ps_m = psB.tile([128, bsz], FP32, tag="b1")
                        nc.tensor.matmul(ps_m, cennT[:, m * 128:(m + 1) * 128],
                                         pfT[:, sl], start=True, stop=True)
                        nc.vector.tensor_mul(simT[:, m, sl], ps_m, rn_rep)
                if debug:
                    nc.sync.dma_start(out=dbg['simT'][:, :, :], in_=simT)
        # mid pool (featT) freed

        with tc.tile_pool(name="nodep", bufs=1) as nodep:
            # sim_node via PE transposes; copyback split Vector/Scalar
            sim_node = nodep.tile([128, NW, C], BF16)
            for ch in range(NW):
                for m in range(2):
                    pstr = psB.tile([128, 128], BF16, tag="b1")
                    nc.tensor.transpose(pstr, simT[:, m, ch * 128:(ch + 1) * 128], ident)
                    if (2 * ch + m) % 2 == 0:
                        nc.vector.tensor_copy(sim_node[:, ch, m * 128:(m + 1) * 128], pstr)
                    else:
                        nc.scalar.activation(sim_node[:, ch, m * 128:(m + 1) * 128],
                                             pstr, AF.Copy)
            if debug:
                nc.sync.dma_start(out=dbg['simnode'][:, :, :], in_=sim_node)

            # ===== phase 5+6: interleaved nc (per-node) & cn (global) sparsemax
            # Both evals are single fused relu+accumulate tensor_scalar ops.
            # nc windows split across DVE and Pool; cn runs on DVE with one
            # AllReduce per probe-Newton iteration, warm-started from the
            # global row max.
            junk_v = scr.tile([128, C], BF16, tag="junk_v")
            junk_p = scr.tile([128, C], BF16, tag="junk_p")
            ngt = scr.tile([128, NW], FP32, tag="ngt")
            nc.vector.tensor_reduce(tau_p, sim_node, axis=AX.X, op=ALU.max)
            nc.vector.tensor_scalar_add(tau_p, tau_p, -1.0)

            # nc windows mostly on DVE max-trick (C=256-term sums: offset bias
            # ~1e-3, fine) with a Scalar tail for balance; cn chunks all on
            # Scalar exact relu-accum (3136-term max-trick sums carry ~0.1-0.5
            # fp32 truncation bias - too noisy)
            NWH = NW - NW // 8

            def nc_eval(tau_tile, g_tile):
                # DVE: acc = sum(max(sim, tau)) = g + C*tau in ONE fused op;
                # Scalar: activation(Relu, bias=-tau) accumulates exact g.
                if NWH < NW:
                    nc.vector.tensor_scalar_mul(ngt[:, NWH:], tau_tile[:, NWH:], -1.0)
                for ch in range(NW):
                    if ch < NWH:
                        nc.vector.tensor_scalar(
                            junk_v, sim_node[:, ch, :],
                            tau_tile[:, ch:ch + 1], None,
                            op0=ALU.max, op1=ALU.add,
                            accum_out=g_tile[:, ch:ch + 1])
                    else:
                        nc.scalar.activation(
                            junk_p, sim_node[:, ch, :], AF.Relu,
                            bias=ngt[:, ch:ch + 1],
                            accum_out=g_tile[:, ch:ch + 1])
                # strip the C*tau offset from the DVE half: g -= C*tau
                nc.vector.scalar_tensor_tensor(
                    g_tile[:, 0:NWH], tau_tile[:, 0:NWH], -float(C),
                    g_tile[:, 0:NWH], op0=ALU.mult, op1=ALU.add)

            nc_eval(tau_p, g_p)
            if debug:
                nc.sync.dma_start(out=dbg['g0'][:, 0, :], in_=g_p)
            st1 = wk.tile([128, NW], FP32, tag="st1")
            nc.vector.tensor_scalar(st1, g_p, -1.0, 1.0 / 256.0,
                                    op0=ALU.add, op1=ALU.mult)
            nc.vector.tensor_add(tau, tau_p, st1)
            if debug:
                nc.sync.dma_start(out=dbg['g0'][:, 1, :], in_=tau)

            def secant_update(tt, tp, gg, gp, wtag, shape):
                num = wk.tile(shape, FP32, tag=wtag + "n")
                nc.vector.tensor_sub(num, tt, tp)
                gm1 = wk.tile(shape, FP32, tag=wtag + "g")
                nc.vector.tensor_scalar_add(gm1, gg, -1.0)
                nc.vector.tensor_mul(num, num, gm1)
                den = wk.tile(shape, FP32, tag=wtag + "d")
                nc.vector.tensor_sub(den, gp, gg)
                # floor guards against den collapse at convergence: tiny den
                # with positive g-noise would clip the step to +1 (overshoot)
                nc.vector.tensor_scalar_max(den, den, 1e-3)
                rden = wk.tile(shape, FP32, tag=wtag + "r")
                nc.vector.reciprocal(rden, den)
                nc.vector.tensor_copy(tp, tt)
                nc.vector.tensor_copy(gp, gg)
                stp = wk.tile(shape, FP32, tag=wtag + "s")
                nc.vector.tensor_mul(stp, num, rden)
                # monotone safeguard: secant from below must step in [0, 1]
                nc.vector.tensor_scalar(stp, stp, 0.0, 1.0,
                                        op0=ALU.max, op1=ALU.min)
                nc.vector.tensor_add(tt, tt, stp)

            # cn eval: fused relu+accum over 4 chunks of simT, split DVE/Scalar
            CNC = L // 4
            cn_junk = scr.tile([128, CNC], BF16, tag="cn_junk")
            cn_junk_s = scr.tile([128, CNC], BF16, tag="cn_junk_s")
            gp4 = scr.tile([128, 4, 4], FP32, tag="gp4")
            CN_DELTA = 1e-3
            CN_WARM = 0.15  # global rowmax - tau* is < 0.19 for this data;
            # the step clip allows downward correction so a high start recovers

            def cn_eval4(tt, out4):
                # out4 columns: [g(t)_m0, g(t)_m1, g(t+d)_m0, g(t+d)_m1]
                ngc = wk.tile([128, 4], FP32, tag="ngc")
                nc.vector.tensor_scalar_mul(ngc[:, 0:2], tt, -1.0)
                nc.vector.tensor_scalar(ngc[:, 2:4], tt, -1.0, -CN_DELTA,
                                        op0=ALU.mult, op1=ALU.add)
                for m in range(2):
                    for pi in (0, 2):
                        col = pi + m
                        for j in range(4):
                            jt = cn_junk_s if j % 2 else cn_junk
                            nc.scalar.activation(
                                jt, simT[:, m, bass.ds(j * CNC, CNC)],
                                AF.Relu, bias=ngc[:, col:col + 1],
                                accum_out=gp4[:, col, j:j + 1])
                        nc.vector.tensor_reduce(
                            out4[:, col:col + 1], gp4[:, col, :],
                            axis=AX.X, op=ALU.add)

            # global row max via AllReduce(max)
            rm4 = wk.tile([128, 4], FP32, tag="rm4")
            nc.vector.tensor_reduce(rm4[:, 0:2], simT, axis=AX.X, op=ALU.max)
            nc.vector.tensor_copy(rm4[:, 2:4], rm4[:, 0:2])
            rmg = wk.tile([128, 4], FP32, tag="rmg")
            with tc.tile_critical():
                nc.gpsimd.dma_start(out=st_lm[:, :], in_=rm4).then_inc(cc_sem, 16)
                ccv[0] += 16
                nc.gpsimd.wait_ge(cc_sem, ccv[0])
                nc.gpsimd.collective_compute(
                    "AllReduce", ALU.max, replica_groups=RG,
                    ins=[st_lm.ap().opt()], outs=[st_am.ap().opt()]).then_inc(cc_sem)
                ccv[0] += 1
                nc.gpsimd.wait_ge(cc_sem, ccv[0])
                nc.gpsimd.dma_start(out=rmg, in_=st_am[:, :]).then_inc(cc_sem, 16)
                ccv[0] += 16
                nc.gpsimd.wait_ge(cc_sem, ccv[0])
            nc.vector.tensor_scalar_add(ctau, rmg[:, 0:2], -CN_WARM)

            # interleave: cn probe evals + AllReduce hide behind nc evals
            for it in range(max(NIT_NC, NIT_GLB)):
                if it < NIT_GLB:
                    stt2 = wk.tile([128, 4], FP32, tag=f"stt{it}", name=f"stt{it}")
                    cn_eval4(ctau, stt2)
                if it < NIT_NC:
                    nc_eval(tau, g_c)
                if it < NIT_GLB:
                    stg2 = wk.tile([128, 4], FP32, tag=f"stg{it}", name=f"stg{it}")
                    with tc.tile_critical():
                        nc.gpsimd.dma_start(out=st_l[it][:, :], in_=stt2).then_inc(cc_sem, 16)
                        ccv[0] += 16
                        nc.gpsimd.wait_ge(cc_sem, ccv[0])
                        nc.gpsimd.collective_compute(
                            "AllReduce", ALU.add, replica_groups=RG,
                            ins=[st_l[it].ap().opt()], outs=[st_a[it].ap().opt()]).then_inc(cc_sem)
                        ccv[0] += 1
                        nc.gpsimd.wait_ge(cc_sem, ccv[0])
                        nc.gpsimd.dma_start(out=stg2, in_=st_a[it][:, :]).then_inc(cc_sem, 16)
                        ccv[0] += 16
                        nc.gpsimd.wait_ge(cc_sem, ccv[0])
                if it < NIT_NC:
                    if debug:
                        nc.sync.dma_start(out=dbg['trace'][:, 2 * it, :], in_=g_c)
                    secant_update(tau, tau_p, g_c, g_p, "ncs", [128, NW])
                    if debug:
                        nc.sync.dma_start(out=dbg['trace'][:, 2 * it + 1, :], in_=tau)
                if it < NIT_GLB and debug:
                    ctr = wk.tile([128, 10], FP32, tag=f"ctr{it}", name=f"ctr{it}")
                    nc.vector.tensor_copy(ctr[:, 0:2], ctau)
                    nc.vector.tensor_copy(ctr[:, 2:6], stt2)
                    nc.vector.tensor_copy(ctr[:, 6:10], stg2)
                    nc.sync.dma_start(out=dbg['cntr'][:, it, :], in_=ctr)
                if it < NIT_GLB:
                    dfc = wk.tile([128, 2], FP32, tag=f"dfc{it}", name=f"dfc{it}")
                    nc.vector.tensor_sub(dfc, stg2[:, 0:2], stg2[:, 2:4])
                    nc.vector.tensor_scalar_max(dfc, dfc, 5e-4)
                    rdf = wk.tile([128, 2], FP32, tag=f"rdf{it}", name=f"rdf{it}")
                    nc.vector.reciprocal(rdf, dfc)
                    gm1 = wk.tile([128, 2], FP32, tag=f"gm1_{it}", name=f"gm1_{it}")
                    nc.vector.tensor_scalar_add(gm1, stg2[:, 0:2], -1.0)
                    stp = wk.tile([128, 2], FP32, tag=f"stp{it}", name=f"stp{it}")
                    nc.vector.tensor_mul(stp, gm1, rdf)
                    nc.vector.tensor_scalar(stp, stp, CN_DELTA, None, op0=ALU.mult)
                    nc.vector.tensor_scalar(stp, stp, -0.1, 1.0, op0=ALU.max, op1=ALU.min)
                    nc.vector.tensor_add(ctau, ctau, stp)
            if debug:
                nc.sync.dma_start(out=dbg['taunc'][:, :], in_=tau)
                nc.sync.dma_start(out=dbg['taucn'][:, :], in_=ctau)
            tau_b = wk.tile([128, NW], BF16, tag="tau_b")
            nc.vector.tensor_copy(tau_b, tau)
            nc.sync.dma_start(
                out=taunc_dram.ap().rearrange("w p -> p w"), in_=tau_b)

            # ===== phase 7: c_memory =====
            taucn_b = wk.tile([128, 2], BF16, tag="taucn_b")
            nc.vector.tensor_copy(taucn_b, ctau)
            nc.sync.dma_start(
                out=taucn_dram.ap().rearrange("m p -> p m"), in_=taucn_b)
            taucn_rep = const.tile([128, C], BF16)
            nc.sync.dma_start(out=taucn_rep, in_=_bcast_row(taucn_dram, C))

            ps_cms = [psA.tile([128, 128], FP32, tag=f"cmacc{m}", name=f"pscm{m}")
                      for m in range(2)]
            for ch in range(NW):
                # rp = relu(sim_node - taucn) computed in place in sim_node
                nc.vector.tensor_sub(sim_node[:, ch, :], sim_node[:, ch, :],
                                     taucn_rep)
                nc.vector.tensor_scalar_max(sim_node[:, ch, :],
                                            sim_node[:, ch, :], 0.0)
                for m in range(2):
                    nc.tensor.matmul(
                        ps_cms[m], sim_node[:, ch, m * 128:(m + 1) * 128],
                        nmem[:, ch, :], start=(ch == 0), stop=(ch == NW - 1))
            cmf = wk.tile([128, 2, 128], FP32, tag="cmf")
            for m in range(2):
                nc.vector.tensor_copy(cmf[:, m, :], ps_cms[m])
            cmgf = wk.tile([128, 2, 128], FP32, tag="cmgf")
            # phase-8 prep overlaps the c_memory AllReduce: ncm = relu(simT -
            # taunc) in place in simT (depends only on taunc)
            tnc_all = nodep.tile([128, L], BF16)
            nc.sync.dma_start(out=tnc_all, in_=_bcast_row(taunc_dram, L))
            for m in range(2):
                nc.vector.tensor_sub(simT[:, m, :], simT[:, m, :], tnc_all)
                nc.vector.tensor_scalar_max(simT[:, m, :], simT[:, m, :], 0.0)
            with tc.tile_critical():
                nc.gpsimd.dma_start(
                    out=cm_local.ap().rearrange("(m p) d -> p m d", p=128),
                    in_=cmf).then_inc(cc_sem, 16)
                ccv[0] += 16
                nc.gpsimd.wait_ge(cc_sem, ccv[0])
                nc.gpsimd.collective_compute(
                    "AllReduce", ALU.add, replica_groups=RG,
                    ins=[cm_local.ap().opt()], outs=[cm_all.ap().opt()]).then_inc(cc_sem)
                ccv[0] += 1
                nc.gpsimd.wait_ge(cc_sem, ccv[0])
                nc.gpsimd.dma_start(
                    out=cmgf,
                    in_=cm_all.ap().rearrange("(m p) d -> p m d", p=128)
                ).then_inc(cc_sem, 16)
                ccv[0] += 16
                nc.gpsimd.wait_ge(cc_sem, ccv[0])
        # nodep (sim_node) freed

        cmg = const.tile([128, 2, 128], BF16)
        nc.vector.tensor_copy(cmg, cmgf)
        if debug:
            nc.sync.dma_start(
                out=dbg['cmem'].ap().rearrange("(m p) d -> p m d", p=128),
                in_=cmgf)

        # ===== phase 8: emb =====
        for ch in range(NW):
            sl = bass.ds(ch * 128, 128)
            ps_z = psB.tile([128, 128], FP32, tag="b1")
            for m in range(2):
                nc.tensor.matmul(ps_z, simT[:, m, sl], cmg[:, m, :],
                                 start=(m == 0), stop=(m == 1))
            emb_c = wk.tile([128, 128], FP32, tag="emb_c")
            nc.vector.tensor_add(emb_c, ps_z, nmem[:, ch, :])
            nc.sync.dma_start(out=emb_out[ch * 128:(ch + 1) * 128, :], in_=emb_c)

    split_waits(nc)
    return nc


# ----------------------------------------------------------------------------
# host side
# ----------------------------------------------------------------------------

_CACHE = {}


def _route(L, src, dst, t):
    idx = np.concatenate([src, dst]).astype(np.int64)
    other = np.concatenate([dst, src]).astype(np.int64)
    tt = np.concatenate([t, t])
    eidx = np.concatenate([np.arange(len(src)), np.arange(len(src))])
    NW = L // 128
    order = np.argsort(idx, kind='stable')
    idx_s, other_s, tt_s, eidx_s = idx[order], other[order], tt[order], eidx[order]
    owner = idx_s // L
    cores = []
    for c in range(NCORES):
        msk = owner == c
        li = idx_s[msk] - c * L
        win = li // 128
        col = li % 128
        wcount = np.bincount(win, minlength=NW)
        assert wcount.max() <= 256, f"window overflow: {wcount.max()}"
        woff = np.zeros(NW + 1, np.int64)
        woff[1:] = np.cumsum(wcount)
        within = np.arange(len(li)) - woff[win]
        slot = win * 256 + within
        cores.append(dict(slot=slot, col=col, li=li, other=other_s[msk],
                          tt=tt_s[msk], eidx=eidx_s[msk]))
    return cores


def kernel(**inputs):
    node_memory = np.asarray(inputs['node_memory'])
    last_update = np.asarray(inputs['last_update'])
    node_features = np.asarray(inputs['node_features'])
    event_feat = np.asarray(inputs['event_feat'])
    t = np.asarray(inputs['t'])
    src = np.asarray(inputs['src']).astype(np.int64)
    dst = np.asarray(inputs['dst']).astype(np.int64)
    time_w = np.asarray(inputs['time_w'])
    time_b = np.asarray(inputs['time_b'])
    W_ih = np.asarray(inputs['W_ih'])
    b_ih = np.asarray(inputs['b_ih'])
    W_hh = np.asarray(inputs['W_hh'])
    b_hh = np.asarray(inputs['b_hh'])
    proj_W = np.asarray(inputs['proj_W'])
    proj_b = np.asarray(inputs['proj_b'])
    centroids = np.asarray(inputs['centroids'])

    Nn = node_memory.shape[0]
    GW = 7
    gran = 128 * GW * NCORES          # L must be multiple of 128*GW
    NP = -(-Nn // gran) * gran
    L = NP // NCORES
    SLOTS = 2 * L
    NW = L // 128

    nmp = np.zeros((NP, D), np.float32); nmp[:Nn] = node_memory
    nfp = np.zeros((NP, D), np.float32); nfp[:Nn] = node_features
    lup = np.zeros(NP, np.float32); lup[:Nn] = last_update

    idx_full = np.concatenate([src, dst])
    cnt_full = np.bincount(idx_full, minlength=NP).astype(np.float32)
    icnt_full = 1.0 / np.maximum(cnt_full, 1.0)
    has_full = (cnt_full > 0).astype(np.float32)

    cores = _route(L, src, dst, t)
    bsum_h = f32c(np.stack([(b_ih + b_hh)[0:128], (b_ih + b_hh)[128:256]], 1))
    wih_h = bfc(W_ih.T.reshape(4, 128, 384).transpose(1, 0, 2))

    in_maps = []
    for c in range(NCORES):
        r = cores[c]
        sl = r['slot']
        ev_mo = np.zeros((SLOTS, D), ml_dtypes.bfloat16)
        ev_ef = np.zeros((SLOTS, F), ml_dtypes.bfloat16)
        ev_dt = np.zeros(SLOTS, np.float32)
        ev_col = np.full(SLOTS, -1.0, np.float32)
        ev_icnt = np.zeros(SLOTS, np.float32)
        ev_mo[sl] = nmp[r['other']].astype(ml_dtypes.bfloat16)
        ev_ef[sl] = event_feat[r['eidx']].astype(ml_dtypes.bfloat16)
        ev_dt[sl] = r['tt'] - lup[r['li'] + c * L]
        ev_col[sl] = r['col'].astype(np.float32)
        ev_icnt[sl] = icnt_full[r['li'] + c * L]
        nsl = slice(c * L, (c + 1) * L)
        in_maps.append({
            'memT': f32c(nmp[nsl].T),
            'mem_node': f32c(nmp[nsl]),
            'nfT': f32c(nfp[nsl].T),
            'has_colT': f32c(has_full[nsl].reshape(NW, 128).T),
            'ev_mo': ev_mo, 'ev_ef': ev_ef, 'ev_dt': ev_dt,
            'ev_col': ev_col, 'ev_icnt': ev_icnt,
            'W_ihT': wih_h,
            'W_hhT': bfc(W_hh.T),
            'bsum': bsum_h,
            'b_hh2': f32c(b_hh[256:384].reshape(128, 1)),
            'b_ih2': f32c(b_ih[256:384].reshape(128, 1)),
            'pWt': bfc(proj_W),
            'pb': f32c(proj_b.reshape(128, 1)),
            'cenT': f32c(centroids.T),
            'w_rep': f32c(np.tile(time_w[None, :], (128, 1))),
            'bpi_rep': f32c(np.tile(time_b[None, :] + HALF_PI, (128, 1))),
            'iota_t': f32c(np.tile(np.arange(128, dtype=np.float32)[None, :],
                                   (128, 1))),
            'core_oh_in': f32c(np.tile(np.eye(NCORES, dtype=np.float32)[c][None, :],
                                       (128, 1))),
        })

    debug = bool(int(os.environ.get("KERNEL_DEBUG", "0")))
    key = (L, debug)
    if key not in _CACHE:
        _CACHE[key] = build_program(L, debug=debug)
    nc = _CACHE[key]
    trace = bool(int(os.environ.get("KERNEL_TRACE", "0")))
    res = run_bass_kernel_spmd(nc, in_maps, list(range(NCORES)), trace=trace)
    emb = np.concatenate([res.results[c]['emb'] for c in range(NCORES)], 0)
    kernel._last_exec_ns = getattr(res, 'exec_time_ns', None)
    kernel._last_profile = getattr(res, 'profile_json', None)
    if debug:
        kernel._last_results = res.results
    return emb[:Nn].astype(np.float32)

